# revision 1
# baseline (speedup 1.0000x reference)
"""DGCNN (gnn_message_passing) Trainium2 Bass kernel, v2.

Strategy (data-parallel over graphs, 8 graphs per NeuronCore):
  - Host builds, per graph, the dense normalized propagation operator
    S[d, s] = (mult(s->d) + I) / deg_out[d]  (512x512 f32), shipped
    transposed as 4 chunks of [128, 512].
  - Layer-1 linear is folded on the host: LIN1 = (emb @ W1)[x] is shipped
    instead of h0 (flag FOLD_LIN1; layer-1 lin matmuls are skipped).
  - Graphs processed in pairs, graph-outer: each pair runs its 4 GCN
    layers on PE while the previous pair's sort-pooling tail runs on
    DVE/ACT/Pool, so the tail is hidden under PE time.
  - Per layer+graph on device: lin = h @ W as 4 chunk matmuls into one
    [128,512] PSUM bank, one copy to SBUF, then msgT = lin^T-chunks
    stationary x S^T chunks -> [128f, 512d] PSUM, tanh -> h (f32
    throughout: the top-64 sort is sensitive to ~1e-8 noise in h5, so
    the whole h chain must be bit-stable f32).
  - Sort-pooling tail per graph: exact stable ranks via comparison
    matrices (DVE/Pool alternating), one-hot rank matrix PT, top-64
    node indices extracted with a tiny iota matmul, converted to the
    wrapped int16 layout, and the pooled features gathered from h with
    gpsimd ap_gather (Pool engine) - no PE transposes or selection
    matmuls.
  - Conv1/maxpool/conv2/dense head batched across the 8 graphs.

Self-contained: hardcodes all shapes; no reads of /root/problem files.
"""

import sys

if "/opt/trn_rl_repo" not in sys.path:
    sys.path.insert(0, "/opt/trn_rl_repo")

import numpy as np

import concourse.bacc as bacc
import concourse.mybir as mybir
import concourse.tile as tile
from concourse.bass_utils import run_bass_kernel_spmd

F32 = mybir.dt.float32
I16 = mybir.dt.int16

NUM_GRAPHS = 64
NPG = 512  # nodes per graph
N_TOTAL = NUM_GRAPHS * NPG
EMB = 128
DIMF = 128
NLAYERS = 4
K = 64
NCORES = 8
GPC = NUM_GRAPHS // NCORES  # graphs per core = 8
NLOC = GPC * NPG  # local nodes = 4096
LATENT = NLAYERS * DIMF + 1  # 513
DD = (K - 2) // 2 + 1  # 32
CONV2_LEN = DD - 5 + 1  # 28

_NC_CACHE = {}


def _build(fold_lin1, debug):
    """Trace + compile the per-core Bass program (same on all 8 cores)."""
    nc = bacc.Bacc("TRN2", target_bir_lowering=False, debug=False,
                   num_devices=NCORES)

    # ---- per-core DRAM I/O ----
    # LIN1M[g] = (emb @ W1)[x] for graph g, chunk-major: [g][p][cc*128+f]
    #          = lin1[node cc*128+p of graph g, feat f]
    if fold_lin1:
        LIN1M = nc.dram_tensor("LIN1M", [GPC, 128, NPG], F32,
                               kind="ExternalInput")
    else:
        H0T = nc.dram_tensor("H0T", [128, NLOC], F32, kind="ExternalInput")
    STD = nc.dram_tensor("STD", [GPC, 4, 128, NPG], F32, kind="ExternalInput")
    WC = nc.dram_tensor("WC", [NLAYERS, 128, 128], F32, kind="ExternalInput")
    W5 = nc.dram_tensor("W5", [128, 1], F32, kind="ExternalInput")
    IDN = nc.dram_tensor("IDN", [128, 128], F32, kind="ExternalInput")
    W1A = nc.dram_tensor("W1A", [4, 128, 16], F32, kind="ExternalInput")
    W1B = nc.dram_tensor("W1B", [1, 16], F32, kind="ExternalInput")
    W2T = nc.dram_tensor("W2T", [5, 16, 32], F32, kind="ExternalInput")
    D1R = nc.dram_tensor("D1R", [32, CONV2_LEN * 32], F32, kind="ExternalInput")
    D2 = nc.dram_tensor("D2", [32, 2], F32, kind="ExternalInput")
    DIFF = nc.dram_tensor("DIFF", [2, 2], F32, kind="ExternalInput")
    B1 = nc.dram_tensor("B1", [16, 1], F32, kind="ExternalInput")
    B2 = nc.dram_tensor("B2", [32, 1], F32, kind="ExternalInput")
    BD1 = nc.dram_tensor("BD1", [32, 1], F32, kind="ExternalInput")
    BD2 = nc.dram_tensor("BD2", [2, 1], F32, kind="ExternalInput")
    KI = nc.dram_tensor("KI", [128, K], F32, kind="ExternalInput")
    MJ = nc.dram_tensor("MJ", [4, 128, NPG], F32, kind="ExternalInput")
    IOTA = nc.dram_tensor("IOTA", [128, 4], F32, kind="ExternalInput")
    QMASK = nc.dram_tensor("QMASK", [64, 4], F32, kind="ExternalInput")
    E1M = nc.dram_tensor("E1M", [64, 16], F32, kind="ExternalInput")
    RM = nc.dram_tensor("RM", [16, 128], F32, kind="ExternalInput")
    OUT = nc.dram_tensor("OUT", [2, GPC], F32, kind="ExternalOutput")
    if debug:
        DBG_H = nc.dram_tensor("DBG_H", [NLAYERS, 128, NLOC], F32,
                               kind="ExternalOutput")
        DBG_H5 = nc.dram_tensor("DBG_H5", [GPC, 1, NPG], F32,
                                kind="ExternalOutput")
        DBG_RANK = nc.dram_tensor("DBG_RANK", [GPC, 128, 4], F32,
                                  kind="ExternalOutput")
        DBG_IDX = nc.dram_tensor("DBG_IDX", [GPC, 128, 4], F32,
                                 kind="ExternalOutput")
        DBG_POOL = nc.dram_tensor("DBG_POOL", [NLAYERS, 128, GPC * K], F32,
                                  kind="ExternalOutput")
        DBG_POOL5 = nc.dram_tensor("DBG_POOL5", [1, GPC * K], F32,
                                   kind="ExternalOutput")
        DBG_Y2 = nc.dram_tensor("DBG_Y2", [32, GPC * CONV2_LEN], F32,
                                kind="ExternalOutput")

    TANH = mybir.ActivationFunctionType.Tanh
    RELU = mybir.ActivationFunctionType.Relu
    SIGM = mybir.ActivationFunctionType.Sigmoid
    ADD = mybir.AluOpType.add
    MULT = mybir.AluOpType.mult
    MAX = mybir.AluOpType.max
    IS_GT = mybir.AluOpType.is_gt
    IS_EQ = mybir.AluOpType.is_equal

    with tile.TileContext(nc) as tc:
        with (
            tc.tile_pool(name="const", bufs=1) as cp,
            tc.tile_pool(name="hg", bufs=6) as hp,          # [128,2048]/graph
            tc.tile_pool(name="st", bufs=32) as stp,        # [128,512]/chunk
            tc.tile_pool(name="lin", bufs=4) as linp,       # [128,512]
            tc.tile_pool(name="sc", bufs=3) as scp,         # compare scratch
            tc.tile_pool(name="vbp", bufs=3) as vbp,        # v broadcast
            tc.tile_pool(name="ptp", bufs=2) as ptp,        # ptt one-hots
            tc.tile_pool(name="sm", bufs=4) as smp,        # small tiles
            tc.tile_pool(name="idx", bufs=3) as idxp_pool,  # idx wrapped
            tc.tile_pool(name="xs", bufs=20) as xsp,        # endgame sel xts
            tc.tile_pool(name="ps512", bufs=2, space="PSUM") as ps5,
            tc.tile_pool(name="ps128", bufs=2, space="PSUM") as ps1,
            tc.tile_pool(name="psy1", bufs=1, space="PSUM") as psy,
        ):
            # ---- load order: pair-0 working set first ----
            lin1_sb = []

            def load_lin1(g, chunked=False):
                if fold_lin1:
                    t = linp.tile([128, NPG], F32, tag="lin1", bufs=GPC,
                                  name=f"lin1_{g}")
                    if chunked:
                        for c in range(4):
                            nc.sync.dma_start(
                                t[:, c * 128:(c + 1) * 128],
                                LIN1M[g, :, c * 128:(c + 1) * 128])
                    else:
                        nc.sync.dma_start(t[:], LIN1M[g, :, :])
                    return t
                return None

            st_sb = {}

            def load_st(g):
                chunks = []
                for c in range(4):
                    t = stp.tile([128, NPG], F32, tag="st", bufs=32,
                                 name=f"st_t{g}_{c}")
                    nc.sync.dma_start(t[:], STD[g, c, :, :])
                    chunks.append(t)
                st_sb[g] = chunks

            if fold_lin1:
                st00 = stp.tile([128, NPG], F32, tag="st", bufs=32,
                                name="st_t0_0")
                nc.sync.dma_start(st00[:], STD[0, 0, :, :])
                lin1_sb = [load_lin1(0, chunked=True)]
                rest0 = []
                for c in range(1, 4):
                    t = stp.tile([128, NPG], F32, tag="st", bufs=32,
                                 name=f"st_t0_{c}")
                    nc.sync.dma_start(t[:], STD[0, c, :, :])
                    rest0.append(t)
                st_sb[0] = [st00] + rest0
                lin1_sb.append(load_lin1(1))
                load_st(1)
            else:
                h0 = hp.tile([128, NLOC], F32, tag="h0x", bufs=1)
                for s in range(4):
                    nc.sync.dma_start(h0[:, s * 512:(s + 1) * 512],
                                      H0T[:, s * 512:(s + 1) * 512])
                load_st(0)
                load_st(1)
            wc_sb = cp.tile([128, NLAYERS * 128], F32, tag="wc")
            l_lo = 1 if fold_lin1 else 0
            for l in range(l_lo, NLAYERS):
                nc.sync.dma_start(wc_sb[:, l * 128:(l + 1) * 128], WC[l, :, :])
            w5_sb = cp.tile([128, 1], F32, tag="w5")
            nc.sync.dma_start(w5_sb[:], W5[:])
            id_sb = cp.tile([128, 128], F32, tag="idn")
            nc.sync.dma_start(id_sb[:], IDN[:])
            if fold_lin1:
                lin1_sb.append(load_lin1(2))
                load_st(2)
            else:
                for s in range(4, 8):
                    nc.sync.dma_start(h0[:, s * 512:(s + 1) * 512],
                                      H0T[:, s * 512:(s + 1) * 512])
                load_st(2)
            mj_sb = cp.tile([128, 4 * NPG], F32, tag="mj")
            for c in range(4):
                nc.sync.dma_start(mj_sb[:, c * NPG:(c + 1) * NPG], MJ[c, :, :])
            ki_sb = cp.tile([128, K], F32, tag="ki")
            nc.sync.dma_start(ki_sb[:], KI[:])
            iota_sb = cp.tile([128, 4], F32, tag="iota")
            nc.sync.dma_start(iota_sb[:], IOTA[:])
            qm_sb = cp.tile([64, 4], F32, tag="qm")
            nc.sync.dma_start(qm_sb[:], QMASK[:])
            e1_sb = cp.tile([64, 16], F32, tag="e1")
            nc.sync.dma_start(e1_sb[:], E1M[:])
            r_sb = cp.tile([16, 128], F32, tag="rm")
            nc.sync.dma_start(r_sb[:], RM[:])
            w1a_sb = cp.tile([128, 64], F32, tag="w1a")
            for c in range(4):
                nc.sync.dma_start(w1a_sb[:, c * 16:(c + 1) * 16], W1A[c, :, :])
            w1b_sb = cp.tile([1, 16], F32, tag="w1b")
            nc.sync.dma_start(w1b_sb[:], W1B[:])
            w2_sb = cp.tile([16, 160], F32, tag="w2t")
            for t5 in range(5):
                nc.sync.dma_start(w2_sb[:, t5 * 32:(t5 + 1) * 32],
                                  W2T[t5, :, :])
            d1_sb = cp.tile([32, CONV2_LEN * 32], F32, tag="d1r")
            nc.sync.dma_start(d1_sb[:], D1R[:])
            d2_sb = cp.tile([32, 2], F32, tag="d2")
            nc.sync.dma_start(d2_sb[:], D2[:])
            diff_sb = cp.tile([2, 2], F32, tag="diff")
            nc.sync.dma_start(diff_sb[:], DIFF[:])
            b1_sb = cp.tile([16, 1], F32, tag="b1")
            nc.sync.dma_start(b1_sb[:], B1[:])
            b2_sb = cp.tile([32, 1], F32, tag="b2")
            nc.sync.dma_start(b2_sb[:], B2[:])
            bd1_sb = cp.tile([32, 1], F32, tag="bd1")
            nc.sync.dma_start(bd1_sb[:], BD1[:])
            bd2_sb = cp.tile([2, 1], F32, tag="bd2")
            nc.sync.dma_start(bd2_sb[:], BD2[:])
            for g in range(3, GPC):
                if fold_lin1:
                    lin1_sb.append(load_lin1(g))
                load_st(g)

            # per-graph state
            hgs = {}       # g -> [128, 4*512] tile (layers 1..4)
            vcols = {}     # g -> [128, 4] node-major h5
            vbs = {}       # g -> [128, 512] v broadcast
            ranks = {}     # g -> [128, 4]
            ptts = {}      # g -> [128, 4K] one-hot rank matrix
            idxw = {}      # g -> [128, 4] int16 wrapped indices
            # pooled features, all graphs side by side, one tile per layer
            pooled_sb = [cp.tile([128, GPC * K], F32, tag=f"pool{l}",
                                 name=f"pool{l}")
                         for l in range(NLAYERS)]
            p5all = cp.tile([16, GPC * K], F32, tag="p5all")
            y1p = psy.tile([16, GPC * K], F32, tag="y1p")
            y2all = cp.tile([32, GPC * CONV2_LEN], F32, tag="y2all")
            y1 = cp.tile([16, GPC * K], F32, tag="y1")
            mp = cp.tile([16, GPC * K // 2], F32, tag="mp")

            # ---------------- layer machinery ----------------
            def lin_stage(g, l):
                # returns SBUF [128, 4*128] chunk-major lin
                if l == 0 and fold_lin1:
                    return lin1_sb[g]
                lp = ps5.tile([128, NPG], F32, tag="linp", bufs=2)
                for cc in range(4):
                    if l == 0:
                        stat = h0[:, g * NPG + cc * 128:
                                  g * NPG + (cc + 1) * 128]
                    else:
                        stat = hgs[g][:, (l - 1) * NPG + cc * 128:
                                      (l - 1) * NPG + (cc + 1) * 128]
                    nc.tensor.matmul(
                        lp[:, cc * 128:(cc + 1) * 128], stat,
                        wc_sb[:, l * 128:(l + 1) * 128],
                        start=True, stop=True)
                ln = linp.tile([128, NPG], F32, tag="lin")
                if (g + l) % 2 == 0:
                    nc.vector.tensor_copy(ln[:], lp[:])
                else:
                    nc.scalar.copy(ln[:], lp[:])
                return ln

            def prop_stage(g, l, ln):
                sp = ps5.tile([128, NPG], F32, tag="msgp", bufs=2)
                for cc in range(4):
                    nc.tensor.matmul(
                        sp[:], ln[:, cc * 128:(cc + 1) * 128],
                        st_sb[g][cc][:],
                        start=(cc == 0), stop=(cc == 3))
                nc.scalar.activation(
                    hgs[g][:, l * NPG:(l + 1) * NPG], sp[:], TANH)

            def alloc_h(g):
                hgs[g] = hp.tile([128, NLAYERS * NPG], F32, tag="hg",
                                 name=f"h_{g}")

            def layers_pair(ga, gb):
                """All 4 GCN layers for graphs ga, gb, interleaved."""
                alloc_h(ga)
                alloc_h(gb)
                for l in range(NLAYERS):
                    lns = {g: lin_stage(g, l) for g in (ga, gb)}
                    for g in (ga, gb):
                        prop_stage(g, l, lns[g])
                    yield l

            # ---------------- tail stages ----------------
            lin5s = {}

            def sA(g):
                """layer-5 matvec (PE) + copy (DVE)."""
                hl = hgs[g]
                l5p = ps1.tile([128, 4], F32, tag="ps128")
                for cc in range(4):
                    nc.tensor.matmul(
                        l5p[:, cc:cc + 1],
                        hl[:, 3 * NPG + cc * 128:3 * NPG + (cc + 1) * 128],
                        w5_sb[:], start=True, stop=True)
                lin5 = smp.tile([128, 4], F32, tag="lin5")
                nc.vector.tensor_copy(lin5[:], l5p[:])
                lin5s[g] = lin5

            def sB(g):
                """msg5 = S @ lin5 (PE, 16 tiny) + tanh (ACT)."""
                lin5 = lin5s[g]
                m5p = ps1.tile([128, 4], F32, tag="ps128")
                for dc in range(4):
                    for sc in range(4):
                        nc.tensor.matmul(
                            m5p[:, dc:dc + 1],
                            st_sb[g][sc][:, dc * 128:(dc + 1) * 128],
                            lin5[:, sc:sc + 1],
                            start=(sc == 0), stop=(sc == 3))
                vcol = smp.tile([128, 4], F32, tag="vcol")
                nc.scalar.activation(vcol[:], m5p[:], TANH)
                vcols[g] = vcol

            def sC(g):
                """h5 row form + broadcast."""
                vcol = vcols[g]
                vtp = ps1.tile([4, 128], F32, tag="ps128")
                nc.tensor.transpose(vtp[:], vcol[:], id_sb[:])
                vts = smp.tile([4, 128], F32, tag="vts")
                nc.vector.tensor_copy(vts[:], vtp[:])
                h5r = smp.tile([1, NPG], F32, tag="h5r", bufs=3)
                for cc in range(4):
                    nc.sync.dma_start(h5r[0:1, cc * 128:(cc + 1) * 128],
                                      vts[cc:cc + 1, :])
                vb = vbp.tile([128, NPG], F32, tag="vb")
                nc.gpsimd.partition_broadcast(vb[:], h5r[0:1, :])
                vbs[g] = vb
                if debug:
                    nc.sync.dma_start(DBG_H5[g, :, :], h5r[:])

            def sD(g, dve_chunks=None):
                """exact stable ranks; dve_chunks picks per-chunk engine."""
                vb, vcol = vbs[g], vcols[g]
                if dve_chunks is None:
                    dve_chunks = (0, 1, 2, 3)
                rank = smp.tile([128, 4], F32, tag="rank")
                for cc in range(4):
                    eng = nc.vector if cc in dve_chunks else nc.gpsimd
                    t1 = scp.tile([128, NPG], F32, tag="tt")
                    ra = smp.tile([128, 2], F32, tag="ra")
                    eng.tensor_scalar(
                        out=t1[:], in0=vb[:], scalar1=vcol[:, cc:cc + 1],
                        scalar2=None, op0=IS_GT, op1=ADD,
                        accum_out=ra[:, 0:1])
                    t2 = scp.tile([128, NPG], F32, tag="tt")
                    eng.scalar_tensor_tensor(
                        out=t2[:], in0=vb[:], scalar=vcol[:, cc:cc + 1],
                        in1=mj_sb[:, cc * NPG:(cc + 1) * NPG],
                        op0=IS_EQ, op1=MULT, accum_out=ra[:, 1:2])
                    nc.vector.tensor_tensor(
                        out=rank[:, cc:cc + 1], in0=ra[:, 0:1],
                        in1=ra[:, 1:2], op=ADD)
                ranks[g] = rank
                if debug:
                    nc.sync.dma_start(DBG_RANK[g, :, :], rank[:])

            def sE(g):
                """one-hot rank matrix (columns in wrapped-permuted order)."""
                rank = ranks[g]
                ptt = ptp.tile([128, 4 * K], F32, tag="pt")
                for cc in range(4):
                    nc.vector.tensor_scalar(
                        out=ptt[:, cc * K:(cc + 1) * K], in0=ki_sb[:],
                        scalar1=rank[:, cc:cc + 1], scalar2=None, op0=IS_EQ)
                ptts[g] = ptt

            def sF(g):
                """ordered top-64 node indices, int16 wrapped for ap_gather."""
                ptt = ptts[g]
                # col64[q] = index of the node with rank perm(q); KI's
                # permutation makes the downstream folds land each index at
                # iw[p, s] = idx[s*16+p], the ap_gather wrapped layout.
                cxp = ps1.tile([K, 1], F32, tag="ps128")
                for cc in range(4):
                    nc.tensor.matmul(cxp[:], ptt[:, cc * K:(cc + 1) * K],
                                     iota_sb[:, cc:cc + 1],
                                     start=(cc == 0), stop=(cc == 3))
                c64 = smp.tile([K, 1], F32, tag="c64")
                nc.vector.tensor_copy(c64[:], cxp[:])
                m64 = smp.tile([K, 4], F32, tag="m64")
                nc.vector.tensor_scalar(out=m64[:], in0=qm_sb[:],
                                        scalar1=c64[:, 0:1], scalar2=None,
                                        op0=MULT)
                wqp = ps1.tile([16, 4], F32, tag="ps128")
                nc.tensor.matmul(wqp[:], e1_sb[:], m64[:],
                                 start=True, stop=True)
                wq = smp.tile([16, 4], F32, tag="wq")
                nc.vector.tensor_copy(wq[:], wqp[:])
                wfp = ps1.tile([128, 4], F32, tag="ps128")
                nc.tensor.matmul(wfp[:], r_sb[:], wq[:],
                                 start=True, stop=True)
                iw = idxp_pool.tile([128, 4], I16, tag="iw")
                nc.vector.tensor_copy(iw[:], wfp[:])
                idxw[g] = iw
                if debug:
                    dbgi = smp.tile([128, 4], F32, tag="dbgi")
                    nc.vector.tensor_copy(dbgi[:], iw[:])
                    nc.sync.dma_start(DBG_IDX[g, :, :], dbgi[:])

            def sG(g, pool5_on_pe=False):
                """gather pooled features on the Pool engine."""
                iw = idxw[g]
                hl = hgs[g]
                for l in range(NLAYERS):
                    nc.gpsimd.ap_gather(
                        pooled_sb[l][:, g * K:(g + 1) * K],
                        hl[:, l * NPG:(l + 1) * NPG], iw[:],
                        channels=128, num_elems=NPG, d=1, num_idxs=K)
                if pool5_on_pe:
                    # v[idx] via selection matmul; undo the column
                    # permutation with a strided copy out of PSUM.
                    vcol, ptt = vcols[g], ptts[g]
                    p5p = ps1.tile([1, K], F32, tag="ps128")
                    for cc in range(4):
                        nc.tensor.matmul(p5p[:], vcol[:, cc:cc + 1],
                                         ptt[:, cc * K:(cc + 1) * K],
                                         start=(cc == 0), stop=(cc == 3))
                    dstv = p5all[0:1, g * K:(g + 1) * K].rearrange(
                        "a (s p) -> a p s", p=16)
                    srcv = p5p[0:1, :].rearrange("a (p s) -> a p s", s=4)
                    nc.vector.tensor_copy(dstv, srcv)
                else:
                    nc.gpsimd.ap_gather(
                        p5all[:, g * K:(g + 1) * K], vbs[g][0:16, :],
                        iw[0:16, :], channels=16, num_elems=NPG, d=1,
                        num_idxs=K)

            xts = {}

            def sSelXt(g, alt_eng=None):
                """node-major h chunks via PE transpose (endgame only);
                4 chunk transposes land in one PSUM tile -> one wide copy."""
                hl = hgs[g]
                lst = []
                for l in range(NLAYERS):
                    tg = "linp" if l % 2 == 0 else "msgp"
                    xp = ps5.tile([128, NPG], F32, tag=tg, bufs=2)
                    for cc in range(4):
                        nc.tensor.transpose(
                            xp[:, cc * 128:(cc + 1) * 128],
                            hl[:, l * NPG + cc * 128:
                               l * NPG + (cc + 1) * 128], id_sb[:])
                    xt = xsp.tile([128, NPG], F32, tag="xtb", bufs=6)
                    if alt_eng is not None and l % 2 == 1:
                        alt_eng.tensor_copy(xt[:], xp[:])
                    else:
                        nc.scalar.copy(xt[:], xp[:])
                    lst.append(xt)
                xts[g] = lst

            def sSelMM(g):
                """selection matmuls; ptt columns are permuted, so the
                copies to pooled_sb/p5all unpermute via strided views."""
                ptt, vcol = ptts[g], vcols[g]
                for l in range(NLAYERS):
                    tg = "linp" if l % 2 == 0 else "msgp"
                    pp = ps5.tile([128, K], F32, tag=tg, bufs=2)
                    for cc in range(4):
                        nc.tensor.matmul(pp[:],
                                         xts[g][l][:, cc * 128:(cc + 1) * 128],
                                         ptt[:, cc * K:(cc + 1) * K],
                                         start=(cc == 0), stop=(cc == 3))
                    dstv = pooled_sb[l][:, g * K:(g + 1) * K].rearrange(
                        "c (s p) -> c p s", p=16)
                    srcv = pp[:].rearrange("c (p s) -> c p s", s=4)
                    nc.scalar.copy(dstv, srcv)
                p5p = ps1.tile([1, K], F32, tag="ps128")
                for cc in range(4):
                    nc.tensor.matmul(p5p[:], vcol[:, cc:cc + 1],
                                     ptt[:, cc * K:(cc + 1) * K],
                                     start=(cc == 0), stop=(cc == 3))
                dstv = p5all[0:1, g * K:(g + 1) * K].rearrange(
                    "a (s p) -> a p s", p=16)
                srcv = p5p[0:1, :].rearrange("a (p s) -> a p s", s=4)
                nc.vector.tensor_copy(dstv, srcv)

            def sHa(g):
                """conv1 + relu + maxpool for graph g."""
                for l in range(NLAYERS):
                    nc.tensor.matmul(y1p[:, g * K:(g + 1) * K],
                                     w1a_sb[:, l * 16:(l + 1) * 16],
                                     pooled_sb[l][:, g * K:(g + 1) * K],
                                     start=(l == 0), stop=False)
                nc.tensor.matmul(y1p[:, g * K:(g + 1) * K], w1b_sb[:],
                                 p5all[0:1, g * K:(g + 1) * K],
                                 start=False, stop=True)
                nc.scalar.activation(y1[:, g * K:(g + 1) * K],
                                     y1p[:, g * K:(g + 1) * K], RELU,
                                     bias=b1_sb[:, 0:1])
                y1v = y1[:, g * K:(g + 1) * K].rearrange(
                    "p (a b) -> p a b", b=2)
                nc.vector.tensor_tensor(
                    out=mp[:, g * DD:(g + 1) * DD], in0=y1v[:, :, 0:1],
                    in1=y1v[:, :, 1:2], op=MAX)

            def sHb(g):
                """conv2 + relu for graph g."""
                y2p = ps1.tile([32, CONV2_LEN], F32, tag="ps128")
                for t5 in range(5):
                    nc.tensor.matmul(
                        y2p[:],
                        w2_sb[:, t5 * 32:(t5 + 1) * 32],
                        mp[:, g * DD + t5:g * DD + t5 + CONV2_LEN],
                        start=(t5 == 0), stop=(t5 == 4))
                nc.scalar.activation(
                    y2all[:, g * CONV2_LEN:(g + 1) * CONV2_LEN], y2p[:],
                    RELU, bias=b2_sb[:, 0:1])

            def sH(g):
                """per-graph conv1 + relu + maxpool + conv2 + relu."""
                for l in range(NLAYERS):
                    nc.tensor.matmul(y1p[:, g * K:(g + 1) * K],
                                     w1a_sb[:, l * 16:(l + 1) * 16],
                                     pooled_sb[l][:, g * K:(g + 1) * K],
                                     start=(l == 0), stop=False)
                nc.tensor.matmul(y1p[:, g * K:(g + 1) * K], w1b_sb[:],
                                 p5all[0:1, g * K:(g + 1) * K],
                                 start=False, stop=True)
                nc.scalar.activation(y1[:, g * K:(g + 1) * K],
                                     y1p[:, g * K:(g + 1) * K], RELU,
                                     bias=b1_sb[:, 0:1])
                y1v = y1[:, g * K:(g + 1) * K].rearrange(
                    "p (a b) -> p a b", b=2)
                nc.vector.tensor_tensor(
                    out=mp[:, g * DD:(g + 1) * DD], in0=y1v[:, :, 0:1],
                    in1=y1v[:, :, 1:2], op=MAX)
                y2p = ps1.tile([32, CONV2_LEN], F32, tag="ps128")
                for t5 in range(5):
                    nc.tensor.matmul(
                        y2p[:],
                        w2_sb[:, t5 * 32:(t5 + 1) * 32],
                        mp[:, g * DD + t5:g * DD + t5 + CONV2_LEN],
                        start=(t5 == 0), stop=(t5 == 4))
                nc.scalar.activation(
                    y2all[:, g * CONV2_LEN:(g + 1) * CONV2_LEN], y2p[:],
                    RELU, bias=b2_sb[:, 0:1])


            # ---------------- schedule ----------------
            # pair p: own sA at l3; pair p-1 runs B,C,D,E at l0..l3;
            # pair p-2 runs F,G,H at l0..l2.
            NP = GPC // 2
            for p in range(NP):
                ga, gb = 2 * p, 2 * p + 1
                gen = layers_pair(ga, gb)
                for l in gen:
                    g1 = (2 * (p - 1), 2 * (p - 1) + 1) if p >= 1 else ()
                    g2 = (2 * (p - 2), 2 * (p - 2) + 1) if p >= 2 else ()
                    if l == 0:
                        for g in g1:
                            sB(g)
                        for g in g2:
                            sF(g)
                    elif l == 1:
                        for g in g1:
                            sC(g)
                        for g in g2:
                            sG(g)
                    elif l == 2:
                        for g in g1:
                            sD(g)
                    elif l == 3:
                        for g in g1:
                            sE(g)
                        if p < NP - 1:
                            for g in g2:
                                sH(g)
                            sA(ga)
                            sA(gb)
                        else:
                            # critical tail chain of the last pair jumps
                            # ahead of the non-critical conv heads in the
                            # ACT/DVE queues
                            sA(ga)
                            sA(gb)
                            sB(ga)
                            sB(gb)
                            sC(ga)
                            sC(gb)
                            for g in g2:
                                sH(g)
                            sSelXt(ga)
            # endgame: pair NP-2 needs F,G,H; pair NP-1 needs D..H
            # (its A-C stages were hoisted into the last layer slot).
            q2 = (2 * (NP - 2), 2 * (NP - 2) + 1)
            q3 = (2 * (NP - 1), 2 * (NP - 1) + 1)
            sD(q3[0])
            sE(q3[0])
            sD(q3[1])
            sE(q3[1])
            sF(q2[0])
            sF(q2[1])
            sG(q2[0])
            sG(q2[1])
            sSelMM(q3[0])
            sSelXt(q3[1], alt_eng=nc.vector)
            sH(q2[0])
            sH(q2[1])
            sSelMM(q3[1])
            sHa(q3[0])
            sHa(q3[1])
            sHb(q3[0])
            sHb(q3[1])

            if debug:
                for l in range(NLAYERS):
                    for g in range(GPC):
                        nc.sync.dma_start(
                            DBG_H[l, :, g * NPG:(g + 1) * NPG],
                            hgs[g][:, l * NPG:(l + 1) * NPG])
                    nc.sync.dma_start(DBG_POOL[l, :, :], pooled_sb[l][:])
                nc.sync.dma_start(DBG_POOL5[:], p5all[0:1, :])

            # ---------------- head (batched over graphs) ----------------
            y1 = smp.tile([16, GPC * K], F32, tag="y1", bufs=1)
            nc.scalar.activation(y1[:], y1p[:], RELU, bias=b1_sb[:, 0:1])
            mp = smp.tile([16, GPC * K // 2], F32, tag="mp", bufs=1)
            y1v = y1[:].rearrange("p (a b) -> p a b", b=2)
            nc.vector.tensor_tensor(out=mp[:], in0=y1v[:, :, 0:1],
                                    in1=y1v[:, :, 1:2], op=MAX)
            y2p = ps1.tile([32, GPC * CONV2_LEN], F32, tag="y2p", bufs=1)
            for g in range(GPC):
                for t5 in range(5):
                    nc.tensor.matmul(
                        y2p[:, g * CONV2_LEN:(g + 1) * CONV2_LEN],
                        w2_sb[:, t5 * 32:(t5 + 1) * 32],
                        mp[:, g * DD + t5:g * DD + t5 + CONV2_LEN],
                        start=(t5 == 0), stop=(t5 == 4))
            nc.scalar.activation(y2all[:], y2p[:], RELU, bias=b2_sb[:, 0:1])
            if debug:
                nc.sync.dma_start(DBG_Y2[:], y2all[:])

            h1p = ps1.tile([32, GPC], F32, tag="ps128")
            y2v = y2all[:].rearrange("p (g t) -> p g t", t=CONV2_LEN)
            for t5 in range(CONV2_LEN):
                nc.tensor.matmul(h1p[:], d1_sb[:, t5 * 32:(t5 + 1) * 32],
                                 y2v[:, :, t5:t5 + 1],
                                 start=(t5 == 0), stop=(t5 == CONV2_LEN - 1))
            h1s = smp.tile([32, GPC], F32, tag="h1s", bufs=1)
            nc.scalar.activation(h1s[:], h1p[:], RELU, bias=bd1_sb[:, 0:1])
            dfp = ps1.tile([2, GPC], F32, tag="ps128")
            nc.tensor.matmul(dfp[:], d2_sb[:], h1s[:], start=True, stop=True)
            pr = smp.tile([2, GPC], F32, tag="pr", bufs=1)
            nc.scalar.activation(pr[:], dfp[:], SIGM, bias=bd2_sb[:, 0:1])
            nc.sync.dma_start(OUT[:], pr[:])

    nc.compile()
    return nc


def _get_nc(fold_lin1, debug):
    key = (fold_lin1, debug)
    if key not in _NC_CACHE:
        _NC_CACHE[key] = _build(fold_lin1, debug)
    return _NC_CACHE[key]


def prepare_host(inputs, fold_lin1=True):
    """All host-side index preprocessing + per-core input maps."""
    x = np.asarray(inputs["x"]).astype(np.int64)
    edge_index = np.asarray(inputs["edge_index"]).astype(np.int64)
    emb = np.ascontiguousarray(np.asarray(inputs["emb"], dtype=np.float32))
    W_convs = np.asarray(inputs["W_convs"], dtype=np.float32)
    conv1_w = np.asarray(inputs["conv1_w"], dtype=np.float32)
    conv1_b = np.asarray(inputs["conv1_b"], dtype=np.float32)
    conv2_w = np.asarray(inputs["conv2_w"], dtype=np.float32)
    conv2_b = np.asarray(inputs["conv2_b"], dtype=np.float32)
    d1_w = np.asarray(inputs["d1_w"], dtype=np.float32)
    d1_b = np.asarray(inputs["d1_b"], dtype=np.float32)
    d2_w = np.asarray(inputs["d2_w"], dtype=np.float32)
    d2_b = np.asarray(inputs["d2_b"], dtype=np.float32)
    W_last = np.asarray(inputs["W_last"], dtype=np.float32)

    src, dst = edge_index[0], edge_index[1]
    deg = (np.bincount(src, minlength=N_TOTAL) + 1).astype(np.float32)
    invdeg = (np.float32(1.0) / deg).astype(np.float32)
    gid = dst >> 9
    flat = (gid * NPG + (dst & 511)) * NPG + (src & 511)
    A = np.bincount(flat, minlength=NUM_GRAPHS * NPG * NPG)
    A = A.astype(np.float32).reshape(NUM_GRAPHS, NPG, NPG)
    idx = np.arange(NPG)
    A[:, idx, idx] += 1.0
    S = A * invdeg.reshape(NUM_GRAPHS, NPG, 1)
    ST = np.ascontiguousarray(S.transpose(0, 2, 1)).reshape(
        NUM_GRAPHS, 4, 128, NPG)

    w1 = np.ascontiguousarray(conv1_w[:, 0, :].T)  # [513, 16]
    shared = {
        "WC": np.ascontiguousarray(W_convs),
        "W5": np.ascontiguousarray(W_last),
        "IDN": np.eye(128, dtype=np.float32),
        "W1A": np.ascontiguousarray(w1[:512].reshape(4, 128, 16)),
        "W1B": np.ascontiguousarray(w1[512:513]),
        "W2T": np.ascontiguousarray(conv2_w.transpose(2, 1, 0)),
        "D1R": np.ascontiguousarray(d1_w.reshape(DD, CONV2_LEN * 32)
                                    .astype(np.float32)),
        "D2": np.ascontiguousarray(
            (d2_w.astype(np.float64)
             @ np.array([[1.0, -1.0], [-1.0, 1.0]])).astype(np.float32)),
        "DIFF": np.array([[1.0, -1.0], [-1.0, 1.0]], dtype=np.float32),
        "B1": np.ascontiguousarray(conv1_b.reshape(16, 1)),
        "B2": np.ascontiguousarray(conv2_b.reshape(32, 1)),
        "BD1": np.ascontiguousarray(d1_b.reshape(32, 1)),
        "BD2": np.ascontiguousarray(
            (np.array([[1.0, -1.0], [-1.0, 1.0]])
             @ d2_b.reshape(2, 1)).astype(np.float32)),
        "KI": np.ascontiguousarray(np.broadcast_to(
            ((np.arange(K) % 4) * 16 + np.arange(K) // 4)
            .astype(np.float32), (128, K))),
        "MJ": np.ascontiguousarray(
            (np.arange(NPG)[None, None, :]
             < (np.arange(4)[:, None, None] * 128
                + np.arange(128)[None, :, None])).astype(np.float32)),
        "IOTA": np.ascontiguousarray(
            (np.arange(4)[None, :] * 128
             + np.arange(128)[:, None]).astype(np.float32)),
        "QMASK": np.ascontiguousarray(
            (np.arange(64)[:, None] % 4 == np.arange(4)[None, :])
            .astype(np.float32)),
        "E1M": np.ascontiguousarray(
            (np.arange(64)[:, None] // 4 == np.arange(16)[None, :])
            .astype(np.float32)),
        "RM": np.ascontiguousarray(
            (np.arange(128)[None, :] % 16 == np.arange(16)[:, None])
            .astype(np.float32)),
    }

    if fold_lin1:
        lin1 = emb @ W_convs[0]        # [1000, 128] f32
        h0lin = lin1[x]                # [N, 128]
    h0 = emb[x]

    in_maps = []
    for c in range(NCORES):
        m = dict(shared)
        if fold_lin1:
            loc = h0lin[c * NLOC:(c + 1) * NLOC]  # [4096, 128]
            # [g][p][cc*128+f] = lin1[g*512+cc*128+p, f]
            lm = loc.reshape(GPC, 4, 128, 128).transpose(0, 2, 1, 3)
            m["LIN1M"] = np.ascontiguousarray(
                lm.reshape(GPC, 128, NPG))
        else:
            m["H0T"] = np.ascontiguousarray(h0[c * NLOC:(c + 1) * NLOC].T)
        m["STD"] = np.ascontiguousarray(ST[c * GPC:(c + 1) * GPC])
        in_maps.append(m)
    return in_maps


def run(inputs, fold_lin1=True, debug=False, **spmd_kwargs):
    in_maps = prepare_host(inputs, fold_lin1)
    nc = _get_nc(fold_lin1, debug)
    res = run_bass_kernel_spmd(nc, in_maps, core_ids=list(range(NCORES)),
                               **spmd_kwargs)
    out = np.empty((NUM_GRAPHS, 2), dtype=np.float32)
    for c in range(NCORES):
        out[c * GPC:(c + 1) * GPC, :] = res.results[c]["OUT"].T
    return out, res


def kernel(**inputs):
    out, _ = run(inputs, fold_lin1=True)
    return out



# revision 9
# speedup vs baseline: 1.2371x; 1.2371x over previous
"""DGCNN (gnn_message_passing) Trainium2 Bass kernel, v2.

Strategy (data-parallel over graphs, 8 graphs per NeuronCore):
  - Host builds, per graph, the dense normalized propagation operator
    S[d, s] = (mult(s->d) + I) / deg_out[d]  (512x512 f32), shipped
    transposed as 4 chunks of [128, 512].
  - Layer-1 linear is folded on the host: LIN1 = (emb @ W1)[x] is shipped
    instead of h0 (flag FOLD_LIN1; layer-1 lin matmuls are skipped).
  - Graphs processed in pairs, graph-outer: each pair runs its 4 GCN
    layers on PE while the previous pair's sort-pooling tail runs on
    DVE/ACT/Pool, so the tail is hidden under PE time.
  - Per layer+graph on device: lin = h @ W as 4 chunk matmuls into one
    [128,512] PSUM bank, one copy to SBUF, then msgT = lin^T-chunks
    stationary x S^T chunks -> [128f, 512d] PSUM, tanh -> h (f32
    throughout: the top-64 sort is sensitive to ~1e-8 noise in h5, so
    the whole h chain must be bit-stable f32).
  - Sort-pooling tail per graph: exact stable ranks via comparison
    matrices (DVE/Pool alternating), one-hot rank matrix PT, top-64
    node indices extracted with a tiny iota matmul, converted to the
    wrapped int16 layout, and the pooled features gathered from h with
    gpsimd ap_gather (Pool engine) - no PE transposes or selection
    matmuls.
  - Conv1/maxpool/conv2/dense head batched across the 8 graphs.

Self-contained: hardcodes all shapes; no reads of /root/problem files.
"""

import sys

if "/opt/trn_rl_repo" not in sys.path:
    sys.path.insert(0, "/opt/trn_rl_repo")

import numpy as np

import concourse.bacc as bacc
import concourse.mybir as mybir
import concourse.tile as tile
from concourse.bass_utils import run_bass_kernel_spmd

F32 = mybir.dt.float32
F32R = mybir.dt.float32r  # same bits/numerics as f32; 4x PE rate at >=256 cols
I16 = mybir.dt.int16

NUM_GRAPHS = 64
NPG = 512  # nodes per graph
N_TOTAL = NUM_GRAPHS * NPG
EMB = 128
DIMF = 128
NLAYERS = 4
K = 64
NCORES = 8
GPC = NUM_GRAPHS // NCORES  # graphs per core = 8
NLOC = GPC * NPG  # local nodes = 4096
LATENT = NLAYERS * DIMF + 1  # 513
DD = (K - 2) // 2 + 1  # 32
CONV2_LEN = DD - 5 + 1  # 28

_NC_CACHE = {}


def _build(fold_lin1, debug):
    """Trace + compile the per-core Bass program (same on all 8 cores)."""
    nc = bacc.Bacc("TRN2", target_bir_lowering=False, debug=False,
                   num_devices=NCORES)

    # ---- per-core DRAM I/O ----
    # LIN1M[g] = (emb @ W1)[x] for graph g, chunk-major: [g][p][cc*128+f]
    #          = lin1[node cc*128+p of graph g, feat f]
    if fold_lin1:
        LIN1M = nc.dram_tensor("LIN1M", [GPC, 128, NPG], F32R,
                               kind="ExternalInput")
    else:
        H0T = nc.dram_tensor("H0T", [128, NLOC], F32, kind="ExternalInput")
    STD = nc.dram_tensor("STD", [GPC, 4, 128, NPG], F32R, kind="ExternalInput")
    WC = nc.dram_tensor("WC", [NLAYERS, 128, 128], F32, kind="ExternalInput")
    W5 = nc.dram_tensor("W5", [128, 1], F32, kind="ExternalInput")
    IDN = nc.dram_tensor("IDN", [128, 128], F32, kind="ExternalInput")
    W1A = nc.dram_tensor("W1A", [4, 128, 16], F32, kind="ExternalInput")
    W1B = nc.dram_tensor("W1B", [1, 16], F32, kind="ExternalInput")
    W2T = nc.dram_tensor("W2T", [5, 16, 32], F32, kind="ExternalInput")
    D1R = nc.dram_tensor("D1R", [32, CONV2_LEN * 32], F32, kind="ExternalInput")
    D2 = nc.dram_tensor("D2", [32, 2], F32, kind="ExternalInput")
    DIFF = nc.dram_tensor("DIFF", [2, 2], F32, kind="ExternalInput")
    B1 = nc.dram_tensor("B1", [16, 1], F32, kind="ExternalInput")
    B2 = nc.dram_tensor("B2", [32, 1], F32, kind="ExternalInput")
    BD1 = nc.dram_tensor("BD1", [32, 1], F32, kind="ExternalInput")
    BD2 = nc.dram_tensor("BD2", [2, 1], F32, kind="ExternalInput")
    KI = nc.dram_tensor("KI", [128, K], F32, kind="ExternalInput")
    MJ = nc.dram_tensor("MJ", [4, 128, NPG], F32, kind="ExternalInput")
    IOTA = nc.dram_tensor("IOTA", [128, 4], F32, kind="ExternalInput")
    QMASK = nc.dram_tensor("QMASK", [64, 4], F32, kind="ExternalInput")
    E1M = nc.dram_tensor("E1M", [64, 16], F32, kind="ExternalInput")
    RM = nc.dram_tensor("RM", [16, 128], F32, kind="ExternalInput")
    OUT = nc.dram_tensor("OUT", [2, GPC], F32, kind="ExternalOutput")
    if debug:
        DBG_H = nc.dram_tensor("DBG_H", [NLAYERS, 128, NLOC], F32,
                               kind="ExternalOutput")
        DBG_H5 = nc.dram_tensor("DBG_H5", [GPC, 1, NPG], F32,
                                kind="ExternalOutput")
        DBG_RANK = nc.dram_tensor("DBG_RANK", [GPC, 128, 4], F32,
                                  kind="ExternalOutput")
        DBG_IDX = nc.dram_tensor("DBG_IDX", [GPC, 128, 4], F32,
                                 kind="ExternalOutput")
        DBG_POOL = nc.dram_tensor("DBG_POOL", [NLAYERS, 128, GPC * K], F32,
                                  kind="ExternalOutput")
        DBG_POOL5 = nc.dram_tensor("DBG_POOL5", [1, GPC * K], F32,
                                   kind="ExternalOutput")
        DBG_Y2 = nc.dram_tensor("DBG_Y2", [32, GPC * CONV2_LEN], F32,
                                kind="ExternalOutput")

    TANH = mybir.ActivationFunctionType.Tanh
    RELU = mybir.ActivationFunctionType.Relu
    SIGM = mybir.ActivationFunctionType.Sigmoid
    ADD = mybir.AluOpType.add
    MULT = mybir.AluOpType.mult
    MAX = mybir.AluOpType.max
    IS_GT = mybir.AluOpType.is_gt
    IS_EQ = mybir.AluOpType.is_equal

    with tile.TileContext(nc) as tc:
        with (
            tc.tile_pool(name="const", bufs=1) as cp,
            tc.tile_pool(name="hg", bufs=6) as hp,          # [128,2048]/graph
            tc.tile_pool(name="st", bufs=32) as stp,        # [128,512]/chunk
            tc.tile_pool(name="lin", bufs=4) as linp,       # [128,512]
            tc.tile_pool(name="sc", bufs=3) as scp,         # compare scratch
            tc.tile_pool(name="vbp", bufs=3) as vbp,        # v broadcast
            tc.tile_pool(name="ptp", bufs=2) as ptp,        # ptt one-hots
            tc.tile_pool(name="sm", bufs=4) as smp,        # small tiles
            tc.tile_pool(name="idx", bufs=3) as idxp_pool,  # idx wrapped
            tc.tile_pool(name="xs", bufs=20) as xsp,        # endgame sel xts
            tc.tile_pool(name="ps512", bufs=2, space="PSUM") as ps5,
            tc.tile_pool(name="ps128", bufs=2, space="PSUM") as ps1,
            tc.tile_pool(name="psy1", bufs=1, space="PSUM") as psy,
        ):
            # ---- load order: pair-0 working set first ----
            lin1_sb = []

            def load_lin1(g, chunked=False):
                if fold_lin1:
                    t = linp.tile([128, NPG], F32R, tag="lin1", bufs=GPC,
                                  name=f"lin1_{g}")
                    if chunked:
                        for c in range(4):
                            nc.sync.dma_start(
                                t[:, c * 128:(c + 1) * 128],
                                LIN1M[g, :, c * 128:(c + 1) * 128])
                    else:
                        nc.sync.dma_start(t[:], LIN1M[g, :, :])
                    return t
                return None

            st_sb = {}

            def load_st(g):
                chunks = []
                for c in range(4):
                    t = stp.tile([128, NPG], F32R, tag="st", bufs=32,
                                 name=f"st_t{g}_{c}")
                    nc.sync.dma_start(t[:], STD[g, c, :, :])
                    chunks.append(t)
                st_sb[g] = chunks

            if fold_lin1:
                st00 = stp.tile([128, NPG], F32R, tag="st", bufs=32,
                                name="st_t0_0")
                nc.sync.dma_start(st00[:], STD[0, 0, :, :])
                lin1_sb = [load_lin1(0, chunked=True)]
                rest0 = []
                for c in range(1, 4):
                    t = stp.tile([128, NPG], F32R, tag="st", bufs=32,
                                 name=f"st_t0_{c}")
                    nc.sync.dma_start(t[:], STD[0, c, :, :])
                    rest0.append(t)
                st_sb[0] = [st00] + rest0
                lin1_sb.append(load_lin1(1))
                load_st(1)
            else:
                h0 = hp.tile([128, NLOC], F32, tag="h0x", bufs=1)
                for s in range(4):
                    nc.sync.dma_start(h0[:, s * 512:(s + 1) * 512],
                                      H0T[:, s * 512:(s + 1) * 512])
                load_st(0)
                load_st(1)
            wc_sb = cp.tile([128, NLAYERS * 128], F32, tag="wc")
            l_lo = 1 if fold_lin1 else 0
            for l in range(l_lo, NLAYERS):
                nc.sync.dma_start(wc_sb[:, l * 128:(l + 1) * 128], WC[l, :, :])
            w5_sb = cp.tile([128, 1], F32, tag="w5")
            nc.sync.dma_start(w5_sb[:], W5[:])
            id_sb = cp.tile([128, 128], F32, tag="idn")
            nc.sync.dma_start(id_sb[:], IDN[:])
            if fold_lin1:
                lin1_sb.append(load_lin1(2))
                load_st(2)
            else:
                for s in range(4, 8):
                    nc.sync.dma_start(h0[:, s * 512:(s + 1) * 512],
                                      H0T[:, s * 512:(s + 1) * 512])
                load_st(2)
            mj_sb = cp.tile([128, 4 * NPG], F32, tag="mj")
            for c in range(4):
                nc.sync.dma_start(mj_sb[:, c * NPG:(c + 1) * NPG], MJ[c, :, :])
            ki_sb = cp.tile([128, K], F32, tag="ki")
            nc.sync.dma_start(ki_sb[:], KI[:])
            iota_sb = cp.tile([128, 4], F32, tag="iota")
            nc.sync.dma_start(iota_sb[:], IOTA[:])
            qm_sb = cp.tile([64, 4], F32, tag="qm")
            nc.sync.dma_start(qm_sb[:], QMASK[:])
            e1_sb = cp.tile([64, 16], F32, tag="e1")
            nc.sync.dma_start(e1_sb[:], E1M[:])
            r_sb = cp.tile([16, 128], F32, tag="rm")
            nc.sync.dma_start(r_sb[:], RM[:])
            w1a_sb = cp.tile([128, 64], F32, tag="w1a")
            for c in range(4):
                nc.sync.dma_start(w1a_sb[:, c * 16:(c + 1) * 16], W1A[c, :, :])
            w1b_sb = cp.tile([1, 16], F32, tag="w1b")
            nc.sync.dma_start(w1b_sb[:], W1B[:])
            w2_sb = cp.tile([16, 160], F32, tag="w2t")
            for t5 in range(5):
                nc.sync.dma_start(w2_sb[:, t5 * 32:(t5 + 1) * 32],
                                  W2T[t5, :, :])
            d1_sb = cp.tile([32, CONV2_LEN * 32], F32, tag="d1r")
            nc.sync.dma_start(d1_sb[:], D1R[:])
            d2_sb = cp.tile([32, 2], F32, tag="d2")
            nc.sync.dma_start(d2_sb[:], D2[:])
            diff_sb = cp.tile([2, 2], F32, tag="diff")
            nc.sync.dma_start(diff_sb[:], DIFF[:])
            b1_sb = cp.tile([16, 1], F32, tag="b1")
            nc.sync.dma_start(b1_sb[:], B1[:])
            b2_sb = cp.tile([32, 1], F32, tag="b2")
            nc.sync.dma_start(b2_sb[:], B2[:])
            bd1_sb = cp.tile([32, 1], F32, tag="bd1")
            nc.sync.dma_start(bd1_sb[:], BD1[:])
            bd2_sb = cp.tile([2, 1], F32, tag="bd2")
            nc.sync.dma_start(bd2_sb[:], BD2[:])
            for g in range(3, GPC):
                if fold_lin1:
                    lin1_sb.append(load_lin1(g))
                load_st(g)

            # per-graph state
            hgs = {}       # g -> [128, 4*512] tile (layers 1..4)
            vcols = {}     # g -> [128, 4] node-major h5
            vbs = {}       # g -> [128, 512] v broadcast
            ranks = {}     # g -> [128, 4]
            ptts = {}      # g -> [128, 4K] one-hot rank matrix
            idxw = {}      # g -> [128, 4] int16 wrapped indices
            # pooled features, all graphs side by side, one tile per layer
            pooled_sb = [cp.tile([128, GPC * K], F32, tag=f"pool{l}",
                                 name=f"pool{l}")
                         for l in range(NLAYERS)]
            p5all = cp.tile([16, GPC * K], F32, tag="p5all")
            y1p = psy.tile([16, GPC * K], F32, tag="y1p")
            y2all = cp.tile([32, GPC * CONV2_LEN], F32, tag="y2all")
            y1 = cp.tile([16, GPC * K], F32, tag="y1")
            mp = cp.tile([16, GPC * K // 2], F32, tag="mp")

            # ---------------- layer machinery ----------------
            def lin_stage(g, l):
                # returns SBUF [128, 4*128] chunk-major lin
                if l == 0 and fold_lin1:
                    return lin1_sb[g]
                lp = ps5.tile([128, NPG], F32, tag="linp", bufs=2)
                for cc in range(4):
                    if l == 0:
                        stat = h0[:, g * NPG + cc * 128:
                                  g * NPG + (cc + 1) * 128]
                    else:
                        stat = hgs[g][:, (l - 1) * NPG + cc * 128:
                                      (l - 1) * NPG + (cc + 1) * 128]
                    nc.tensor.matmul(
                        lp[:, cc * 128:(cc + 1) * 128], stat,
                        wc_sb[:, l * 128:(l + 1) * 128],
                        start=True, stop=True)
                ln = linp.tile([128, NPG], F32R, tag="lin")
                if (g + l) % 2 == 0:
                    nc.vector.tensor_copy(ln[:], lp[:])
                else:
                    nc.scalar.copy(ln[:], lp[:])
                return ln

            def prop_stage(g, l, ln):
                sp = ps5.tile([128, NPG], F32, tag="msgp", bufs=2)
                for cc in range(4):
                    nc.tensor.matmul(
                        sp[:], ln[:, cc * 128:(cc + 1) * 128],
                        st_sb[g][cc][:],
                        start=(cc == 0), stop=(cc == 3))
                nc.scalar.activation(
                    hgs[g][:, l * NPG:(l + 1) * NPG], sp[:], TANH)

            def alloc_h(g):
                hgs[g] = hp.tile([128, NLAYERS * NPG], F32, tag="hg",
                                 name=f"h_{g}")

            def layers_pair(ga, gb):
                """All 4 GCN layers for graphs ga, gb, interleaved."""
                alloc_h(ga)
                alloc_h(gb)
                for l in range(NLAYERS):
                    lns = {g: lin_stage(g, l) for g in (ga, gb)}
                    for g in (ga, gb):
                        prop_stage(g, l, lns[g])
                    yield l

            # ---------------- tail stages ----------------
            lin5s = {}

            def sA(g):
                """layer-5 matvec (PE) + copy (DVE)."""
                hl = hgs[g]
                l5p = ps1.tile([128, 4], F32, tag="ps128")
                for cc in range(4):
                    nc.tensor.matmul(
                        l5p[:, cc:cc + 1],
                        hl[:, 3 * NPG + cc * 128:3 * NPG + (cc + 1) * 128],
                        w5_sb[:], start=True, stop=True)
                lin5 = smp.tile([128, 4], F32R, tag="lin5")
                nc.vector.tensor_copy(lin5[:], l5p[:])
                lin5s[g] = lin5

            def sB(g):
                """msg5 = S @ lin5 (PE, 16 tiny) + tanh (ACT)."""
                lin5 = lin5s[g]
                m5p = ps1.tile([128, 4], F32, tag="ps128")
                for dc in range(4):
                    for sc in range(4):
                        nc.tensor.matmul(
                            m5p[:, dc:dc + 1],
                            st_sb[g][sc][:, dc * 128:(dc + 1) * 128],
                            lin5[:, sc:sc + 1],
                            start=(sc == 0), stop=(sc == 3))
                vcol = smp.tile([128, 4], F32, tag="vcol")
                nc.scalar.activation(vcol[:], m5p[:], TANH)
                vcols[g] = vcol

            def sC(g):
                """h5 row form + broadcast."""
                vcol = vcols[g]
                vtp = ps1.tile([4, 128], F32, tag="ps128")
                nc.tensor.transpose(vtp[:], vcol[:], id_sb[:])
                vts = smp.tile([4, 128], F32, tag="vts")
                nc.vector.tensor_copy(vts[:], vtp[:])
                h5r = smp.tile([1, NPG], F32, tag="h5r", bufs=3)
                for cc in range(4):
                    nc.sync.dma_start(h5r[0:1, cc * 128:(cc + 1) * 128],
                                      vts[cc:cc + 1, :])
                vb = vbp.tile([128, NPG], F32, tag="vb")
                nc.gpsimd.partition_broadcast(vb[:], h5r[0:1, :])
                vbs[g] = vb
                if debug:
                    nc.sync.dma_start(DBG_H5[g, :, :], h5r[:])

            def sD(g, dve_chunks=None):
                """exact stable ranks; dve_chunks picks per-chunk engine."""
                vb, vcol = vbs[g], vcols[g]
                if dve_chunks is None:
                    dve_chunks = (0, 1, 2, 3)
                rank = smp.tile([128, 4], F32, tag="rank")
                for cc in range(4):
                    eng = nc.vector if cc in dve_chunks else nc.gpsimd
                    t1 = scp.tile([128, NPG], F32, tag="tt")
                    ra = smp.tile([128, 2], F32, tag="ra")
                    eng.tensor_scalar(
                        out=t1[:], in0=vb[:], scalar1=vcol[:, cc:cc + 1],
                        scalar2=None, op0=IS_GT, op1=ADD,
                        accum_out=ra[:, 0:1])
                    t2 = scp.tile([128, NPG], F32, tag="tt")
                    eng.scalar_tensor_tensor(
                        out=t2[:], in0=vb[:], scalar=vcol[:, cc:cc + 1],
                        in1=mj_sb[:, cc * NPG:(cc + 1) * NPG],
                        op0=IS_EQ, op1=MULT, accum_out=ra[:, 1:2])
                    nc.vector.tensor_tensor(
                        out=rank[:, cc:cc + 1], in0=ra[:, 0:1],
                        in1=ra[:, 1:2], op=ADD)
                ranks[g] = rank
                if debug:
                    nc.sync.dma_start(DBG_RANK[g, :, :], rank[:])

            def sE(g):
                """one-hot rank matrix (columns in wrapped-permuted order)."""
                rank = ranks[g]
                ptt = ptp.tile([128, 4 * K], F32, tag="pt")
                for cc in range(4):
                    nc.vector.tensor_scalar(
                        out=ptt[:, cc * K:(cc + 1) * K], in0=ki_sb[:],
                        scalar1=rank[:, cc:cc + 1], scalar2=None, op0=IS_EQ)
                ptts[g] = ptt

            def sF(g):
                """ordered top-64 node indices, int16 wrapped for ap_gather."""
                ptt = ptts[g]
                # col64[q] = index of the node with rank perm(q); KI's
                # permutation makes the downstream folds land each index at
                # iw[p, s] = idx[s*16+p], the ap_gather wrapped layout.
                cxp = ps1.tile([K, 1], F32, tag="ps128")
                for cc in range(4):
                    nc.tensor.matmul(cxp[:], ptt[:, cc * K:(cc + 1) * K],
                                     iota_sb[:, cc:cc + 1],
                                     start=(cc == 0), stop=(cc == 3))
                c64 = smp.tile([K, 1], F32, tag="c64")
                nc.vector.tensor_copy(c64[:], cxp[:])
                m64 = smp.tile([K, 4], F32, tag="m64")
                nc.vector.tensor_scalar(out=m64[:], in0=qm_sb[:],
                                        scalar1=c64[:, 0:1], scalar2=None,
                                        op0=MULT)
                wqp = ps1.tile([16, 4], F32, tag="ps128")
                nc.tensor.matmul(wqp[:], e1_sb[:], m64[:],
                                 start=True, stop=True)
                wq = smp.tile([16, 4], F32, tag="wq")
                nc.vector.tensor_copy(wq[:], wqp[:])
                wfp = ps1.tile([128, 4], F32, tag="ps128")
                nc.tensor.matmul(wfp[:], r_sb[:], wq[:],
                                 start=True, stop=True)
                iw = idxp_pool.tile([128, 4], I16, tag="iw")
                nc.vector.tensor_copy(iw[:], wfp[:])
                idxw[g] = iw
                if debug:
                    dbgi = smp.tile([128, 4], F32, tag="dbgi")
                    nc.vector.tensor_copy(dbgi[:], iw[:])
                    nc.sync.dma_start(DBG_IDX[g, :, :], dbgi[:])

            def sG(g, pool5_on_pe=False):
                """gather pooled features on the Pool engine."""
                iw = idxw[g]
                hl = hgs[g]
                for l in range(NLAYERS):
                    nc.gpsimd.ap_gather(
                        pooled_sb[l][:, g * K:(g + 1) * K],
                        hl[:, l * NPG:(l + 1) * NPG], iw[:],
                        channels=128, num_elems=NPG, d=1, num_idxs=K)
                if pool5_on_pe:
                    # v[idx] via selection matmul; undo the column
                    # permutation with a strided copy out of PSUM.
                    vcol, ptt = vcols[g], ptts[g]
                    p5p = ps1.tile([1, K], F32, tag="ps128")
                    for cc in range(4):
                        nc.tensor.matmul(p5p[:], vcol[:, cc:cc + 1],
                                         ptt[:, cc * K:(cc + 1) * K],
                                         start=(cc == 0), stop=(cc == 3))
                    dstv = p5all[0:1, g * K:(g + 1) * K].rearrange(
                        "a (s p) -> a p s", p=16)
                    srcv = p5p[0:1, :].rearrange("a (p s) -> a p s", s=4)
                    nc.vector.tensor_copy(dstv, srcv)
                else:
                    nc.gpsimd.ap_gather(
                        p5all[:, g * K:(g + 1) * K], vbs[g][0:16, :],
                        iw[0:16, :], channels=16, num_elems=NPG, d=1,
                        num_idxs=K)

            xts = {}

            def sSelXt(g, alt_eng=None):
                """node-major h chunks via PE transpose (endgame only);
                4 chunk transposes land in one PSUM tile -> one wide copy."""
                hl = hgs[g]
                lst = []
                for l in range(NLAYERS):
                    tg = "linp" if l % 2 == 0 else "msgp"
                    xp = ps5.tile([128, NPG], F32, tag=tg, bufs=2)
                    for cc in range(4):
                        nc.tensor.transpose(
                            xp[:, cc * 128:(cc + 1) * 128],
                            hl[:, l * NPG + cc * 128:
                               l * NPG + (cc + 1) * 128], id_sb[:])
                    xt = xsp.tile([128, NPG], F32, tag="xtb", bufs=6)
                    if alt_eng is not None and l % 2 == 1:
                        alt_eng.tensor_copy(xt[:], xp[:])
                    else:
                        nc.scalar.copy(xt[:], xp[:])
                    lst.append(xt)
                xts[g] = lst

            def sSelMM(g):
                """selection matmuls; ptt columns are permuted, so the
                copies to pooled_sb/p5all unpermute via strided views."""
                ptt, vcol = ptts[g], vcols[g]
                for l in range(NLAYERS):
                    tg = "linp" if l % 2 == 0 else "msgp"
                    pp = ps5.tile([128, K], F32, tag=tg, bufs=2)
                    for cc in range(4):
                        nc.tensor.matmul(pp[:],
                                         xts[g][l][:, cc * 128:(cc + 1) * 128],
                                         ptt[:, cc * K:(cc + 1) * K],
                                         start=(cc == 0), stop=(cc == 3))
                    dstv = pooled_sb[l][:, g * K:(g + 1) * K].rearrange(
                        "c (s p) -> c p s", p=16)
                    srcv = pp[:].rearrange("c (p s) -> c p s", s=4)
                    nc.scalar.copy(dstv, srcv)
                p5p = ps1.tile([1, K], F32, tag="ps128")
                for cc in range(4):
                    nc.tensor.matmul(p5p[:], vcol[:, cc:cc + 1],
                                     ptt[:, cc * K:(cc + 1) * K],
                                     start=(cc == 0), stop=(cc == 3))
                dstv = p5all[0:1, g * K:(g + 1) * K].rearrange(
                    "a (s p) -> a p s", p=16)
                srcv = p5p[0:1, :].rearrange("a (p s) -> a p s", s=4)
                nc.vector.tensor_copy(dstv, srcv)

            def sHa(g):
                """conv1 + relu + maxpool for graph g."""
                for l in range(NLAYERS):
                    nc.tensor.matmul(y1p[:, g * K:(g + 1) * K],
                                     w1a_sb[:, l * 16:(l + 1) * 16],
                                     pooled_sb[l][:, g * K:(g + 1) * K],
                                     start=(l == 0), stop=False)
                nc.tensor.matmul(y1p[:, g * K:(g + 1) * K], w1b_sb[:],
                                 p5all[0:1, g * K:(g + 1) * K],
                                 start=False, stop=True)
                nc.scalar.activation(y1[:, g * K:(g + 1) * K],
                                     y1p[:, g * K:(g + 1) * K], RELU,
                                     bias=b1_sb[:, 0:1])
                y1v = y1[:, g * K:(g + 1) * K].rearrange(
                    "p (a b) -> p a b", b=2)
                nc.vector.tensor_tensor(
                    out=mp[:, g * DD:(g + 1) * DD], in0=y1v[:, :, 0:1],
                    in1=y1v[:, :, 1:2], op=MAX)

            def sHb(g):
                """conv2 + relu for graph g."""
                y2p = ps1.tile([32, CONV2_LEN], F32, tag="ps128")
                for t5 in range(5):
                    nc.tensor.matmul(
                        y2p[:],
                        w2_sb[:, t5 * 32:(t5 + 1) * 32],
                        mp[:, g * DD + t5:g * DD + t5 + CONV2_LEN],
                        start=(t5 == 0), stop=(t5 == 4))
                nc.scalar.activation(
                    y2all[:, g * CONV2_LEN:(g + 1) * CONV2_LEN], y2p[:],
                    RELU, bias=b2_sb[:, 0:1])

            def sH(g):
                """per-graph conv1 + relu + maxpool + conv2 + relu."""
                for l in range(NLAYERS):
                    nc.tensor.matmul(y1p[:, g * K:(g + 1) * K],
                                     w1a_sb[:, l * 16:(l + 1) * 16],
                                     pooled_sb[l][:, g * K:(g + 1) * K],
                                     start=(l == 0), stop=False)
                nc.tensor.matmul(y1p[:, g * K:(g + 1) * K], w1b_sb[:],
                                 p5all[0:1, g * K:(g + 1) * K],
                                 start=False, stop=True)
                nc.scalar.activation(y1[:, g * K:(g + 1) * K],
                                     y1p[:, g * K:(g + 1) * K], RELU,
                                     bias=b1_sb[:, 0:1])
                y1v = y1[:, g * K:(g + 1) * K].rearrange(
                    "p (a b) -> p a b", b=2)
                nc.vector.tensor_tensor(
                    out=mp[:, g * DD:(g + 1) * DD], in0=y1v[:, :, 0:1],
                    in1=y1v[:, :, 1:2], op=MAX)
                y2p = ps1.tile([32, CONV2_LEN], F32, tag="ps128")
                for t5 in range(5):
                    nc.tensor.matmul(
                        y2p[:],
                        w2_sb[:, t5 * 32:(t5 + 1) * 32],
                        mp[:, g * DD + t5:g * DD + t5 + CONV2_LEN],
                        start=(t5 == 0), stop=(t5 == 4))
                nc.scalar.activation(
                    y2all[:, g * CONV2_LEN:(g + 1) * CONV2_LEN], y2p[:],
                    RELU, bias=b2_sb[:, 0:1])


            # ---------------- schedule ----------------
            # pair p: own sA at l3; pair p-1 runs B,C,D,E at l0..l3;
            # pair p-2 runs F,G,H at l0..l2.
            NP = GPC // 2
            for p in range(NP):
                ga, gb = 2 * p, 2 * p + 1
                gen = layers_pair(ga, gb)
                for l in gen:
                    g1 = (2 * (p - 1), 2 * (p - 1) + 1) if p >= 1 else ()
                    g2 = (2 * (p - 2), 2 * (p - 2) + 1) if p >= 2 else ()
                    if l == 0:
                        for g in g1:
                            sB(g)
                        for g in g2:
                            sF(g)
                    elif l == 1:
                        for g in g1:
                            sC(g)
                        for g in g2:
                            sG(g)
                    elif l == 2:
                        for g in g1:
                            sD(g)
                    elif l == 3:
                        for g in g1:
                            sE(g)
                        if p < NP - 1:
                            for g in g2:
                                sH(g)
                            sA(ga)
                            sA(gb)
                        else:
                            # critical tail chain of the last pair jumps
                            # ahead of the non-critical conv heads in the
                            # ACT/DVE queues
                            sA(ga)
                            sA(gb)
                            sB(ga)
                            sB(gb)
                            sC(ga)
                            sC(gb)
                            for g in g2:
                                sH(g)
                            sSelXt(ga)
            # endgame: pair NP-2 needs F,G,H; pair NP-1 needs D..H
            # (its A-C stages were hoisted into the last layer slot).
            q2 = (2 * (NP - 2), 2 * (NP - 2) + 1)
            q3 = (2 * (NP - 1), 2 * (NP - 1) + 1)
            sD(q3[0])
            sE(q3[0])
            sD(q3[1])
            sE(q3[1])
            sF(q2[0])
            sF(q2[1])
            sG(q2[0])
            sG(q2[1])
            sSelMM(q3[0])
            sSelXt(q3[1], alt_eng=nc.vector)
            sH(q2[0])
            sH(q2[1])
            sSelMM(q3[1])
            sHa(q3[0])
            sHa(q3[1])
            sHb(q3[0])
            sHb(q3[1])

            if debug:
                for l in range(NLAYERS):
                    for g in range(GPC):
                        nc.sync.dma_start(
                            DBG_H[l, :, g * NPG:(g + 1) * NPG],
                            hgs[g][:, l * NPG:(l + 1) * NPG])
                    nc.sync.dma_start(DBG_POOL[l, :, :], pooled_sb[l][:])
                nc.sync.dma_start(DBG_POOL5[:], p5all[0:1, :])

            # ---------------- head (batched over graphs) ----------------
            y1 = smp.tile([16, GPC * K], F32, tag="y1", bufs=1)
            nc.scalar.activation(y1[:], y1p[:], RELU, bias=b1_sb[:, 0:1])
            mp = smp.tile([16, GPC * K // 2], F32, tag="mp", bufs=1)
            y1v = y1[:].rearrange("p (a b) -> p a b", b=2)
            nc.vector.tensor_tensor(out=mp[:], in0=y1v[:, :, 0:1],
                                    in1=y1v[:, :, 1:2], op=MAX)
            y2p = ps1.tile([32, GPC * CONV2_LEN], F32, tag="y2p", bufs=1)
            for g in range(GPC):
                for t5 in range(5):
                    nc.tensor.matmul(
                        y2p[:, g * CONV2_LEN:(g + 1) * CONV2_LEN],
                        w2_sb[:, t5 * 32:(t5 + 1) * 32],
                        mp[:, g * DD + t5:g * DD + t5 + CONV2_LEN],
                        start=(t5 == 0), stop=(t5 == 4))
            nc.scalar.activation(y2all[:], y2p[:], RELU, bias=b2_sb[:, 0:1])
            if debug:
                nc.sync.dma_start(DBG_Y2[:], y2all[:])

            h1p = ps1.tile([32, GPC], F32, tag="ps128")
            y2v = y2all[:].rearrange("p (g t) -> p g t", t=CONV2_LEN)
            for t5 in range(CONV2_LEN):
                nc.tensor.matmul(h1p[:], d1_sb[:, t5 * 32:(t5 + 1) * 32],
                                 y2v[:, :, t5:t5 + 1],
                                 start=(t5 == 0), stop=(t5 == CONV2_LEN - 1))
            h1s = smp.tile([32, GPC], F32, tag="h1s", bufs=1)
            nc.scalar.activation(h1s[:], h1p[:], RELU, bias=bd1_sb[:, 0:1])
            dfp = ps1.tile([2, GPC], F32, tag="ps128")
            nc.tensor.matmul(dfp[:], d2_sb[:], h1s[:], start=True, stop=True)
            pr = smp.tile([2, GPC], F32, tag="pr", bufs=1)
            nc.scalar.activation(pr[:], dfp[:], SIGM, bias=bd2_sb[:, 0:1])
            nc.sync.dma_start(OUT[:], pr[:])

    nc.compile()
    return nc


def _get_nc(fold_lin1, debug):
    key = (fold_lin1, debug)
    if key not in _NC_CACHE:
        _NC_CACHE[key] = _build(fold_lin1, debug)
    return _NC_CACHE[key]


def prepare_host(inputs, fold_lin1=True):
    """All host-side index preprocessing + per-core input maps."""
    x = np.asarray(inputs["x"]).astype(np.int64)
    edge_index = np.asarray(inputs["edge_index"]).astype(np.int64)
    emb = np.ascontiguousarray(np.asarray(inputs["emb"], dtype=np.float32))
    W_convs = np.asarray(inputs["W_convs"], dtype=np.float32)
    conv1_w = np.asarray(inputs["conv1_w"], dtype=np.float32)
    conv1_b = np.asarray(inputs["conv1_b"], dtype=np.float32)
    conv2_w = np.asarray(inputs["conv2_w"], dtype=np.float32)
    conv2_b = np.asarray(inputs["conv2_b"], dtype=np.float32)
    d1_w = np.asarray(inputs["d1_w"], dtype=np.float32)
    d1_b = np.asarray(inputs["d1_b"], dtype=np.float32)
    d2_w = np.asarray(inputs["d2_w"], dtype=np.float32)
    d2_b = np.asarray(inputs["d2_b"], dtype=np.float32)
    W_last = np.asarray(inputs["W_last"], dtype=np.float32)

    src, dst = edge_index[0], edge_index[1]
    deg = (np.bincount(src, minlength=N_TOTAL) + 1).astype(np.float32)
    invdeg = (np.float32(1.0) / deg).astype(np.float32)
    gid = dst >> 9
    flat = (gid * NPG + (dst & 511)) * NPG + (src & 511)
    A = np.bincount(flat, minlength=NUM_GRAPHS * NPG * NPG)
    A = A.astype(np.float32).reshape(NUM_GRAPHS, NPG, NPG)
    idx = np.arange(NPG)
    A[:, idx, idx] += 1.0
    S = A * invdeg.reshape(NUM_GRAPHS, NPG, 1)
    ST = np.ascontiguousarray(S.transpose(0, 2, 1)).reshape(
        NUM_GRAPHS, 4, 128, NPG)

    w1 = np.ascontiguousarray(conv1_w[:, 0, :].T)  # [513, 16]
    shared = {
        "WC": np.ascontiguousarray(W_convs),
        "W5": np.ascontiguousarray(W_last),
        "IDN": np.eye(128, dtype=np.float32),
        "W1A": np.ascontiguousarray(w1[:512].reshape(4, 128, 16)),
        "W1B": np.ascontiguousarray(w1[512:513]),
        "W2T": np.ascontiguousarray(conv2_w.transpose(2, 1, 0)),
        "D1R": np.ascontiguousarray(d1_w.reshape(DD, CONV2_LEN * 32)
                                    .astype(np.float32)),
        "D2": np.ascontiguousarray(
            (d2_w.astype(np.float64)
             @ np.array([[1.0, -1.0], [-1.0, 1.0]])).astype(np.float32)),
        "DIFF": np.array([[1.0, -1.0], [-1.0, 1.0]], dtype=np.float32),
        "B1": np.ascontiguousarray(conv1_b.reshape(16, 1)),
        "B2": np.ascontiguousarray(conv2_b.reshape(32, 1)),
        "BD1": np.ascontiguousarray(d1_b.reshape(32, 1)),
        "BD2": np.ascontiguousarray(
            (np.array([[1.0, -1.0], [-1.0, 1.0]])
             @ d2_b.reshape(2, 1)).astype(np.float32)),
        "KI": np.ascontiguousarray(np.broadcast_to(
            ((np.arange(K) % 4) * 16 + np.arange(K) // 4)
            .astype(np.float32), (128, K))),
        "MJ": np.ascontiguousarray(
            (np.arange(NPG)[None, None, :]
             < (np.arange(4)[:, None, None] * 128
                + np.arange(128)[None, :, None])).astype(np.float32)),
        "IOTA": np.ascontiguousarray(
            (np.arange(4)[None, :] * 128
             + np.arange(128)[:, None]).astype(np.float32)),
        "QMASK": np.ascontiguousarray(
            (np.arange(64)[:, None] % 4 == np.arange(4)[None, :])
            .astype(np.float32)),
        "E1M": np.ascontiguousarray(
            (np.arange(64)[:, None] // 4 == np.arange(16)[None, :])
            .astype(np.float32)),
        "RM": np.ascontiguousarray(
            (np.arange(128)[None, :] % 16 == np.arange(16)[:, None])
            .astype(np.float32)),
    }

    if fold_lin1:
        lin1 = emb @ W_convs[0]        # [1000, 128] f32
        h0lin = lin1[x]                # [N, 128]
    h0 = emb[x]

    in_maps = []
    for c in range(NCORES):
        m = dict(shared)
        if fold_lin1:
            loc = h0lin[c * NLOC:(c + 1) * NLOC]  # [4096, 128]
            # [g][p][cc*128+f] = lin1[g*512+cc*128+p, f]
            lm = loc.reshape(GPC, 4, 128, 128).transpose(0, 2, 1, 3)
            m["LIN1M"] = np.ascontiguousarray(
                lm.reshape(GPC, 128, NPG))
        else:
            m["H0T"] = np.ascontiguousarray(h0[c * NLOC:(c + 1) * NLOC].T)
        m["STD"] = np.ascontiguousarray(ST[c * GPC:(c + 1) * GPC])
        in_maps.append(m)
    return in_maps


def run(inputs, fold_lin1=True, debug=False, **spmd_kwargs):
    in_maps = prepare_host(inputs, fold_lin1)
    nc = _get_nc(fold_lin1, debug)
    res = run_bass_kernel_spmd(nc, in_maps, core_ids=list(range(NCORES)),
                               **spmd_kwargs)
    out = np.empty((NUM_GRAPHS, 2), dtype=np.float32)
    for c in range(NCORES):
        out[c * GPC:(c + 1) * GPC, :] = res.results[c]["OUT"].T
    return out, res


def kernel(**inputs):
    out, _ = run(inputs, fold_lin1=True)
    return out



# revision 15
# speedup vs baseline: 1.2542x; 1.0138x over previous
"""DGCNN (gnn_message_passing) Trainium2 Bass kernel, v2.

Strategy (data-parallel over graphs, 8 graphs per NeuronCore):
  - Host builds, per graph, the dense normalized propagation operator
    S[d, s] = (mult(s->d) + I) / deg_out[d]  (512x512 f32), shipped
    transposed as 4 chunks of [128, 512].
  - Layer-1 linear is folded on the host: LIN1 = (emb @ W1)[x] is shipped
    instead of h0 (flag FOLD_LIN1; layer-1 lin matmuls are skipped).
  - Graphs processed in pairs, graph-outer: each pair runs its 4 GCN
    layers on PE while the previous pair's sort-pooling tail runs on
    DVE/ACT/Pool, so the tail is hidden under PE time.
  - Per layer+graph on device: lin = h @ W as 4 chunk matmuls into one
    [128,512] PSUM bank, one copy to SBUF, then msgT = lin^T-chunks
    stationary x S^T chunks -> [128f, 512d] PSUM, tanh -> h (f32
    throughout: the top-64 sort is sensitive to ~1e-8 noise in h5, so
    the whole h chain must be bit-stable f32).
  - Sort-pooling tail per graph: exact stable ranks via comparison
    matrices (DVE/Pool alternating), one-hot rank matrix PT, top-64
    node indices extracted with a tiny iota matmul, converted to the
    wrapped int16 layout, and the pooled features gathered from h with
    gpsimd ap_gather (Pool engine) - no PE transposes or selection
    matmuls.
  - Conv1/maxpool/conv2/dense head batched across the 8 graphs.

Self-contained: hardcodes all shapes; no reads of /root/problem files.
"""

import sys

if "/opt/trn_rl_repo" not in sys.path:
    sys.path.insert(0, "/opt/trn_rl_repo")

import ml_dtypes
import numpy as np

import concourse.bacc as bacc
import concourse.mybir as mybir
import concourse.tile as tile
from concourse.bass_utils import run_bass_kernel_spmd

F32 = mybir.dt.float32
F32R = mybir.dt.float32r  # same bits/numerics as f32; 4x PE rate at >=256 cols
I16 = mybir.dt.int16
BF16 = mybir.dt.bfloat16

NUM_GRAPHS = 64
NPG = 512  # nodes per graph
N_TOTAL = NUM_GRAPHS * NPG
EMB = 128
DIMF = 128
NLAYERS = 4
K = 64
NCORES = 8
GPC = NUM_GRAPHS // NCORES  # graphs per core = 8
NLOC = GPC * NPG  # local nodes = 4096
LATENT = NLAYERS * DIMF + 1  # 513
DD = (K - 2) // 2 + 1  # 32
CONV2_LEN = DD - 5 + 1  # 28

_NC_CACHE = {}


def _build(fold_lin1, debug):
    """Trace + compile the per-core Bass program (same on all 8 cores)."""
    nc = bacc.Bacc("TRN2", target_bir_lowering=False, debug=False,
                   num_devices=NCORES)

    # ---- per-core DRAM I/O ----
    # LIN1M[g] = (emb @ W1)[x] for graph g, chunk-major: [g][p][cc*128+f]
    #          = lin1[node cc*128+p of graph g, feat f]
    if fold_lin1:
        LIN1M = nc.dram_tensor("LIN1M", [GPC, 128, NPG], F32R,
                               kind="ExternalInput")
    else:
        H0T = nc.dram_tensor("H0T", [128, NLOC], F32, kind="ExternalInput")
    STD = nc.dram_tensor("STD", [GPC, 4, 128, NPG], F32R, kind="ExternalInput")
    WC = nc.dram_tensor("WC", [NLAYERS, 128, 128], F32R, kind="ExternalInput")
    W5 = nc.dram_tensor("W5", [128, 1], F32R, kind="ExternalInput")
    IDN = nc.dram_tensor("IDN", [128, 128], BF16, kind="ExternalInput")
    W1A = nc.dram_tensor("W1A", [4, 128, 16], F32R, kind="ExternalInput")
    W1B = nc.dram_tensor("W1B", [1, 16], F32R, kind="ExternalInput")
    W2T = nc.dram_tensor("W2T", [5, 16, 32], F32R, kind="ExternalInput")
    D1R = nc.dram_tensor("D1R", [32, CONV2_LEN * 32], F32R, kind="ExternalInput")
    D2 = nc.dram_tensor("D2", [32, 2], F32R, kind="ExternalInput")
    DIFF = nc.dram_tensor("DIFF", [2, 2], F32, kind="ExternalInput")
    B1 = nc.dram_tensor("B1", [16, 1], F32, kind="ExternalInput")
    B2 = nc.dram_tensor("B2", [32, 1], F32, kind="ExternalInput")
    BD1 = nc.dram_tensor("BD1", [32, 1], F32, kind="ExternalInput")
    BD2 = nc.dram_tensor("BD2", [2, 1], F32, kind="ExternalInput")
    KI = nc.dram_tensor("KI", [128, K], F32, kind="ExternalInput")
    MJ = nc.dram_tensor("MJ", [4, 128, NPG], F32, kind="ExternalInput")
    IOTA = nc.dram_tensor("IOTA", [128, 4], F32R, kind="ExternalInput")
    QMASK = nc.dram_tensor("QMASK", [64, 4], F32R, kind="ExternalInput")
    E1M = nc.dram_tensor("E1M", [64, 16], F32R, kind="ExternalInput")
    RM = nc.dram_tensor("RM", [16, 128], F32R, kind="ExternalInput")
    OUT = nc.dram_tensor("OUT", [2, GPC], F32, kind="ExternalOutput")
    if debug:
        DBG_H = nc.dram_tensor("DBG_H", [NLAYERS, 128, NLOC], F32,
                               kind="ExternalOutput")
        DBG_H5 = nc.dram_tensor("DBG_H5", [GPC, 1, NPG], F32,
                                kind="ExternalOutput")
        DBG_RANK = nc.dram_tensor("DBG_RANK", [GPC, 128, 4], F32,
                                  kind="ExternalOutput")
        DBG_IDX = nc.dram_tensor("DBG_IDX", [GPC, 128, 4], F32,
                                 kind="ExternalOutput")
        DBG_POOL = nc.dram_tensor("DBG_POOL", [NLAYERS, 128, GPC * K], F32,
                                  kind="ExternalOutput")
        DBG_POOL5 = nc.dram_tensor("DBG_POOL5", [1, GPC * K], F32,
                                   kind="ExternalOutput")
        DBG_Y2 = nc.dram_tensor("DBG_Y2", [32, GPC * CONV2_LEN], F32,
                                kind="ExternalOutput")

    TANH = mybir.ActivationFunctionType.Tanh
    RELU = mybir.ActivationFunctionType.Relu
    SIGM = mybir.ActivationFunctionType.Sigmoid
    ADD = mybir.AluOpType.add
    MULT = mybir.AluOpType.mult
    MAX = mybir.AluOpType.max
    IS_GT = mybir.AluOpType.is_gt
    IS_EQ = mybir.AluOpType.is_equal

    with tile.TileContext(nc) as tc:
        with (
            tc.tile_pool(name="const", bufs=1) as cp,
            tc.tile_pool(name="hg", bufs=6) as hp,          # [128,2048]/graph
            tc.tile_pool(name="st", bufs=32) as stp,        # [128,512]/chunk
            tc.tile_pool(name="lin", bufs=4) as linp,       # [128,512]
            tc.tile_pool(name="sc", bufs=3) as scp,         # compare scratch
            tc.tile_pool(name="vbp", bufs=3) as vbp,        # v broadcast
            tc.tile_pool(name="ptp", bufs=2) as ptp,        # ptt one-hots
            tc.tile_pool(name="sm", bufs=4) as smp,        # small tiles
            tc.tile_pool(name="idx", bufs=3) as idxp_pool,  # idx wrapped
            tc.tile_pool(name="xs", bufs=20) as xsp,        # endgame sel xts
            tc.tile_pool(name="ps512", bufs=2, space="PSUM") as ps5,
            tc.tile_pool(name="ps128", bufs=2, space="PSUM") as ps1,
            tc.tile_pool(name="psy1", bufs=1, space="PSUM") as psy,
        ):
            # ---- load order: pair-0 working set first ----
            lin1_sb = []

            def load_lin1(g, chunked=False):
                if fold_lin1:
                    t = linp.tile([128, NPG], F32R, tag="lin1", bufs=GPC,
                                  name=f"lin1_{g}")
                    if chunked:
                        for c in range(4):
                            nc.sync.dma_start(
                                t[:, c * 128:(c + 1) * 128],
                                LIN1M[g, :, c * 128:(c + 1) * 128])
                    else:
                        nc.sync.dma_start(t[:], LIN1M[g, :, :])
                    return t
                return None

            st_sb = {}

            def load_st(g):
                chunks = []
                for c in range(4):
                    t = stp.tile([128, NPG], F32R, tag="st", bufs=32,
                                 name=f"st_t{g}_{c}")
                    nc.sync.dma_start(t[:], STD[g, c, :, :])
                    chunks.append(t)
                st_sb[g] = chunks

            if fold_lin1:
                st00 = stp.tile([128, NPG], F32R, tag="st", bufs=32,
                                name="st_t0_0")
                nc.sync.dma_start(st00[:], STD[0, 0, :, :])
                lin1_sb = [load_lin1(0, chunked=True)]
                rest0 = []
                for c in range(1, 4):
                    t = stp.tile([128, NPG], F32R, tag="st", bufs=32,
                                 name=f"st_t0_{c}")
                    nc.sync.dma_start(t[:], STD[0, c, :, :])
                    rest0.append(t)
                st_sb[0] = [st00] + rest0
                lin1_sb.append(load_lin1(1))
                load_st(1)
            else:
                h0 = hp.tile([128, NLOC], F32, tag="h0x", bufs=1)
                for s in range(4):
                    nc.sync.dma_start(h0[:, s * 512:(s + 1) * 512],
                                      H0T[:, s * 512:(s + 1) * 512])
                load_st(0)
                load_st(1)
            wc_sb = cp.tile([128, NLAYERS * 128], F32R, tag="wc")
            l_lo = 1 if fold_lin1 else 0
            for l in range(l_lo, NLAYERS):
                nc.sync.dma_start(wc_sb[:, l * 128:(l + 1) * 128], WC[l, :, :])
            w5_sb = cp.tile([128, 1], F32R, tag="w5")
            nc.sync.dma_start(w5_sb[:], W5[:])
            id_sb = cp.tile([128, 128], BF16, tag="idn")
            nc.sync.dma_start(id_sb[:], IDN[:])
            if fold_lin1:
                lin1_sb.append(load_lin1(2))
                load_st(2)
            else:
                for s in range(4, 8):
                    nc.sync.dma_start(h0[:, s * 512:(s + 1) * 512],
                                      H0T[:, s * 512:(s + 1) * 512])
                load_st(2)
            mj_sb = cp.tile([128, 4 * NPG], F32, tag="mj")
            for c in range(4):
                nc.sync.dma_start(mj_sb[:, c * NPG:(c + 1) * NPG], MJ[c, :, :])
            ki_sb = cp.tile([128, K], F32, tag="ki")
            nc.sync.dma_start(ki_sb[:], KI[:])
            iota_sb = cp.tile([128, 4], F32R, tag="iota")
            nc.sync.dma_start(iota_sb[:], IOTA[:])
            qm_sb = cp.tile([64, 4], F32R, tag="qm")
            nc.sync.dma_start(qm_sb[:], QMASK[:])
            e1_sb = cp.tile([64, 16], F32R, tag="e1")
            nc.sync.dma_start(e1_sb[:], E1M[:])
            r_sb = cp.tile([16, 128], F32R, tag="rm")
            nc.sync.dma_start(r_sb[:], RM[:])
            w1a_sb = cp.tile([128, 64], F32R, tag="w1a")
            for c in range(4):
                nc.sync.dma_start(w1a_sb[:, c * 16:(c + 1) * 16], W1A[c, :, :])
            w1b_sb = cp.tile([1, 16], F32R, tag="w1b")
            nc.sync.dma_start(w1b_sb[:], W1B[:])
            w2_sb = cp.tile([16, 160], F32R, tag="w2t")
            for t5 in range(5):
                nc.sync.dma_start(w2_sb[:, t5 * 32:(t5 + 1) * 32],
                                  W2T[t5, :, :])
            d1_sb = cp.tile([32, CONV2_LEN * 32], F32R, tag="d1r")
            nc.sync.dma_start(d1_sb[:], D1R[:])
            d2_sb = cp.tile([32, 2], F32R, tag="d2")
            nc.sync.dma_start(d2_sb[:], D2[:])
            diff_sb = cp.tile([2, 2], F32, tag="diff")
            nc.sync.dma_start(diff_sb[:], DIFF[:])
            b1_sb = cp.tile([16, 1], F32, tag="b1")
            nc.sync.dma_start(b1_sb[:], B1[:])
            b2_sb = cp.tile([32, 1], F32, tag="b2")
            nc.sync.dma_start(b2_sb[:], B2[:])
            bd1_sb = cp.tile([32, 1], F32, tag="bd1")
            nc.sync.dma_start(bd1_sb[:], BD1[:])
            bd2_sb = cp.tile([2, 1], F32, tag="bd2")
            nc.sync.dma_start(bd2_sb[:], BD2[:])
            for g in range(3, GPC):
                if fold_lin1:
                    lin1_sb.append(load_lin1(g))
                load_st(g)

            # per-graph state
            hgs = {}       # g -> [128, 4*512] tile (layers 1..4)
            vcols = {}     # g -> [128, 4] node-major h5
            vbs = {}       # g -> [128, 512] v broadcast
            ranks = {}     # g -> [128, 4]
            ptts = {}      # g -> [128, 4K] one-hot rank matrix
            idxw = {}      # g -> [128, 4] int16 wrapped indices
            # pooled features, all graphs side by side, one tile per layer
            pooled_sb = [cp.tile([128, GPC * K], F32R, tag=f"pool{l}",
                                 name=f"pool{l}")
                         for l in range(NLAYERS)]
            p5all = cp.tile([16, GPC * K], F32R, tag="p5all")
            y1p = psy.tile([16, GPC * K], F32, tag="y1p")
            y2all = cp.tile([32, GPC * CONV2_LEN], F32R, tag="y2all")
            y1 = cp.tile([16, GPC * K], F32, tag="y1")
            mp = cp.tile([16, GPC * K // 2], F32R, tag="mp")

            # ---------------- layer machinery ----------------
            def lin_stage(g, l):
                # returns SBUF [128, 4*128] chunk-major lin
                if l == 0 and fold_lin1:
                    return lin1_sb[g]
                lp = ps5.tile([128, NPG], F32, tag="linp", bufs=2)
                for cc in range(4):
                    if l == 0:
                        stat = h0[:, g * NPG + cc * 128:
                                  g * NPG + (cc + 1) * 128]
                    else:
                        stat = hgs[g][:, (l - 1) * NPG + cc * 128:
                                      (l - 1) * NPG + (cc + 1) * 128]
                    nc.tensor.matmul(
                        lp[:, cc * 128:(cc + 1) * 128], stat,
                        wc_sb[:, l * 128:(l + 1) * 128],
                        start=True, stop=True)
                ln = linp.tile([128, NPG], F32R, tag="lin")
                if (g + l) % 2 == 0:
                    nc.vector.tensor_copy(ln[:], lp[:])
                else:
                    nc.scalar.copy(ln[:], lp[:])
                return ln

            def prop_stage(g, l, ln):
                sp = ps5.tile([128, NPG], F32, tag="msgp", bufs=2)
                for cc in range(4):
                    nc.tensor.matmul(
                        sp[:], ln[:, cc * 128:(cc + 1) * 128],
                        st_sb[g][cc][:],
                        start=(cc == 0), stop=(cc == 3))
                nc.scalar.activation(
                    hgs[g][:, l * NPG:(l + 1) * NPG], sp[:], TANH)

            def alloc_h(g):
                hgs[g] = hp.tile([128, NLAYERS * NPG], F32R, tag="hg",
                                 name=f"h_{g}")

            def layers_pair(ga, gb):
                """All 4 GCN layers for graphs ga, gb, interleaved."""
                alloc_h(ga)
                alloc_h(gb)
                for l in range(NLAYERS):
                    lns = {g: lin_stage(g, l) for g in (ga, gb)}
                    for g in (ga, gb):
                        prop_stage(g, l, lns[g])
                    yield l

            # ---------------- tail stages ----------------
            lin5s = {}

            def sA(g):
                """layer-5 matvec (PE) + copy (DVE)."""
                hl = hgs[g]
                l5p = ps1.tile([128, 4], F32, tag="ps128")
                for cc in range(4):
                    nc.tensor.matmul(
                        l5p[:, cc:cc + 1],
                        hl[:, 3 * NPG + cc * 128:3 * NPG + (cc + 1) * 128],
                        w5_sb[:], start=True, stop=True)
                lin5 = smp.tile([128, 4], F32R, tag="lin5")
                nc.vector.tensor_copy(lin5[:], l5p[:])
                lin5s[g] = lin5

            def sB(g):
                """msg5 = S @ lin5 (PE, 16 tiny) + tanh (ACT)."""
                lin5 = lin5s[g]
                m5p = ps1.tile([128, 4], F32, tag="ps128")
                for dc in range(4):
                    for sc in range(4):
                        nc.tensor.matmul(
                            m5p[:, dc:dc + 1],
                            st_sb[g][sc][:, dc * 128:(dc + 1) * 128],
                            lin5[:, sc:sc + 1],
                            start=(sc == 0), stop=(sc == 3))
                vcol = smp.tile([128, 4], F32, tag="vcol")
                nc.scalar.activation(vcol[:], m5p[:], TANH)
                vcols[g] = vcol

            def sC(g):
                """h5 row form + broadcast."""
                vcol = vcols[g]
                vtp = ps1.tile([4, 128], F32R, tag="ps128")
                nc.tensor.transpose(vtp[:], vcol[:].bitcast(F32R), id_sb[:])
                vts = smp.tile([4, 128], F32R, tag="vts")
                nc.vector.tensor_copy(vts[:], vtp[:])
                h5r = smp.tile([1, NPG], F32R, tag="h5r", bufs=3)
                for cc in range(4):
                    nc.sync.dma_start(h5r[0:1, cc * 128:(cc + 1) * 128],
                                      vts[cc:cc + 1, :])
                vb = vbp.tile([128, NPG], F32R, tag="vb")
                nc.gpsimd.partition_broadcast(vb[:], h5r[0:1, :])
                vbs[g] = vb
                if debug:
                    nc.sync.dma_start(DBG_H5[g, :, :], h5r[:])

            def sD(g, dve_chunks=None):
                """exact stable ranks; dve_chunks picks per-chunk engine."""
                vb, vcol = vbs[g], vcols[g]
                if dve_chunks is None:
                    dve_chunks = (0, 1, 2, 3)
                rank = smp.tile([128, 4], F32, tag="rank")
                for cc in range(4):
                    eng = nc.vector if cc in dve_chunks else nc.gpsimd
                    t1 = scp.tile([128, NPG], F32, tag="tt")
                    ra = smp.tile([128, 2], F32, tag="ra")
                    eng.tensor_scalar(
                        out=t1[:], in0=vb[:], scalar1=vcol[:, cc:cc + 1],
                        scalar2=None, op0=IS_GT, op1=ADD,
                        accum_out=ra[:, 0:1])
                    t2 = scp.tile([128, NPG], F32, tag="tt")
                    eng.scalar_tensor_tensor(
                        out=t2[:], in0=vb[:], scalar=vcol[:, cc:cc + 1],
                        in1=mj_sb[:, cc * NPG:(cc + 1) * NPG],
                        op0=IS_EQ, op1=MULT, accum_out=ra[:, 1:2])
                    nc.vector.tensor_tensor(
                        out=rank[:, cc:cc + 1], in0=ra[:, 0:1],
                        in1=ra[:, 1:2], op=ADD)
                ranks[g] = rank
                if debug:
                    nc.sync.dma_start(DBG_RANK[g, :, :], rank[:])

            def sE(g):
                """one-hot rank matrix (columns in wrapped-permuted order)."""
                rank = ranks[g]
                ptt = ptp.tile([128, 4 * K], F32R, tag="pt")
                for cc in range(4):
                    nc.vector.tensor_scalar(
                        out=ptt[:, cc * K:(cc + 1) * K], in0=ki_sb[:],
                        scalar1=rank[:, cc:cc + 1], scalar2=None, op0=IS_EQ)
                ptts[g] = ptt

            def sF(g):
                """ordered top-64 node indices, int16 wrapped for ap_gather."""
                ptt = ptts[g]
                # col64[q] = index of the node with rank perm(q); KI's
                # permutation makes the downstream folds land each index at
                # iw[p, s] = idx[s*16+p], the ap_gather wrapped layout.
                cxp = ps1.tile([K, 1], F32, tag="ps128")
                for cc in range(4):
                    nc.tensor.matmul(cxp[:], ptt[:, cc * K:(cc + 1) * K],
                                     iota_sb[:, cc:cc + 1],
                                     start=(cc == 0), stop=(cc == 3))
                c64 = smp.tile([K, 1], F32, tag="c64")
                nc.vector.tensor_copy(c64[:], cxp[:])
                m64 = smp.tile([K, 4], F32R, tag="m64")
                nc.vector.tensor_scalar(out=m64[:], in0=qm_sb[:],
                                        scalar1=c64[:, 0:1], scalar2=None,
                                        op0=MULT)
                wqp = ps1.tile([16, 4], F32, tag="ps128")
                nc.tensor.matmul(wqp[:], e1_sb[:], m64[:],
                                 start=True, stop=True)
                wq = smp.tile([16, 4], F32R, tag="wq")
                nc.vector.tensor_copy(wq[:], wqp[:])
                wfp = ps1.tile([128, 4], F32, tag="ps128")
                nc.tensor.matmul(wfp[:], r_sb[:], wq[:],
                                 start=True, stop=True)
                iw = idxp_pool.tile([128, 4], I16, tag="iw")
                nc.vector.tensor_copy(iw[:], wfp[:])
                idxw[g] = iw
                if debug:
                    dbgi = smp.tile([128, 4], F32, tag="dbgi")
                    nc.vector.tensor_copy(dbgi[:], iw[:])
                    nc.sync.dma_start(DBG_IDX[g, :, :], dbgi[:])

            def sG(g, pool5_on_pe=False):
                """gather pooled features on the Pool engine."""
                iw = idxw[g]
                hl = hgs[g]
                for l in range(NLAYERS):
                    nc.gpsimd.ap_gather(
                        pooled_sb[l][:, g * K:(g + 1) * K],
                        hl[:, l * NPG:(l + 1) * NPG], iw[:],
                        channels=128, num_elems=NPG, d=1, num_idxs=K)
                if pool5_on_pe:
                    # v[idx] via selection matmul; undo the column
                    # permutation with a strided copy out of PSUM.
                    vcol, ptt = vcols[g], ptts[g]
                    p5p = ps1.tile([1, K], F32, tag="ps128")
                    for cc in range(4):
                        nc.tensor.matmul(p5p[:],
                                         vcol[:, cc:cc + 1].bitcast(F32R),
                                         ptt[:, cc * K:(cc + 1) * K],
                                         start=(cc == 0), stop=(cc == 3))
                    dstv = p5all[0:1, g * K:(g + 1) * K].rearrange(
                        "a (s p) -> a p s", p=16)
                    srcv = p5p[0:1, :].rearrange("a (p s) -> a p s", s=4)
                    nc.vector.tensor_copy(dstv, srcv)
                else:
                    nc.gpsimd.ap_gather(
                        p5all[:, g * K:(g + 1) * K], vbs[g][0:16, :],
                        iw[0:16, :], channels=16, num_elems=NPG, d=1,
                        num_idxs=K)

            xts = {}

            def sSelXt(g, alt_eng=None):
                """node-major h chunks via PE transpose (endgame only);
                4 chunk transposes land in one PSUM tile -> one wide copy."""
                hl = hgs[g]
                lst = []
                for l in range(NLAYERS):
                    tg = "linp" if l % 2 == 0 else "msgp"
                    xp = ps5.tile([128, NPG], F32R, tag=tg, bufs=2)
                    for cc in range(4):
                        nc.tensor.transpose(
                            xp[:, cc * 128:(cc + 1) * 128],
                            hl[:, l * NPG + cc * 128:
                               l * NPG + (cc + 1) * 128], id_sb[:])
                    xt = xsp.tile([128, NPG], F32R, tag="xtb", bufs=6)
                    if alt_eng is not None and l % 2 == 1:
                        alt_eng.tensor_copy(xt[:], xp[:])
                    else:
                        nc.scalar.copy(xt[:], xp[:])
                    lst.append(xt)
                xts[g] = lst

            def sSelMM(g):
                """selection matmuls; ptt columns are permuted, so the
                copies to pooled_sb/p5all unpermute via strided views."""
                ptt, vcol = ptts[g], vcols[g]
                for l in range(NLAYERS):
                    tg = "linp" if l % 2 == 0 else "msgp"
                    pp = ps5.tile([128, K], F32, tag=tg, bufs=2)
                    for cc in range(4):
                        nc.tensor.matmul(pp[:],
                                         xts[g][l][:, cc * 128:(cc + 1) * 128],
                                         ptt[:, cc * K:(cc + 1) * K],
                                         start=(cc == 0), stop=(cc == 3))
                    dstv = pooled_sb[l][:, g * K:(g + 1) * K].rearrange(
                        "c (s p) -> c p s", p=16)
                    srcv = pp[:].rearrange("c (p s) -> c p s", s=4)
                    nc.scalar.copy(dstv, srcv)
                p5p = ps1.tile([1, K], F32, tag="ps128")
                for cc in range(4):
                    nc.tensor.matmul(p5p[:],
                                     vcol[:, cc:cc + 1].bitcast(F32R),
                                     ptt[:, cc * K:(cc + 1) * K],
                                     start=(cc == 0), stop=(cc == 3))
                dstv = p5all[0:1, g * K:(g + 1) * K].rearrange(
                    "a (s p) -> a p s", p=16)
                srcv = p5p[0:1, :].rearrange("a (p s) -> a p s", s=4)
                nc.vector.tensor_copy(dstv, srcv)

            def sHa(g):
                """conv1 + relu + maxpool for graph g."""
                for l in range(NLAYERS):
                    nc.tensor.matmul(y1p[:, g * K:(g + 1) * K],
                                     w1a_sb[:, l * 16:(l + 1) * 16],
                                     pooled_sb[l][:, g * K:(g + 1) * K],
                                     start=(l == 0), stop=False)
                nc.tensor.matmul(y1p[:, g * K:(g + 1) * K], w1b_sb[:],
                                 p5all[0:1, g * K:(g + 1) * K],
                                 start=False, stop=True)
                nc.scalar.activation(y1[:, g * K:(g + 1) * K],
                                     y1p[:, g * K:(g + 1) * K], RELU,
                                     bias=b1_sb[:, 0:1])
                y1v = y1[:, g * K:(g + 1) * K].rearrange(
                    "p (a b) -> p a b", b=2)
                nc.vector.tensor_tensor(
                    out=mp[:, g * DD:(g + 1) * DD], in0=y1v[:, :, 0:1],
                    in1=y1v[:, :, 1:2], op=MAX)

            def sHb(g):
                """conv2 + relu for graph g."""
                y2p = ps1.tile([32, CONV2_LEN], F32, tag="ps128")
                for t5 in range(5):
                    nc.tensor.matmul(
                        y2p[:],
                        w2_sb[:, t5 * 32:(t5 + 1) * 32],
                        mp[:, g * DD + t5:g * DD + t5 + CONV2_LEN],
                        start=(t5 == 0), stop=(t5 == 4))
                nc.scalar.activation(
                    y2all[:, g * CONV2_LEN:(g + 1) * CONV2_LEN], y2p[:],
                    RELU, bias=b2_sb[:, 0:1])

            def sH(g):
                """per-graph conv1 + relu + maxpool + conv2 + relu."""
                for l in range(NLAYERS):
                    nc.tensor.matmul(y1p[:, g * K:(g + 1) * K],
                                     w1a_sb[:, l * 16:(l + 1) * 16],
                                     pooled_sb[l][:, g * K:(g + 1) * K],
                                     start=(l == 0), stop=False)
                nc.tensor.matmul(y1p[:, g * K:(g + 1) * K], w1b_sb[:],
                                 p5all[0:1, g * K:(g + 1) * K],
                                 start=False, stop=True)
                nc.scalar.activation(y1[:, g * K:(g + 1) * K],
                                     y1p[:, g * K:(g + 1) * K], RELU,
                                     bias=b1_sb[:, 0:1])
                y1v = y1[:, g * K:(g + 1) * K].rearrange(
                    "p (a b) -> p a b", b=2)
                nc.vector.tensor_tensor(
                    out=mp[:, g * DD:(g + 1) * DD], in0=y1v[:, :, 0:1],
                    in1=y1v[:, :, 1:2], op=MAX)
                y2p = ps1.tile([32, CONV2_LEN], F32, tag="ps128")
                for t5 in range(5):
                    nc.tensor.matmul(
                        y2p[:],
                        w2_sb[:, t5 * 32:(t5 + 1) * 32],
                        mp[:, g * DD + t5:g * DD + t5 + CONV2_LEN],
                        start=(t5 == 0), stop=(t5 == 4))
                nc.scalar.activation(
                    y2all[:, g * CONV2_LEN:(g + 1) * CONV2_LEN], y2p[:],
                    RELU, bias=b2_sb[:, 0:1])


            # ---------------- schedule ----------------
            # pair p: own sA at l3; pair p-1 runs B,C,D,E at l0..l3;
            # pair p-2 runs F,G,H at l0..l2.
            NP = GPC // 2
            for p in range(NP):
                ga, gb = 2 * p, 2 * p + 1
                gen = layers_pair(ga, gb)
                for l in gen:
                    g1 = (2 * (p - 1), 2 * (p - 1) + 1) if p >= 1 else ()
                    g2 = (2 * (p - 2), 2 * (p - 2) + 1) if p >= 2 else ()
                    if l == 0:
                        for g in g1:
                            sB(g)
                        for g in g2:
                            sF(g)
                    elif l == 1:
                        for g in g1:
                            sC(g)
                        for g in g2:
                            sG(g)
                    elif l == 2:
                        for g in g1:
                            sD(g)
                    elif l == 3:
                        for g in g1:
                            sE(g)
                        if p < NP - 1:
                            for g in g2:
                                sH(g)
                            sA(ga)
                            sA(gb)
                        else:
                            # critical tail chain of the last pair jumps
                            # ahead of the non-critical conv heads in the
                            # ACT/DVE queues
                            sA(ga)
                            sA(gb)
                            sB(ga)
                            sB(gb)
                            sC(ga)
                            sC(gb)
                            for g in g2:
                                sH(g)
                            sSelXt(ga)
            # endgame: pair NP-2 needs F,G,H; pair NP-1 needs D..H
            # (its A-C stages were hoisted into the last layer slot).
            q2 = (2 * (NP - 2), 2 * (NP - 2) + 1)
            q3 = (2 * (NP - 1), 2 * (NP - 1) + 1)
            sD(q3[0])
            sE(q3[0])
            sD(q3[1])
            sE(q3[1])
            sF(q2[0])
            sF(q2[1])
            sG(q2[0])
            sG(q2[1])
            sSelMM(q3[0])
            sSelXt(q3[1], alt_eng=nc.vector)
            sH(q2[0])
            sH(q2[1])
            sSelMM(q3[1])
            sHa(q3[0])
            sHa(q3[1])
            sHb(q3[0])
            sHb(q3[1])

            if debug:
                for l in range(NLAYERS):
                    for g in range(GPC):
                        nc.sync.dma_start(
                            DBG_H[l, :, g * NPG:(g + 1) * NPG],
                            hgs[g][:, l * NPG:(l + 1) * NPG])
                    nc.sync.dma_start(DBG_POOL[l, :, :], pooled_sb[l][:])
                nc.sync.dma_start(DBG_POOL5[:], p5all[0:1, :])

            # ---------------- head (batched over graphs) ----------------
            y1 = smp.tile([16, GPC * K], F32, tag="y1", bufs=1)
            nc.scalar.activation(y1[:], y1p[:], RELU, bias=b1_sb[:, 0:1])
            mp = smp.tile([16, GPC * K // 2], F32R, tag="mp", bufs=1)
            y1v = y1[:].rearrange("p (a b) -> p a b", b=2)
            nc.vector.tensor_tensor(out=mp[:], in0=y1v[:, :, 0:1],
                                    in1=y1v[:, :, 1:2], op=MAX)
            y2p = ps1.tile([32, GPC * CONV2_LEN], F32, tag="y2p", bufs=1)
            for g in range(GPC):
                for t5 in range(5):
                    nc.tensor.matmul(
                        y2p[:, g * CONV2_LEN:(g + 1) * CONV2_LEN],
                        w2_sb[:, t5 * 32:(t5 + 1) * 32],
                        mp[:, g * DD + t5:g * DD + t5 + CONV2_LEN],
                        start=(t5 == 0), stop=(t5 == 4))
            nc.scalar.activation(y2all[:], y2p[:], RELU, bias=b2_sb[:, 0:1])
            if debug:
                nc.sync.dma_start(DBG_Y2[:], y2all[:])

            h1p = ps1.tile([32, GPC], F32, tag="ps128")
            y2v = y2all[:].rearrange("p (g t) -> p g t", t=CONV2_LEN)
            for t5 in range(CONV2_LEN):
                nc.tensor.matmul(h1p[:], d1_sb[:, t5 * 32:(t5 + 1) * 32],
                                 y2v[:, :, t5:t5 + 1],
                                 start=(t5 == 0), stop=(t5 == CONV2_LEN - 1))
            h1s = smp.tile([32, GPC], F32R, tag="h1s", bufs=1)
            nc.scalar.activation(h1s[:], h1p[:], RELU, bias=bd1_sb[:, 0:1])
            dfp = ps1.tile([2, GPC], F32, tag="ps128")
            nc.tensor.matmul(dfp[:], d2_sb[:], h1s[:], start=True, stop=True)
            pr = smp.tile([2, GPC], F32, tag="pr", bufs=1)
            nc.scalar.activation(pr[:], dfp[:], SIGM, bias=bd2_sb[:, 0:1])
            nc.sync.dma_start(OUT[:], pr[:])

    nc.compile()
    return nc


def _get_nc(fold_lin1, debug):
    key = (fold_lin1, debug)
    if key not in _NC_CACHE:
        _NC_CACHE[key] = _build(fold_lin1, debug)
    return _NC_CACHE[key]


def prepare_host(inputs, fold_lin1=True):
    """All host-side index preprocessing + per-core input maps."""
    x = np.asarray(inputs["x"]).astype(np.int64)
    edge_index = np.asarray(inputs["edge_index"]).astype(np.int64)
    emb = np.ascontiguousarray(np.asarray(inputs["emb"], dtype=np.float32))
    W_convs = np.asarray(inputs["W_convs"], dtype=np.float32)
    conv1_w = np.asarray(inputs["conv1_w"], dtype=np.float32)
    conv1_b = np.asarray(inputs["conv1_b"], dtype=np.float32)
    conv2_w = np.asarray(inputs["conv2_w"], dtype=np.float32)
    conv2_b = np.asarray(inputs["conv2_b"], dtype=np.float32)
    d1_w = np.asarray(inputs["d1_w"], dtype=np.float32)
    d1_b = np.asarray(inputs["d1_b"], dtype=np.float32)
    d2_w = np.asarray(inputs["d2_w"], dtype=np.float32)
    d2_b = np.asarray(inputs["d2_b"], dtype=np.float32)
    W_last = np.asarray(inputs["W_last"], dtype=np.float32)

    src, dst = edge_index[0], edge_index[1]
    deg = (np.bincount(src, minlength=N_TOTAL) + 1).astype(np.float32)
    invdeg = (np.float32(1.0) / deg).astype(np.float32)
    gid = dst >> 9
    flat = (gid * NPG + (dst & 511)) * NPG + (src & 511)
    A = np.bincount(flat, minlength=NUM_GRAPHS * NPG * NPG)
    A = A.astype(np.float32).reshape(NUM_GRAPHS, NPG, NPG)
    idx = np.arange(NPG)
    A[:, idx, idx] += 1.0
    S = A * invdeg.reshape(NUM_GRAPHS, NPG, 1)
    ST = np.ascontiguousarray(S.transpose(0, 2, 1)).reshape(
        NUM_GRAPHS, 4, 128, NPG)

    w1 = np.ascontiguousarray(conv1_w[:, 0, :].T)  # [513, 16]
    shared = {
        "WC": np.ascontiguousarray(W_convs),
        "W5": np.ascontiguousarray(W_last),
        "IDN": np.eye(128, dtype=ml_dtypes.bfloat16),
        "W1A": np.ascontiguousarray(w1[:512].reshape(4, 128, 16)),
        "W1B": np.ascontiguousarray(w1[512:513]),
        "W2T": np.ascontiguousarray(conv2_w.transpose(2, 1, 0)),
        "D1R": np.ascontiguousarray(d1_w.reshape(DD, CONV2_LEN * 32)
                                    .astype(np.float32)),
        "D2": np.ascontiguousarray(
            (d2_w.astype(np.float64)
             @ np.array([[1.0, -1.0], [-1.0, 1.0]])).astype(np.float32)),
        "DIFF": np.array([[1.0, -1.0], [-1.0, 1.0]], dtype=np.float32),
        "B1": np.ascontiguousarray(conv1_b.reshape(16, 1)),
        "B2": np.ascontiguousarray(conv2_b.reshape(32, 1)),
        "BD1": np.ascontiguousarray(d1_b.reshape(32, 1)),
        "BD2": np.ascontiguousarray(
            (np.array([[1.0, -1.0], [-1.0, 1.0]])
             @ d2_b.reshape(2, 1)).astype(np.float32)),
        "KI": np.ascontiguousarray(np.broadcast_to(
            ((np.arange(K) % 4) * 16 + np.arange(K) // 4)
            .astype(np.float32), (128, K))),
        "MJ": np.ascontiguousarray(
            (np.arange(NPG)[None, None, :]
             < (np.arange(4)[:, None, None] * 128
                + np.arange(128)[None, :, None])).astype(np.float32)),
        "IOTA": np.ascontiguousarray(
            (np.arange(4)[None, :] * 128
             + np.arange(128)[:, None]).astype(np.float32)),
        "QMASK": np.ascontiguousarray(
            (np.arange(64)[:, None] % 4 == np.arange(4)[None, :])
            .astype(np.float32)),
        "E1M": np.ascontiguousarray(
            (np.arange(64)[:, None] // 4 == np.arange(16)[None, :])
            .astype(np.float32)),
        "RM": np.ascontiguousarray(
            (np.arange(128)[None, :] % 16 == np.arange(16)[:, None])
            .astype(np.float32)),
    }

    if fold_lin1:
        lin1 = emb @ W_convs[0]        # [1000, 128] f32
        h0lin = lin1[x]                # [N, 128]
    h0 = emb[x]

    in_maps = []
    for c in range(NCORES):
        m = dict(shared)
        if fold_lin1:
            loc = h0lin[c * NLOC:(c + 1) * NLOC]  # [4096, 128]
            # [g][p][cc*128+f] = lin1[g*512+cc*128+p, f]
            lm = loc.reshape(GPC, 4, 128, 128).transpose(0, 2, 1, 3)
            m["LIN1M"] = np.ascontiguousarray(
                lm.reshape(GPC, 128, NPG))
        else:
            m["H0T"] = np.ascontiguousarray(h0[c * NLOC:(c + 1) * NLOC].T)
        m["STD"] = np.ascontiguousarray(ST[c * GPC:(c + 1) * GPC])
        in_maps.append(m)
    return in_maps


def run(inputs, fold_lin1=True, debug=False, **spmd_kwargs):
    in_maps = prepare_host(inputs, fold_lin1)
    nc = _get_nc(fold_lin1, debug)
    res = run_bass_kernel_spmd(nc, in_maps, core_ids=list(range(NCORES)),
                               **spmd_kwargs)
    out = np.empty((NUM_GRAPHS, 2), dtype=np.float32)
    for c in range(NCORES):
        out[c * GPC:(c + 1) * GPC, :] = res.results[c]["OUT"].T
    return out, res


def kernel(**inputs):
    out, _ = run(inputs, fold_lin1=True)
    return out



# revision 19
# speedup vs baseline: 1.4212x; 1.1331x over previous
"""DGCNN (gnn_message_passing) Trainium2 Bass kernel, v2.

Strategy (data-parallel over graphs, 8 graphs per NeuronCore):
  - Host builds, per graph, the dense normalized propagation operator
    S[d, s] = (mult(s->d) + I) / deg_out[d]  (512x512 f32), shipped
    transposed as 4 chunks of [128, 512].
  - Layer-1 linear is folded on the host: LIN1 = (emb @ W1)[x] is shipped
    instead of h0 (flag FOLD_LIN1; layer-1 lin matmuls are skipped).
  - Graphs processed in pairs, graph-outer: each pair runs its 4 GCN
    layers on PE while the previous pair's sort-pooling tail runs on
    DVE/ACT/Pool, so the tail is hidden under PE time.
  - Per layer+graph on device: lin = h @ W as 4 chunk matmuls into one
    [128,512] PSUM bank, one copy to SBUF, then msgT = lin^T-chunks
    stationary x S^T chunks -> [128f, 512d] PSUM, tanh -> h (f32
    throughout: the top-64 sort is sensitive to ~1e-8 noise in h5, so
    the whole h chain must be bit-stable f32).
  - Sort-pooling tail per graph: exact stable ranks via comparison
    matrices (DVE/Pool alternating), one-hot rank matrix PT, top-64
    node indices extracted with a tiny iota matmul, converted to the
    wrapped int16 layout, and the pooled features gathered from h with
    gpsimd ap_gather (Pool engine) - no PE transposes or selection
    matmuls.
  - Conv1/maxpool/conv2/dense head batched across the 8 graphs.

Self-contained: hardcodes all shapes; no reads of /root/problem files.
"""

import sys

if "/opt/trn_rl_repo" not in sys.path:
    sys.path.insert(0, "/opt/trn_rl_repo")

import ml_dtypes
import numpy as np

import concourse.bacc as bacc
import concourse.mybir as mybir
import concourse.tile as tile
from concourse.bass_utils import run_bass_kernel_spmd

F32 = mybir.dt.float32
F32R = mybir.dt.float32r  # same bits/numerics as f32; 4x PE rate at >=256 cols
I16 = mybir.dt.int16
BF16 = mybir.dt.bfloat16

NUM_GRAPHS = 64
NPG = 512  # nodes per graph
N_TOTAL = NUM_GRAPHS * NPG
EMB = 128
DIMF = 128
NLAYERS = 4
K = 64
NCORES = 8
GPC = NUM_GRAPHS // NCORES  # graphs per core = 8
NLOC = GPC * NPG  # local nodes = 4096
LATENT = NLAYERS * DIMF + 1  # 513
DD = (K - 2) // 2 + 1  # 32
CONV2_LEN = DD - 5 + 1  # 28

HOT_W = 577     # wc 512 | w5 1 | idn(bf16 packed) 64
COLD_W = 3406   # mj 2048 | ki 64 | iota 4 | w1a 64 | w1b 16 | w2 160 |
                # d1 896 | d2 2 | rm 128 | e1 16 | qm 4 | b1 b2 bd1 bd2 4

_NC_CACHE = {}


def _build(fold_lin1, debug):
    """Trace + compile the per-core Bass program (same on all 8 cores)."""
    nc = bacc.Bacc("TRN2", target_bir_lowering=False, debug=False,
                   num_devices=NCORES)

    # ---- per-core DRAM I/O ----
    # LIN1M[g] = (emb @ W1)[x] for graph g, chunk-major: [g][p][cc*128+f]
    #          = lin1[node cc*128+p of graph g, feat f]
    if fold_lin1:
        LIN1M = nc.dram_tensor("LIN1M", [GPC, 128, NPG], F32R,
                               kind="ExternalInput")
    else:
        H0T = nc.dram_tensor("H0T", [128, NLOC], F32, kind="ExternalInput")
    STD = nc.dram_tensor("STD", [GPC, 4, 128, NPG], F32R, kind="ExternalInput")
    # all small constants packed into two blobs (1 DMA each): HWDGE serial
    # overhead is ~630ns per DMA instruction, so DMA count dominates tiny
    # transfers. Layout documented in prepare_host.
    HOT = nc.dram_tensor("HOT", [128, HOT_W], F32R, kind="ExternalInput")
    COLD = nc.dram_tensor("COLD", [128, COLD_W], F32R, kind="ExternalInput")
    OUT = nc.dram_tensor("OUT", [2, GPC], F32, kind="ExternalOutput")
    if debug:
        DBG_H = nc.dram_tensor("DBG_H", [NLAYERS, 128, NLOC], F32,
                               kind="ExternalOutput")
        DBG_H5 = nc.dram_tensor("DBG_H5", [GPC, 1, NPG], F32,
                                kind="ExternalOutput")
        DBG_RANK = nc.dram_tensor("DBG_RANK", [GPC, 128, 4], F32,
                                  kind="ExternalOutput")
        DBG_IDX = nc.dram_tensor("DBG_IDX", [GPC, 128, 4], F32,
                                 kind="ExternalOutput")
        DBG_POOL = nc.dram_tensor("DBG_POOL", [NLAYERS, 128, GPC * K], F32,
                                  kind="ExternalOutput")
        DBG_POOL5 = nc.dram_tensor("DBG_POOL5", [1, GPC * K], F32,
                                   kind="ExternalOutput")
        DBG_Y2 = nc.dram_tensor("DBG_Y2", [32, GPC * CONV2_LEN], F32,
                                kind="ExternalOutput")

    TANH = mybir.ActivationFunctionType.Tanh
    RELU = mybir.ActivationFunctionType.Relu
    SIGM = mybir.ActivationFunctionType.Sigmoid
    ADD = mybir.AluOpType.add
    MULT = mybir.AluOpType.mult
    MAX = mybir.AluOpType.max
    IS_GT = mybir.AluOpType.is_gt
    IS_EQ = mybir.AluOpType.is_equal

    with tile.TileContext(nc) as tc:
        with (
            tc.tile_pool(name="const", bufs=1) as cp,
            tc.tile_pool(name="hg", bufs=6) as hp,          # [128,2048]/graph
            tc.tile_pool(name="st", bufs=32) as stp,        # [128,512]/chunk
            tc.tile_pool(name="lin", bufs=4) as linp,       # [128,512]
            tc.tile_pool(name="sc", bufs=3) as scp,         # compare scratch
            tc.tile_pool(name="vbp", bufs=3) as vbp,        # v broadcast
            tc.tile_pool(name="ptp", bufs=2) as ptp,        # ptt one-hots
            tc.tile_pool(name="sm", bufs=4) as smp,        # small tiles
            tc.tile_pool(name="idx", bufs=3) as idxp_pool,  # idx wrapped
            tc.tile_pool(name="xs", bufs=20) as xsp,        # endgame sel xts
            tc.tile_pool(name="ps512", bufs=2, space="PSUM") as ps5,
            tc.tile_pool(name="ps128", bufs=2, space="PSUM") as ps1,
            tc.tile_pool(name="psy1", bufs=1, space="PSUM") as psy,
        ):
            # ---- load order: pair-0 working set first ----
            lin1_sb = []

            def load_lin1(g):
                if fold_lin1:
                    t = linp.tile([128, NPG], F32R, tag="lin1", bufs=GPC,
                                  name=f"lin1_{g}")
                    nc.sync.dma_start(t[:], LIN1M[g, :, :])
                    return t
                return None

            st_sb = {}

            def load_st(g):
                # one DMA per graph; DRAM src iterated [p][c][w] to match
                # the [128, 4*512] chunk-major SBUF layout
                t = stp.tile([128, 4 * NPG], F32R, tag="st", bufs=8,
                             name=f"st_t{g}")
                nc.sync.dma_start(t[:].rearrange("p (c w) -> p c w", c=4),
                                  STD[g].rearrange("c p w -> p c w"))
                st_sb[g] = [t[:, c * NPG:(c + 1) * NPG] for c in range(4)]

            if fold_lin1:
                lin1_sb = [load_lin1(0)]
                load_st(0)
                lin1_sb.append(load_lin1(1))
                load_st(1)
            else:
                h0 = hp.tile([128, NLOC], F32, tag="h0x", bufs=1)
                for s in range(0, 8):
                    nc.sync.dma_start(h0[:, s * 512:(s + 1) * 512],
                                      H0T[:, s * 512:(s + 1) * 512])
                load_st(0)
                load_st(1)
            hot = cp.tile([128, HOT_W], F32R, tag="hot")
            nc.sync.dma_start(hot[:], HOT[:])
            wc_sb = hot[:, 0:512]
            w5_sb = hot[:, 512:513]
            id_sb = hot[:, 513:577].bitcast(BF16)
            if fold_lin1:
                lin1_sb.append(load_lin1(2))
            load_st(2)
            cold = cp.tile([128, COLD_W], F32R, tag="cold")
            nc.sync.dma_start(cold[:], COLD[:])
            mj_sb = cold[:, 0:2048].bitcast(F32)
            ki_sb = cold[:, 2048:2112].bitcast(F32)
            iota_sb = cold[:, 2112:2116]
            w1a_sb = cold[:, 2116:2180]
            w1b_sb = cold[0:1, 2180:2196]
            w2_sb = cold[0:16, 2196:2356]
            d1_sb = cold[0:32, 2356:3252]
            d2_sb = cold[0:32, 3252:3254]
            r_sb = cold[0:16, 3254:3382]
            e1_sb = cold[0:64, 3382:3398]
            qm_sb = cold[0:64, 3398:3402]
            b1_sb = cold[0:16, 3402:3403].bitcast(F32)
            b2_sb = cold[0:32, 3403:3404].bitcast(F32)
            bd1_sb = cold[0:32, 3404:3405].bitcast(F32)
            bd2_sb = cold[0:2, 3405:3406].bitcast(F32)
            for g in range(3, GPC):
                if fold_lin1:
                    lin1_sb.append(load_lin1(g))
                load_st(g)

            # per-graph state
            hgs = {}       # g -> [128, 4*512] tile (layers 1..4)
            vcols = {}     # g -> [128, 4] node-major h5
            vbs = {}       # g -> [128, 512] v broadcast
            ranks = {}     # g -> [128, 4]
            ptts = {}      # g -> [128, 4K] one-hot rank matrix
            idxw = {}      # g -> [128, 4] int16 wrapped indices
            # pooled features, all graphs side by side, one tile per layer
            pooled_sb = [cp.tile([128, GPC * K], F32R, tag=f"pool{l}",
                                 name=f"pool{l}")
                         for l in range(NLAYERS)]
            p5all = cp.tile([16, GPC * K], F32R, tag="p5all")
            y1p = psy.tile([16, GPC * K], F32, tag="y1p")
            y2all = cp.tile([32, GPC * CONV2_LEN], F32R, tag="y2all")
            y1 = cp.tile([16, GPC * K], F32, tag="y1")
            mp = cp.tile([16, GPC * K // 2], F32R, tag="mp")

            # ---------------- layer machinery ----------------
            def lin_stage(g, l):
                # returns SBUF [128, 4*128] chunk-major lin
                if l == 0 and fold_lin1:
                    return lin1_sb[g]
                lp = ps5.tile([128, NPG], F32, tag="linp", bufs=2)
                for cc in range(4):
                    if l == 0:
                        stat = h0[:, g * NPG + cc * 128:
                                  g * NPG + (cc + 1) * 128]
                    else:
                        stat = hgs[g][:, (l - 1) * NPG + cc * 128:
                                      (l - 1) * NPG + (cc + 1) * 128]
                    nc.tensor.matmul(
                        lp[:, cc * 128:(cc + 1) * 128], stat,
                        wc_sb[:, l * 128:(l + 1) * 128],
                        start=True, stop=True)
                ln = linp.tile([128, NPG], F32R, tag="lin")
                if (g + l) % 2 == 0:
                    nc.vector.tensor_copy(ln[:], lp[:])
                else:
                    nc.scalar.copy(ln[:], lp[:])
                return ln

            def prop_stage(g, l, ln):
                sp = ps5.tile([128, NPG], F32, tag="msgp", bufs=2)
                for cc in range(4):
                    nc.tensor.matmul(
                        sp[:], ln[:, cc * 128:(cc + 1) * 128],
                        st_sb[g][cc][:],
                        start=(cc == 0), stop=(cc == 3))
                nc.scalar.activation(
                    hgs[g][:, l * NPG:(l + 1) * NPG], sp[:], TANH)

            def alloc_h(g):
                hgs[g] = hp.tile([128, NLAYERS * NPG], F32R, tag="hg",
                                 name=f"h_{g}")

            def layers_pair(ga, gb):
                """All 4 GCN layers for graphs ga, gb, interleaved."""
                alloc_h(ga)
                alloc_h(gb)
                for l in range(NLAYERS):
                    lns = {g: lin_stage(g, l) for g in (ga, gb)}
                    for g in (ga, gb):
                        prop_stage(g, l, lns[g])
                    yield l

            # ---------------- tail stages ----------------
            lin5s = {}

            def sA(g):
                """layer-5 matvec (PE) + copy (DVE)."""
                hl = hgs[g]
                l5p = ps1.tile([128, 4], F32, tag="ps128")
                for cc in range(4):
                    nc.tensor.matmul(
                        l5p[:, cc:cc + 1],
                        hl[:, 3 * NPG + cc * 128:3 * NPG + (cc + 1) * 128],
                        w5_sb[:], start=True, stop=True)
                lin5 = smp.tile([128, 4], F32R, tag="lin5")
                nc.vector.tensor_copy(lin5[:], l5p[:])
                lin5s[g] = lin5

            def sB(g):
                """msg5 = S @ lin5 (PE, 16 tiny) + tanh (ACT)."""
                lin5 = lin5s[g]
                m5p = ps1.tile([128, 4], F32, tag="ps128")
                for dc in range(4):
                    for sc in range(4):
                        nc.tensor.matmul(
                            m5p[:, dc:dc + 1],
                            st_sb[g][sc][:, dc * 128:(dc + 1) * 128],
                            lin5[:, sc:sc + 1],
                            start=(sc == 0), stop=(sc == 3))
                vcol = smp.tile([128, 4], F32, tag="vcol")
                nc.scalar.activation(vcol[:], m5p[:], TANH)
                vcols[g] = vcol

            def sC(g):
                """h5 row form + broadcast."""
                vcol = vcols[g]
                vtp = ps1.tile([4, 128], F32R, tag="ps128")
                nc.tensor.transpose(vtp[:], vcol[:].bitcast(F32R), id_sb[:])
                vts = smp.tile([4, 128], F32R, tag="vts")
                nc.vector.tensor_copy(vts[:], vtp[:])
                h5r = smp.tile([1, NPG], F32R, tag="h5r", bufs=3)
                # single cross-partition DMA: flat AP order pairs
                # vts[c][w] -> h5r col c*128+w
                nc.sync.dma_start(h5r[0:1, :], vts[:])
                vb = vbp.tile([128, NPG], F32R, tag="vb")
                nc.gpsimd.partition_broadcast(vb[:], h5r[0:1, :])
                vbs[g] = vb
                if debug:
                    nc.sync.dma_start(DBG_H5[g, :, :], h5r[:])

            def sD(g, dve_chunks=None):
                """exact stable ranks; dve_chunks picks per-chunk engine."""
                vb, vcol = vbs[g], vcols[g]
                if dve_chunks is None:
                    dve_chunks = (0, 1, 2, 3)
                rank = smp.tile([128, 4], F32, tag="rank")
                for cc in range(4):
                    eng = nc.vector if cc in dve_chunks else nc.gpsimd
                    t1 = scp.tile([128, NPG], F32, tag="tt")
                    ra = smp.tile([128, 2], F32, tag="ra")
                    eng.tensor_scalar(
                        out=t1[:], in0=vb[:], scalar1=vcol[:, cc:cc + 1],
                        scalar2=None, op0=IS_GT, op1=ADD,
                        accum_out=ra[:, 0:1])
                    t2 = scp.tile([128, NPG], F32, tag="tt")
                    eng.scalar_tensor_tensor(
                        out=t2[:], in0=vb[:], scalar=vcol[:, cc:cc + 1],
                        in1=mj_sb[:, cc * NPG:(cc + 1) * NPG],
                        op0=IS_EQ, op1=MULT, accum_out=ra[:, 1:2])
                    nc.vector.tensor_tensor(
                        out=rank[:, cc:cc + 1], in0=ra[:, 0:1],
                        in1=ra[:, 1:2], op=ADD)
                ranks[g] = rank
                if debug:
                    nc.sync.dma_start(DBG_RANK[g, :, :], rank[:])

            def sE(g):
                """one-hot rank matrix (columns in wrapped-permuted order)."""
                rank = ranks[g]
                ptt = ptp.tile([128, 4 * K], F32R, tag="pt")
                for cc in range(4):
                    nc.vector.tensor_scalar(
                        out=ptt[:, cc * K:(cc + 1) * K], in0=ki_sb[:],
                        scalar1=rank[:, cc:cc + 1], scalar2=None, op0=IS_EQ)
                ptts[g] = ptt

            def sF(g):
                """ordered top-64 node indices, int16 wrapped for ap_gather."""
                ptt = ptts[g]
                # col64[q] = index of the node with rank perm(q); KI's
                # permutation makes the downstream folds land each index at
                # iw[p, s] = idx[s*16+p], the ap_gather wrapped layout.
                cxp = ps1.tile([K, 1], F32, tag="ps128")
                for cc in range(4):
                    nc.tensor.matmul(cxp[:], ptt[:, cc * K:(cc + 1) * K],
                                     iota_sb[:, cc:cc + 1],
                                     start=(cc == 0), stop=(cc == 3))
                c64 = smp.tile([K, 1], F32, tag="c64")
                nc.vector.tensor_copy(c64[:], cxp[:])
                m64 = smp.tile([K, 4], F32R, tag="m64")
                nc.vector.tensor_scalar(out=m64[:], in0=qm_sb[:],
                                        scalar1=c64[:, 0:1], scalar2=None,
                                        op0=MULT)
                wqp = ps1.tile([16, 4], F32, tag="ps128")
                nc.tensor.matmul(wqp[:], e1_sb[:], m64[:],
                                 start=True, stop=True)
                wq = smp.tile([16, 4], F32R, tag="wq")
                nc.vector.tensor_copy(wq[:], wqp[:])
                wfp = ps1.tile([128, 4], F32, tag="ps128")
                nc.tensor.matmul(wfp[:], r_sb[:], wq[:],
                                 start=True, stop=True)
                iw = idxp_pool.tile([128, 4], I16, tag="iw")
                nc.vector.tensor_copy(iw[:], wfp[:])
                idxw[g] = iw
                if debug:
                    dbgi = smp.tile([128, 4], F32, tag="dbgi")
                    nc.vector.tensor_copy(dbgi[:], iw[:])
                    nc.sync.dma_start(DBG_IDX[g, :, :], dbgi[:])

            def sG(g, pool5_on_pe=False):
                """gather pooled features on the Pool engine."""
                iw = idxw[g]
                hl = hgs[g]
                for l in range(NLAYERS):
                    nc.gpsimd.ap_gather(
                        pooled_sb[l][:, g * K:(g + 1) * K],
                        hl[:, l * NPG:(l + 1) * NPG], iw[:],
                        channels=128, num_elems=NPG, d=1, num_idxs=K)
                if pool5_on_pe:
                    # v[idx] via selection matmul; undo the column
                    # permutation with a strided copy out of PSUM.
                    vcol, ptt = vcols[g], ptts[g]
                    p5p = ps1.tile([1, K], F32, tag="ps128")
                    for cc in range(4):
                        nc.tensor.matmul(p5p[:],
                                         vcol[:, cc:cc + 1].bitcast(F32R),
                                         ptt[:, cc * K:(cc + 1) * K],
                                         start=(cc == 0), stop=(cc == 3))
                    dstv = p5all[0:1, g * K:(g + 1) * K].rearrange(
                        "a (s p) -> a p s", p=16)
                    srcv = p5p[0:1, :].rearrange("a (p s) -> a p s", s=4)
                    nc.vector.tensor_copy(dstv, srcv)
                else:
                    nc.gpsimd.ap_gather(
                        p5all[:, g * K:(g + 1) * K], vbs[g][0:16, :],
                        iw[0:16, :], channels=16, num_elems=NPG, d=1,
                        num_idxs=K)

            xts = {}

            def sSelXt(g, alt_eng=None):
                """node-major h chunks via PE transpose (endgame only);
                4 chunk transposes land in one PSUM tile -> one wide copy."""
                hl = hgs[g]
                lst = []
                for l in range(NLAYERS):
                    tg = "linp" if l % 2 == 0 else "msgp"
                    xp = ps5.tile([128, NPG], F32R, tag=tg, bufs=2)
                    for cc in range(4):
                        nc.tensor.transpose(
                            xp[:, cc * 128:(cc + 1) * 128],
                            hl[:, l * NPG + cc * 128:
                               l * NPG + (cc + 1) * 128], id_sb[:])
                    xt = xsp.tile([128, NPG], F32R, tag="xtb", bufs=6)
                    if alt_eng is not None and l % 2 == 1:
                        alt_eng.tensor_copy(xt[:], xp[:])
                    else:
                        nc.scalar.copy(xt[:], xp[:])
                    lst.append(xt)
                xts[g] = lst

            def sSelMM(g):
                """selection matmuls; ptt columns are permuted, so the
                copies to pooled_sb/p5all unpermute via strided views."""
                ptt, vcol = ptts[g], vcols[g]
                for l in range(NLAYERS):
                    tg = "linp" if l % 2 == 0 else "msgp"
                    pp = ps5.tile([128, K], F32, tag=tg, bufs=2)
                    for cc in range(4):
                        nc.tensor.matmul(pp[:],
                                         xts[g][l][:, cc * 128:(cc + 1) * 128],
                                         ptt[:, cc * K:(cc + 1) * K],
                                         start=(cc == 0), stop=(cc == 3))
                    dstv = pooled_sb[l][:, g * K:(g + 1) * K].rearrange(
                        "c (s p) -> c p s", p=16)
                    srcv = pp[:].rearrange("c (p s) -> c p s", s=4)
                    nc.scalar.copy(dstv, srcv)
                p5p = ps1.tile([1, K], F32, tag="ps128")
                for cc in range(4):
                    nc.tensor.matmul(p5p[:],
                                     vcol[:, cc:cc + 1].bitcast(F32R),
                                     ptt[:, cc * K:(cc + 1) * K],
                                     start=(cc == 0), stop=(cc == 3))
                dstv = p5all[0:1, g * K:(g + 1) * K].rearrange(
                    "a (s p) -> a p s", p=16)
                srcv = p5p[0:1, :].rearrange("a (p s) -> a p s", s=4)
                nc.vector.tensor_copy(dstv, srcv)

            def sHa(g):
                """conv1 + relu + maxpool for graph g."""
                for l in range(NLAYERS):
                    nc.tensor.matmul(y1p[:, g * K:(g + 1) * K],
                                     w1a_sb[:, l * 16:(l + 1) * 16],
                                     pooled_sb[l][:, g * K:(g + 1) * K],
                                     start=(l == 0), stop=False)
                nc.tensor.matmul(y1p[:, g * K:(g + 1) * K], w1b_sb[:],
                                 p5all[0:1, g * K:(g + 1) * K],
                                 start=False, stop=True)
                nc.scalar.activation(y1[:, g * K:(g + 1) * K],
                                     y1p[:, g * K:(g + 1) * K], RELU,
                                     bias=b1_sb[:, 0:1])
                y1v = y1[:, g * K:(g + 1) * K].rearrange(
                    "p (a b) -> p a b", b=2)
                nc.vector.tensor_tensor(
                    out=mp[:, g * DD:(g + 1) * DD], in0=y1v[:, :, 0:1],
                    in1=y1v[:, :, 1:2], op=MAX)

            def sHb(g):
                """conv2 + relu for graph g."""
                y2p = ps1.tile([32, CONV2_LEN], F32, tag="ps128")
                for t5 in range(5):
                    nc.tensor.matmul(
                        y2p[:],
                        w2_sb[:, t5 * 32:(t5 + 1) * 32],
                        mp[:, g * DD + t5:g * DD + t5 + CONV2_LEN],
                        start=(t5 == 0), stop=(t5 == 4))
                nc.scalar.activation(
                    y2all[:, g * CONV2_LEN:(g + 1) * CONV2_LEN], y2p[:],
                    RELU, bias=b2_sb[:, 0:1])

            def sH(g):
                """per-graph conv1 + relu + maxpool + conv2 + relu."""
                for l in range(NLAYERS):
                    nc.tensor.matmul(y1p[:, g * K:(g + 1) * K],
                                     w1a_sb[:, l * 16:(l + 1) * 16],
                                     pooled_sb[l][:, g * K:(g + 1) * K],
                                     start=(l == 0), stop=False)
                nc.tensor.matmul(y1p[:, g * K:(g + 1) * K], w1b_sb[:],
                                 p5all[0:1, g * K:(g + 1) * K],
                                 start=False, stop=True)
                nc.scalar.activation(y1[:, g * K:(g + 1) * K],
                                     y1p[:, g * K:(g + 1) * K], RELU,
                                     bias=b1_sb[:, 0:1])
                y1v = y1[:, g * K:(g + 1) * K].rearrange(
                    "p (a b) -> p a b", b=2)
                nc.vector.tensor_tensor(
                    out=mp[:, g * DD:(g + 1) * DD], in0=y1v[:, :, 0:1],
                    in1=y1v[:, :, 1:2], op=MAX)
                y2p = ps1.tile([32, CONV2_LEN], F32, tag="ps128")
                for t5 in range(5):
                    nc.tensor.matmul(
                        y2p[:],
                        w2_sb[:, t5 * 32:(t5 + 1) * 32],
                        mp[:, g * DD + t5:g * DD + t5 + CONV2_LEN],
                        start=(t5 == 0), stop=(t5 == 4))
                nc.scalar.activation(
                    y2all[:, g * CONV2_LEN:(g + 1) * CONV2_LEN], y2p[:],
                    RELU, bias=b2_sb[:, 0:1])


            # ---------------- schedule ----------------
            # pair p: own sA at l3; pair p-1 runs B,C,D,E at l0..l3;
            # pair p-2 runs F,G,H at l0..l2.
            NP = GPC // 2
            for p in range(NP):
                ga, gb = 2 * p, 2 * p + 1
                gen = layers_pair(ga, gb)
                for l in gen:
                    g1 = (2 * (p - 1), 2 * (p - 1) + 1) if p >= 1 else ()
                    g2 = (2 * (p - 2), 2 * (p - 2) + 1) if p >= 2 else ()
                    if l == 0:
                        for g in g1:
                            sB(g)
                        for g in g2:
                            sF(g)
                    elif l == 1:
                        for g in g1:
                            sC(g)
                        for g in g2:
                            sG(g)
                    elif l == 2:
                        for g in g1:
                            sD(g)
                    elif l == 3:
                        for g in g1:
                            sE(g)
                        if p < NP - 1:
                            for g in g2:
                                sH(g)
                            sA(ga)
                            sA(gb)
                        else:
                            # critical tail chain of the last pair jumps
                            # ahead of the non-critical conv heads in the
                            # ACT/DVE queues
                            sA(ga)
                            sA(gb)
                            sB(ga)
                            sB(gb)
                            sC(ga)
                            sC(gb)
                            for g in g2:
                                sH(g)
                            sSelXt(ga)
            # endgame: pair NP-2 needs F,G,H; pair NP-1 needs D..H
            # (its A-C stages were hoisted into the last layer slot).
            q2 = (2 * (NP - 2), 2 * (NP - 2) + 1)
            q3 = (2 * (NP - 1), 2 * (NP - 1) + 1)
            sD(q3[0])
            sE(q3[0])
            sD(q3[1])
            sE(q3[1])
            sF(q2[0])
            sF(q2[1])
            sG(q2[0])
            sG(q2[1])
            sSelMM(q3[0])
            sSelXt(q3[1], alt_eng=nc.vector)
            sH(q2[0])
            sH(q2[1])
            sSelMM(q3[1])
            sHa(q3[0])
            sHa(q3[1])
            sHb(q3[0])
            sHb(q3[1])

            if debug:
                for l in range(NLAYERS):
                    for g in range(GPC):
                        nc.sync.dma_start(
                            DBG_H[l, :, g * NPG:(g + 1) * NPG],
                            hgs[g][:, l * NPG:(l + 1) * NPG])
                    nc.sync.dma_start(DBG_POOL[l, :, :], pooled_sb[l][:])
                nc.sync.dma_start(DBG_POOL5[:], p5all[0:1, :])

            # ---------------- head (batched over graphs) ----------------
            y1 = smp.tile([16, GPC * K], F32, tag="y1", bufs=1)
            nc.scalar.activation(y1[:], y1p[:], RELU, bias=b1_sb[:, 0:1])
            mp = smp.tile([16, GPC * K // 2], F32R, tag="mp", bufs=1)
            y1v = y1[:].rearrange("p (a b) -> p a b", b=2)
            nc.vector.tensor_tensor(out=mp[:], in0=y1v[:, :, 0:1],
                                    in1=y1v[:, :, 1:2], op=MAX)
            y2p = ps1.tile([32, GPC * CONV2_LEN], F32, tag="y2p", bufs=1)
            for g in range(GPC):
                for t5 in range(5):
                    nc.tensor.matmul(
                        y2p[:, g * CONV2_LEN:(g + 1) * CONV2_LEN],
                        w2_sb[:, t5 * 32:(t5 + 1) * 32],
                        mp[:, g * DD + t5:g * DD + t5 + CONV2_LEN],
                        start=(t5 == 0), stop=(t5 == 4))
            nc.scalar.activation(y2all[:], y2p[:], RELU, bias=b2_sb[:, 0:1])
            if debug:
                nc.sync.dma_start(DBG_Y2[:], y2all[:])

            h1p = ps1.tile([32, GPC], F32, tag="ps128")
            y2v = y2all[:].rearrange("p (g t) -> p g t", t=CONV2_LEN)
            for t5 in range(CONV2_LEN):
                nc.tensor.matmul(h1p[:], d1_sb[:, t5 * 32:(t5 + 1) * 32],
                                 y2v[:, :, t5:t5 + 1],
                                 start=(t5 == 0), stop=(t5 == CONV2_LEN - 1))
            h1s = smp.tile([32, GPC], F32R, tag="h1s", bufs=1)
            nc.scalar.activation(h1s[:], h1p[:], RELU, bias=bd1_sb[:, 0:1])
            dfp = ps1.tile([2, GPC], F32, tag="ps128")
            nc.tensor.matmul(dfp[:], d2_sb[:], h1s[:], start=True, stop=True)
            pr = smp.tile([2, GPC], F32, tag="pr", bufs=1)
            nc.scalar.activation(pr[:], dfp[:], SIGM, bias=bd2_sb[:, 0:1])
            nc.sync.dma_start(OUT[:], pr[:])

    nc.compile()
    return nc


def _get_nc(fold_lin1, debug):
    key = (fold_lin1, debug)
    if key not in _NC_CACHE:
        _NC_CACHE[key] = _build(fold_lin1, debug)
    return _NC_CACHE[key]


def prepare_host(inputs, fold_lin1=True):
    """All host-side index preprocessing + per-core input maps."""
    x = np.asarray(inputs["x"]).astype(np.int64)
    edge_index = np.asarray(inputs["edge_index"]).astype(np.int64)
    emb = np.ascontiguousarray(np.asarray(inputs["emb"], dtype=np.float32))
    W_convs = np.asarray(inputs["W_convs"], dtype=np.float32)
    conv1_w = np.asarray(inputs["conv1_w"], dtype=np.float32)
    conv1_b = np.asarray(inputs["conv1_b"], dtype=np.float32)
    conv2_w = np.asarray(inputs["conv2_w"], dtype=np.float32)
    conv2_b = np.asarray(inputs["conv2_b"], dtype=np.float32)
    d1_w = np.asarray(inputs["d1_w"], dtype=np.float32)
    d1_b = np.asarray(inputs["d1_b"], dtype=np.float32)
    d2_w = np.asarray(inputs["d2_w"], dtype=np.float32)
    d2_b = np.asarray(inputs["d2_b"], dtype=np.float32)
    W_last = np.asarray(inputs["W_last"], dtype=np.float32)

    src, dst = edge_index[0], edge_index[1]
    deg = (np.bincount(src, minlength=N_TOTAL) + 1).astype(np.float32)
    invdeg = (np.float32(1.0) / deg).astype(np.float32)
    gid = dst >> 9
    flat = (gid * NPG + (dst & 511)) * NPG + (src & 511)
    A = np.bincount(flat, minlength=NUM_GRAPHS * NPG * NPG)
    A = A.astype(np.float32).reshape(NUM_GRAPHS, NPG, NPG)
    idx = np.arange(NPG)
    A[:, idx, idx] += 1.0
    S = A * invdeg.reshape(NUM_GRAPHS, NPG, 1)
    ST = np.ascontiguousarray(S.transpose(0, 2, 1)).reshape(
        NUM_GRAPHS, 4, 128, NPG)

    w1 = np.ascontiguousarray(conv1_w[:, 0, :].T)  # [513, 16]

    # ---- HOT blob [128, 577]: wc 0:512 | w5 512 | idn(bf16) 513:577 ----
    hot = np.zeros((128, HOT_W), np.float32)
    hot[:, 0:512] = W_convs.transpose(1, 0, 2).reshape(128, 512)
    hot[:, 512:513] = W_last
    hot[:, 513:577] = (np.eye(128, dtype=ml_dtypes.bfloat16)
                       .view(np.uint16).view(np.uint32).view(np.float32))

    # ---- COLD blob [128, 3406] ----
    cold = np.zeros((128, COLD_W), np.float32)
    cold[:, 0:2048] = (np.arange(NPG)[None, None, :]
                       < (np.arange(4)[:, None, None] * 128
                          + np.arange(128)[None, :, None])
                       ).astype(np.float32).transpose(1, 0, 2).reshape(128, 2048)
    cold[:, 2048:2112] = np.broadcast_to(
        ((np.arange(K) % 4) * 16 + np.arange(K) // 4)
        .astype(np.float32), (128, K))
    cold[:, 2112:2116] = (np.arange(4)[None, :] * 128
                          + np.arange(128)[:, None]).astype(np.float32)
    cold[:, 2116:2180] = w1[:512].reshape(4, 128, 16).transpose(1, 0, 2) \
        .reshape(128, 64)
    cold[0:1, 2180:2196] = w1[512:513]
    cold[0:16, 2196:2356] = conv2_w.transpose(2, 1, 0) \
        .transpose(1, 0, 2).reshape(16, 160)
    cold[0:32, 2356:3252] = d1_w.reshape(DD, CONV2_LEN * 32)
    cold[0:32, 3252:3254] = (d2_w.astype(np.float64)
                             @ np.array([[1.0, -1.0], [-1.0, 1.0]])
                             ).astype(np.float32)
    cold[0:16, 3254:3382] = (np.arange(128)[None, :] % 16
                             == np.arange(16)[:, None]).astype(np.float32)
    cold[0:64, 3382:3398] = (np.arange(64)[:, None] // 4
                             == np.arange(16)[None, :]).astype(np.float32)
    cold[0:64, 3398:3402] = (np.arange(64)[:, None] % 4
                             == np.arange(4)[None, :]).astype(np.float32)
    cold[0:16, 3402:3403] = conv1_b.reshape(16, 1)
    cold[0:32, 3403:3404] = conv2_b.reshape(32, 1)
    cold[0:32, 3404:3405] = d1_b.reshape(32, 1)
    cold[0:2, 3405:3406] = (np.array([[1.0, -1.0], [-1.0, 1.0]])
                            @ d2_b.reshape(2, 1)).astype(np.float32)

    shared = {"HOT": hot, "COLD": cold}

    if fold_lin1:
        lin1 = emb @ W_convs[0]        # [1000, 128] f32
        h0lin = lin1[x]                # [N, 128]
    h0 = emb[x]

    in_maps = []
    for c in range(NCORES):
        m = dict(shared)
        if fold_lin1:
            loc = h0lin[c * NLOC:(c + 1) * NLOC]  # [4096, 128]
            # [g][p][cc*128+f] = lin1[g*512+cc*128+p, f]
            lm = loc.reshape(GPC, 4, 128, 128).transpose(0, 2, 1, 3)
            m["LIN1M"] = np.ascontiguousarray(
                lm.reshape(GPC, 128, NPG))
        else:
            m["H0T"] = np.ascontiguousarray(h0[c * NLOC:(c + 1) * NLOC].T)
        m["STD"] = np.ascontiguousarray(ST[c * GPC:(c + 1) * GPC])
        in_maps.append(m)
    return in_maps


def run(inputs, fold_lin1=True, debug=False, **spmd_kwargs):
    in_maps = prepare_host(inputs, fold_lin1)
    nc = _get_nc(fold_lin1, debug)
    res = run_bass_kernel_spmd(nc, in_maps, core_ids=list(range(NCORES)),
                               **spmd_kwargs)
    out = np.empty((NUM_GRAPHS, 2), dtype=np.float32)
    for c in range(NCORES):
        out[c * GPC:(c + 1) * GPC, :] = res.results[c]["OUT"].T
    return out, res


def kernel(**inputs):
    out, _ = run(inputs, fold_lin1=True)
    return out



# revision 21
# speedup vs baseline: 1.6234x; 1.1423x over previous
"""DGCNN (gnn_message_passing) Trainium2 Bass kernel, v2.

Strategy (data-parallel over graphs, 8 graphs per NeuronCore):
  - Host builds, per graph, the dense normalized propagation operator
    S[d, s] = (mult(s->d) + I) / deg_out[d]  (512x512 f32), shipped
    transposed as 4 chunks of [128, 512].
  - Layer-1 linear is folded on the host: LIN1 = (emb @ W1)[x] is shipped
    instead of h0 (flag FOLD_LIN1; layer-1 lin matmuls are skipped).
  - Graphs processed in pairs, graph-outer: each pair runs its 4 GCN
    layers on PE while the previous pair's sort-pooling tail runs on
    DVE/ACT/Pool, so the tail is hidden under PE time.
  - Per layer+graph on device: lin = h @ W as 4 chunk matmuls into one
    [128,512] PSUM bank, one copy to SBUF, then msgT = lin^T-chunks
    stationary x S^T chunks -> [128f, 512d] PSUM, tanh -> h (f32
    throughout: the top-64 sort is sensitive to ~1e-8 noise in h5, so
    the whole h chain must be bit-stable f32).
  - Sort-pooling tail per graph: exact stable ranks via comparison
    matrices (DVE/Pool alternating), one-hot rank matrix PT, top-64
    node indices extracted with a tiny iota matmul, converted to the
    wrapped int16 layout, and the pooled features gathered from h with
    gpsimd ap_gather (Pool engine) - no PE transposes or selection
    matmuls.
  - Conv1/maxpool/conv2/dense head batched across the 8 graphs.

Self-contained: hardcodes all shapes; no reads of /root/problem files.
"""

import sys

if "/opt/trn_rl_repo" not in sys.path:
    sys.path.insert(0, "/opt/trn_rl_repo")

import ml_dtypes
import numpy as np

import concourse.bacc as bacc
import concourse.mybir as mybir
import concourse.tile as tile
from concourse.bass_utils import run_bass_kernel_spmd

F32 = mybir.dt.float32
F32R = mybir.dt.float32r  # same bits/numerics as f32; 4x PE rate at >=256 cols
I16 = mybir.dt.int16
BF16 = mybir.dt.bfloat16

NUM_GRAPHS = 64
NPG = 512  # nodes per graph
N_TOTAL = NUM_GRAPHS * NPG
EMB = 128
DIMF = 128
NLAYERS = 4
K = 64
NCORES = 8
GPC = NUM_GRAPHS // NCORES  # graphs per core = 8
NLOC = GPC * NPG  # local nodes = 4096
LATENT = NLAYERS * DIMF + 1  # 513
DD = (K - 2) // 2 + 1  # 32
CONV2_LEN = DD - 5 + 1  # 28

HOT_W = 577     # wc 512 | w5 1 | idn(bf16 packed) 64
COLD_W = 3406   # mj 2048 | ki 64 | iota 4 | w1a 64 | w1b 16 | w2 160 |
                # d1 896 | d2 2 | rm 128 | e1 16 | qm 4 | b1 b2 bd1 bd2 4

_NC_CACHE = {}


def _build(fold_lin1, debug):
    """Trace + compile the per-core Bass program (same on all 8 cores)."""
    nc = bacc.Bacc("TRN2", target_bir_lowering=False, debug=False,
                   num_devices=NCORES)

    # ---- per-core DRAM I/O ----
    # LIN1M[g] = (emb @ W1)[x] for graph g, chunk-major: [g][p][cc*128+f]
    #          = lin1[node cc*128+p of graph g, feat f]
    if fold_lin1:
        LIN1M = nc.dram_tensor("LIN1M", [GPC, 128, NPG], F32R,
                               kind="ExternalInput")
    else:
        H0T = nc.dram_tensor("H0T", [128, NLOC], F32, kind="ExternalInput")
    STD = nc.dram_tensor("STD", [GPC, 4, 128, NPG], F32R, kind="ExternalInput")
    # all small constants packed into two blobs (1 DMA each): HWDGE serial
    # overhead is ~630ns per DMA instruction, so DMA count dominates tiny
    # transfers. Layout documented in prepare_host.
    HOT = nc.dram_tensor("HOT", [128, HOT_W], F32R, kind="ExternalInput")
    COLD = nc.dram_tensor("COLD", [128, COLD_W], F32R, kind="ExternalInput")
    OUT = nc.dram_tensor("OUT", [2, GPC], F32, kind="ExternalOutput")
    if debug:
        DBG_H = nc.dram_tensor("DBG_H", [NLAYERS, 128, NLOC], F32,
                               kind="ExternalOutput")
        DBG_H5 = nc.dram_tensor("DBG_H5", [GPC, 1, NPG], F32,
                                kind="ExternalOutput")
        DBG_RANK = nc.dram_tensor("DBG_RANK", [GPC, 128, 4], F32,
                                  kind="ExternalOutput")
        DBG_IDX = nc.dram_tensor("DBG_IDX", [GPC, 128, 4], F32,
                                 kind="ExternalOutput")
        DBG_POOL = nc.dram_tensor("DBG_POOL", [NLAYERS, 128, GPC * K], F32,
                                  kind="ExternalOutput")
        DBG_POOL5 = nc.dram_tensor("DBG_POOL5", [1, GPC * K], F32,
                                   kind="ExternalOutput")
        DBG_Y2 = nc.dram_tensor("DBG_Y2", [32, GPC * CONV2_LEN], F32,
                                kind="ExternalOutput")

    TANH = mybir.ActivationFunctionType.Tanh
    RELU = mybir.ActivationFunctionType.Relu
    SIGM = mybir.ActivationFunctionType.Sigmoid
    ADD = mybir.AluOpType.add
    MULT = mybir.AluOpType.mult
    MAX = mybir.AluOpType.max
    IS_GT = mybir.AluOpType.is_gt
    IS_EQ = mybir.AluOpType.is_equal

    with tile.TileContext(nc) as tc:
        with (
            tc.tile_pool(name="const", bufs=1) as cp,
            tc.tile_pool(name="hg", bufs=6) as hp,          # [128,2048]/graph
            tc.tile_pool(name="st", bufs=32) as stp,        # [128,512]/chunk
            tc.tile_pool(name="lin", bufs=4) as linp,       # [128,512]
            tc.tile_pool(name="sc", bufs=3) as scp,         # compare scratch
            tc.tile_pool(name="vbp", bufs=3) as vbp,        # v broadcast
            tc.tile_pool(name="ptp", bufs=2) as ptp,        # ptt one-hots
            tc.tile_pool(name="sm", bufs=4) as smp,        # small tiles
            tc.tile_pool(name="idx", bufs=3) as idxp_pool,  # idx wrapped
            tc.tile_pool(name="xs", bufs=20) as xsp,        # endgame sel xts
            tc.tile_pool(name="ps512", bufs=2, space="PSUM") as ps5,
            tc.tile_pool(name="ps128", bufs=2, space="PSUM") as ps1,
            tc.tile_pool(name="psy1", bufs=1, space="PSUM") as psy,
        ):
            # ---- load order: pair-0 working set first ----
            lin1_sb = []

            def load_lin1(g):
                if fold_lin1:
                    t = linp.tile([128, NPG], F32R, tag="lin1", bufs=GPC,
                                  name=f"lin1_{g}")
                    nc.sync.dma_start(t[:], LIN1M[g, :, :])
                    return t
                return None

            st_sb = {}

            def load_st(g, chunked=False):
                # one DMA per graph (chunked for g=0 so the first prop can
                # start after the first quarter lands)
                t = stp.tile([128, 4 * NPG], F32R, tag="st", bufs=8,
                             name=f"st_t{g}")
                if chunked:
                    for c in range(4):
                        nc.sync.dma_start(t[:, c * NPG:(c + 1) * NPG],
                                          STD[g, c])
                else:
                    nc.sync.dma_start(t[:].rearrange("p (c w) -> p c w", c=4),
                                      STD[g].rearrange("c p w -> p c w"))
                st_sb[g] = [t[:, c * NPG:(c + 1) * NPG] for c in range(4)]

            if fold_lin1:
                lin1_sb = [load_lin1(0)]
                load_st(0, chunked=True)
                lin1_sb.append(load_lin1(1))
                load_st(1)
            else:
                h0 = hp.tile([128, NLOC], F32, tag="h0x", bufs=1)
                for s in range(0, 8):
                    nc.sync.dma_start(h0[:, s * 512:(s + 1) * 512],
                                      H0T[:, s * 512:(s + 1) * 512])
                load_st(0)
                load_st(1)
            hot = cp.tile([128, HOT_W], F32R, tag="hot")
            nc.sync.dma_start(hot[:], HOT[:])
            wc_sb = hot[:, 0:512]
            w5_sb = hot[:, 512:513]
            id_sb = hot[:, 513:577].bitcast(BF16)
            if fold_lin1:
                lin1_sb.append(load_lin1(2))
            load_st(2)
            cold = cp.tile([128, COLD_W], F32R, tag="cold")
            nc.sync.dma_start(cold[:], COLD[:])
            mj_sb = cold[:, 0:2048].bitcast(F32)
            ki_sb = cold[:, 2048:2112].bitcast(F32)
            iota_sb = cold[:, 2112:2116]
            w1a_sb = cold[:, 2116:2180]
            w1b_sb = cold[0:1, 2180:2196]
            w2_sb = cold[0:16, 2196:2356]
            d1_sb = cold[0:32, 2356:3252]
            d2_sb = cold[0:32, 3252:3254]
            r_sb = cold[0:16, 3254:3382]
            e1_sb = cold[0:64, 3382:3398]
            qm_sb = cold[0:64, 3398:3402]
            b1_sb = cold[0:16, 3402:3403].bitcast(F32)
            b2_sb = cold[0:32, 3403:3404].bitcast(F32)
            bd1_sb = cold[0:32, 3404:3405].bitcast(F32)
            bd2_sb = cold[0:2, 3405:3406].bitcast(F32)
            for g in range(3, GPC):
                if fold_lin1:
                    lin1_sb.append(load_lin1(g))
                load_st(g)

            # per-graph state
            hgs = {}       # g -> [128, 4*512] tile (layers 1..4)
            vcols = {}     # g -> [128, 4] node-major h5
            vbs = {}       # g -> [128, 512] v broadcast
            ranks = {}     # g -> [128, 4]
            ptts = {}      # g -> [128, 4K] one-hot rank matrix
            idxw = {}      # g -> [128, 4] int16 wrapped indices
            # pooled features, all graphs side by side, one tile per layer
            pooled_sb = [cp.tile([128, GPC * K], F32R, tag=f"pool{l}",
                                 name=f"pool{l}")
                         for l in range(NLAYERS)]
            p5all = cp.tile([16, GPC * K], F32R, tag="p5all")
            y1p = psy.tile([16, GPC * K], F32, tag="y1p")
            y2all = cp.tile([32, GPC * CONV2_LEN], F32R, tag="y2all")
            y1 = cp.tile([16, GPC * K], F32, tag="y1")
            mp = cp.tile([16, GPC * K // 2], F32R, tag="mp")

            # ---------------- layer machinery ----------------
            def lin_stage(g, l):
                # returns SBUF [128, 4*128] chunk-major lin
                if l == 0 and fold_lin1:
                    return lin1_sb[g]
                lp = ps5.tile([128, NPG], F32, tag="linp", bufs=2)
                for cc in range(4):
                    if l == 0:
                        stat = h0[:, g * NPG + cc * 128:
                                  g * NPG + (cc + 1) * 128]
                    else:
                        stat = hgs[g][:, (l - 1) * NPG + cc * 128:
                                      (l - 1) * NPG + (cc + 1) * 128]
                    nc.tensor.matmul(
                        lp[:, cc * 128:(cc + 1) * 128], stat,
                        wc_sb[:, l * 128:(l + 1) * 128],
                        start=True, stop=True)
                ln = linp.tile([128, NPG], F32R, tag="lin")
                # ACT, not DVE: DVE runs multi-us rank-compare bursts and a
                # lin copy queued behind one stalls the next prop on PE
                nc.scalar.copy(ln[:], lp[:])
                return ln

            def prop_stage(g, l, ln):
                sp = ps5.tile([128, NPG], F32, tag="msgp", bufs=2)
                for cc in range(4):
                    nc.tensor.matmul(
                        sp[:], ln[:, cc * 128:(cc + 1) * 128],
                        st_sb[g][cc][:],
                        start=(cc == 0), stop=(cc == 3))
                nc.scalar.activation(
                    hgs[g][:, l * NPG:(l + 1) * NPG], sp[:], TANH)

            def alloc_h(g):
                hgs[g] = hp.tile([128, NLAYERS * NPG], F32R, tag="hg",
                                 name=f"h_{g}")

            def layers_pair(ga, gb):
                """All 4 GCN layers for graphs ga, gb, interleaved."""
                alloc_h(ga)
                alloc_h(gb)
                for l in range(NLAYERS):
                    lns = {g: lin_stage(g, l) for g in (ga, gb)}
                    for g in (ga, gb):
                        prop_stage(g, l, lns[g])
                    yield l

            # ---------------- tail stages ----------------
            lin5s = {}

            def sA(g):
                """layer-5 matvec (PE) + copy (DVE)."""
                hl = hgs[g]
                l5p = ps1.tile([128, 4], F32, tag="ps128")
                for cc in range(4):
                    nc.tensor.matmul(
                        l5p[:, cc:cc + 1],
                        hl[:, 3 * NPG + cc * 128:3 * NPG + (cc + 1) * 128],
                        w5_sb[:], start=True, stop=True)
                lin5 = smp.tile([128, 4], F32R, tag="lin5")
                nc.vector.tensor_copy(lin5[:], l5p[:])
                lin5s[g] = lin5

            def sB(g):
                """msg5 = S @ lin5 (PE, 16 tiny) + tanh (ACT)."""
                lin5 = lin5s[g]
                m5p = ps1.tile([128, 4], F32, tag="ps128")
                for dc in range(4):
                    for sc in range(4):
                        nc.tensor.matmul(
                            m5p[:, dc:dc + 1],
                            st_sb[g][sc][:, dc * 128:(dc + 1) * 128],
                            lin5[:, sc:sc + 1],
                            start=(sc == 0), stop=(sc == 3))
                vcol = smp.tile([128, 4], F32, tag="vcol")
                nc.scalar.activation(vcol[:], m5p[:], TANH)
                vcols[g] = vcol

            def sC(g):
                """h5 row form + broadcast (4 Pool broadcasts, no DMA:
                a DMA costs ~2.3us of semaphore latency on the tail chain)."""
                vcol = vcols[g]
                vtp = ps1.tile([4, 128], F32R, tag="ps128")
                nc.tensor.transpose(vtp[:], vcol[:].bitcast(F32R), id_sb[:])
                vts = smp.tile([4, 128], F32R, tag="vts")
                nc.vector.tensor_copy(vts[:], vtp[:])
                vb = vbp.tile([128, NPG], F32R, tag="vb")
                for cc in range(4):
                    nc.gpsimd.partition_broadcast(
                        vb[:, cc * 128:(cc + 1) * 128], vts[cc:cc + 1, :])
                vbs[g] = vb
                if debug:
                    h5r = smp.tile([1, NPG], F32R, tag="h5r", bufs=3)
                    nc.sync.dma_start(h5r[0:1, :], vts[:])
                    nc.sync.dma_start(DBG_H5[g, :, :], h5r[:].bitcast(F32))

            def _cmp_eng(g):
                # even graphs -> DVE, odd -> Pool: the two graphs of a pair
                # rank concurrently instead of serializing on DVE
                return nc.vector if g % 2 == 0 else nc.gpsimd

            def sD(g):
                """exact stable ranks (engine by graph parity)."""
                vb, vcol = vbs[g], vcols[g]
                eng = _cmp_eng(g)
                rank = smp.tile([128, 4], F32, tag="rank")
                for cc in range(4):
                    t1 = scp.tile([128, NPG], F32, tag="tt")
                    ra = smp.tile([128, 2], F32, tag="ra")
                    eng.tensor_scalar(
                        out=t1[:], in0=vb[:], scalar1=vcol[:, cc:cc + 1],
                        scalar2=None, op0=IS_GT, op1=ADD,
                        accum_out=ra[:, 0:1])
                    t2 = scp.tile([128, NPG], F32, tag="tt")
                    eng.scalar_tensor_tensor(
                        out=t2[:], in0=vb[:], scalar=vcol[:, cc:cc + 1],
                        in1=mj_sb[:, cc * NPG:(cc + 1) * NPG],
                        op0=IS_EQ, op1=MULT, accum_out=ra[:, 1:2])
                    eng.tensor_tensor(
                        out=rank[:, cc:cc + 1], in0=ra[:, 0:1],
                        in1=ra[:, 1:2], op=ADD)
                ranks[g] = rank
                if debug:
                    nc.sync.dma_start(DBG_RANK[g, :, :], rank[:])

            def sE(g):
                """one-hot rank matrix (columns in wrapped-permuted order)."""
                rank = ranks[g]
                eng = _cmp_eng(g)
                ptt = ptp.tile([128, 4 * K], F32R, tag="pt")
                for cc in range(4):
                    eng.tensor_scalar(
                        out=ptt[:, cc * K:(cc + 1) * K], in0=ki_sb[:],
                        scalar1=rank[:, cc:cc + 1], scalar2=None, op0=IS_EQ)
                ptts[g] = ptt

            def sF(g):
                """ordered top-64 node indices, int16 wrapped for ap_gather."""
                ptt = ptts[g]
                # col64[q] = index of the node with rank perm(q); KI's
                # permutation makes the downstream folds land each index at
                # iw[p, s] = idx[s*16+p], the ap_gather wrapped layout.
                cxp = ps1.tile([K, 1], F32, tag="ps128")
                for cc in range(4):
                    nc.tensor.matmul(cxp[:], ptt[:, cc * K:(cc + 1) * K],
                                     iota_sb[:, cc:cc + 1],
                                     start=(cc == 0), stop=(cc == 3))
                c64 = smp.tile([K, 1], F32, tag="c64")
                nc.vector.tensor_copy(c64[:], cxp[:])
                m64 = smp.tile([K, 4], F32R, tag="m64")
                nc.vector.tensor_scalar(out=m64[:], in0=qm_sb[:],
                                        scalar1=c64[:, 0:1], scalar2=None,
                                        op0=MULT)
                wqp = ps1.tile([16, 4], F32, tag="ps128")
                nc.tensor.matmul(wqp[:], e1_sb[:], m64[:],
                                 start=True, stop=True)
                wq = smp.tile([16, 4], F32R, tag="wq")
                nc.vector.tensor_copy(wq[:], wqp[:])
                wfp = ps1.tile([128, 4], F32, tag="ps128")
                nc.tensor.matmul(wfp[:], r_sb[:], wq[:],
                                 start=True, stop=True)
                iw = idxp_pool.tile([128, 4], I16, tag="iw")
                nc.vector.tensor_copy(iw[:], wfp[:])
                idxw[g] = iw
                if debug:
                    dbgi = smp.tile([128, 4], F32, tag="dbgi")
                    nc.vector.tensor_copy(dbgi[:], iw[:])
                    nc.sync.dma_start(DBG_IDX[g, :, :], dbgi[:])

            def sG(g, pool5_on_pe=False):
                """gather pooled features on the Pool engine."""
                iw = idxw[g]
                hl = hgs[g]
                for l in range(NLAYERS):
                    nc.gpsimd.ap_gather(
                        pooled_sb[l][:, g * K:(g + 1) * K],
                        hl[:, l * NPG:(l + 1) * NPG], iw[:],
                        channels=128, num_elems=NPG, d=1, num_idxs=K)
                if pool5_on_pe:
                    # v[idx] via selection matmul; undo the column
                    # permutation with a strided copy out of PSUM.
                    vcol, ptt = vcols[g], ptts[g]
                    p5p = ps1.tile([1, K], F32, tag="ps128")
                    for cc in range(4):
                        nc.tensor.matmul(p5p[:],
                                         vcol[:, cc:cc + 1].bitcast(F32R),
                                         ptt[:, cc * K:(cc + 1) * K],
                                         start=(cc == 0), stop=(cc == 3))
                    dstv = p5all[0:1, g * K:(g + 1) * K].rearrange(
                        "a (s p) -> a p s", p=16)
                    srcv = p5p[0:1, :].rearrange("a (p s) -> a p s", s=4)
                    nc.vector.tensor_copy(dstv, srcv)
                else:
                    nc.gpsimd.ap_gather(
                        p5all[:, g * K:(g + 1) * K], vbs[g][0:16, :],
                        iw[0:16, :], channels=16, num_elems=NPG, d=1,
                        num_idxs=K)

            xts = {}

            def sSelXt(g, alt_eng=None):
                """node-major h chunks via PE transpose (endgame only);
                4 chunk transposes land in one PSUM tile -> one wide copy."""
                hl = hgs[g]
                lst = []
                for l in range(NLAYERS):
                    tg = "linp" if l % 2 == 0 else "msgp"
                    xp = ps5.tile([128, NPG], F32R, tag=tg, bufs=2)
                    for cc in range(4):
                        nc.tensor.transpose(
                            xp[:, cc * 128:(cc + 1) * 128],
                            hl[:, l * NPG + cc * 128:
                               l * NPG + (cc + 1) * 128], id_sb[:])
                    xt = xsp.tile([128, NPG], F32R, tag="xtb", bufs=6)
                    if alt_eng is not None and l % 2 == 1:
                        alt_eng.tensor_copy(xt[:], xp[:])
                    else:
                        nc.scalar.copy(xt[:], xp[:])
                    lst.append(xt)
                xts[g] = lst

            def sSelMM(g):
                """selection matmuls; ptt columns are permuted, so the
                copies to pooled_sb/p5all unpermute via strided views."""
                ptt, vcol = ptts[g], vcols[g]
                for l in range(NLAYERS):
                    tg = "linp" if l % 2 == 0 else "msgp"
                    pp = ps5.tile([128, K], F32, tag=tg, bufs=2)
                    for cc in range(4):
                        nc.tensor.matmul(pp[:],
                                         xts[g][l][:, cc * 128:(cc + 1) * 128],
                                         ptt[:, cc * K:(cc + 1) * K],
                                         start=(cc == 0), stop=(cc == 3))
                    dstv = pooled_sb[l][:, g * K:(g + 1) * K].rearrange(
                        "c (s p) -> c p s", p=16)
                    srcv = pp[:].rearrange("c (p s) -> c p s", s=4)
                    nc.scalar.copy(dstv, srcv)
                p5p = ps1.tile([1, K], F32, tag="ps128")
                for cc in range(4):
                    nc.tensor.matmul(p5p[:],
                                     vcol[:, cc:cc + 1].bitcast(F32R),
                                     ptt[:, cc * K:(cc + 1) * K],
                                     start=(cc == 0), stop=(cc == 3))
                dstv = p5all[0:1, g * K:(g + 1) * K].rearrange(
                    "a (s p) -> a p s", p=16)
                srcv = p5p[0:1, :].rearrange("a (p s) -> a p s", s=4)
                nc.vector.tensor_copy(dstv, srcv)

            def sHa(g):
                """conv1 + relu + maxpool for graph g."""
                for l in range(NLAYERS):
                    nc.tensor.matmul(y1p[:, g * K:(g + 1) * K],
                                     w1a_sb[:, l * 16:(l + 1) * 16],
                                     pooled_sb[l][:, g * K:(g + 1) * K],
                                     start=(l == 0), stop=False)
                nc.tensor.matmul(y1p[:, g * K:(g + 1) * K], w1b_sb[:],
                                 p5all[0:1, g * K:(g + 1) * K],
                                 start=False, stop=True)
                nc.scalar.activation(y1[:, g * K:(g + 1) * K],
                                     y1p[:, g * K:(g + 1) * K], RELU,
                                     bias=b1_sb[:, 0:1])
                y1v = y1[:, g * K:(g + 1) * K].rearrange(
                    "p (a b) -> p a b", b=2)
                nc.vector.tensor_tensor(
                    out=mp[:, g * DD:(g + 1) * DD], in0=y1v[:, :, 0:1],
                    in1=y1v[:, :, 1:2], op=MAX)

            def sHb(g):
                """conv2 + relu for graph g."""
                y2p = ps1.tile([32, CONV2_LEN], F32, tag="ps128")
                for t5 in range(5):
                    nc.tensor.matmul(
                        y2p[:],
                        w2_sb[:, t5 * 32:(t5 + 1) * 32],
                        mp[:, g * DD + t5:g * DD + t5 + CONV2_LEN],
                        start=(t5 == 0), stop=(t5 == 4))
                nc.scalar.activation(
                    y2all[:, g * CONV2_LEN:(g + 1) * CONV2_LEN], y2p[:],
                    RELU, bias=b2_sb[:, 0:1])

            def sH(g):
                """per-graph conv1 + relu + maxpool + conv2 + relu."""
                for l in range(NLAYERS):
                    nc.tensor.matmul(y1p[:, g * K:(g + 1) * K],
                                     w1a_sb[:, l * 16:(l + 1) * 16],
                                     pooled_sb[l][:, g * K:(g + 1) * K],
                                     start=(l == 0), stop=False)
                nc.tensor.matmul(y1p[:, g * K:(g + 1) * K], w1b_sb[:],
                                 p5all[0:1, g * K:(g + 1) * K],
                                 start=False, stop=True)
                nc.scalar.activation(y1[:, g * K:(g + 1) * K],
                                     y1p[:, g * K:(g + 1) * K], RELU,
                                     bias=b1_sb[:, 0:1])
                y1v = y1[:, g * K:(g + 1) * K].rearrange(
                    "p (a b) -> p a b", b=2)
                nc.vector.tensor_tensor(
                    out=mp[:, g * DD:(g + 1) * DD], in0=y1v[:, :, 0:1],
                    in1=y1v[:, :, 1:2], op=MAX)
                y2p = ps1.tile([32, CONV2_LEN], F32, tag="ps128")
                for t5 in range(5):
                    nc.tensor.matmul(
                        y2p[:],
                        w2_sb[:, t5 * 32:(t5 + 1) * 32],
                        mp[:, g * DD + t5:g * DD + t5 + CONV2_LEN],
                        start=(t5 == 0), stop=(t5 == 4))
                nc.scalar.activation(
                    y2all[:, g * CONV2_LEN:(g + 1) * CONV2_LEN], y2p[:],
                    RELU, bias=b2_sb[:, 0:1])


            # ---------------- schedule ----------------
            # pair p: own sA at l3; pair p-1 runs B,C,D,E+F at l0..l3;
            # pair p-2 runs G at l0 and H at l3.
            NP = GPC // 2
            for p in range(NP):
                ga, gb = 2 * p, 2 * p + 1
                gen = layers_pair(ga, gb)
                for l in gen:
                    g1 = (2 * (p - 1), 2 * (p - 1) + 1) if p >= 1 else ()
                    g2 = (2 * (p - 2), 2 * (p - 2) + 1) if p >= 2 else ()
                    if l == 0:
                        for g in g1:
                            sB(g)
                        for g in g2:
                            sG(g)
                    elif l == 1:
                        for g in g1:
                            sC(g)
                    elif l == 2:
                        for g in g1:
                            sD(g)
                    elif l == 3:
                        for g in g1:
                            sE(g)
                        for g in g1:
                            sF(g)
                        if p < NP - 1:
                            for g in g2:
                                sH(g)
                            sA(ga)
                            sA(gb)
                        else:
                            # critical tail chain of the last pair jumps
                            # ahead of the non-critical conv heads in the
                            # ACT/DVE queues
                            sA(ga)
                            sA(gb)
                            sB(ga)
                            sB(gb)
                            sC(ga)
                            sC(gb)
                            for g in g2:
                                sH(g)
                            sSelXt(ga)
            # endgame: pair NP-2 needs G,H; pair NP-1 needs D..H via the
            # selection-matmul path (no F/G). sD/sE split DVE/Pool by
            # parity so the two tails rank concurrently.
            q2 = (2 * (NP - 2), 2 * (NP - 2) + 1)
            q3 = (2 * (NP - 1), 2 * (NP - 1) + 1)
            sD(q3[0])   # DVE
            sD(q3[1])   # Pool
            sE(q3[0])
            sE(q3[1])
            sG(q2[0])   # Pool, queued after sD/sE(q3[1])
            sG(q2[1])
            sSelXt(q3[1])            # PE-ready immediately; copies on ACT
            sSelMM(q3[0])
            sSelMM(q3[1])
            sH(q2[0])
            sH(q2[1])
            sHa(q3[0])
            sHa(q3[1])
            sHb(q3[0])
            sHb(q3[1])

            if debug:
                for l in range(NLAYERS):
                    for g in range(GPC):
                        nc.sync.dma_start(
                            DBG_H[l, :, g * NPG:(g + 1) * NPG],
                            hgs[g][:, l * NPG:(l + 1) * NPG].bitcast(F32))
                    nc.sync.dma_start(DBG_POOL[l, :, :], pooled_sb[l][:].bitcast(F32))
                nc.sync.dma_start(DBG_POOL5[:], p5all[0:1, :].bitcast(F32))

            # ---------------- final head (y2all filled per-graph) ----------
            if debug:
                nc.sync.dma_start(DBG_Y2[:], y2all[:].bitcast(F32))

            h1p = ps1.tile([32, GPC], F32, tag="ps128")
            y2v = y2all[:].rearrange("p (g t) -> p g t", t=CONV2_LEN)
            for t5 in range(CONV2_LEN):
                nc.tensor.matmul(h1p[:], d1_sb[:, t5 * 32:(t5 + 1) * 32],
                                 y2v[:, :, t5:t5 + 1],
                                 start=(t5 == 0), stop=(t5 == CONV2_LEN - 1))
            h1s = smp.tile([32, GPC], F32R, tag="h1s", bufs=1)
            nc.scalar.activation(h1s[:], h1p[:], RELU, bias=bd1_sb[:, 0:1])
            dfp = ps1.tile([2, GPC], F32, tag="ps128")
            nc.tensor.matmul(dfp[:], d2_sb[:], h1s[:], start=True, stop=True)
            pr = smp.tile([2, GPC], F32, tag="pr", bufs=1)
            nc.scalar.activation(pr[:], dfp[:], SIGM, bias=bd2_sb[:, 0:1])
            nc.sync.dma_start(OUT[:], pr[:])

    nc.compile()
    return nc


def _get_nc(fold_lin1, debug):
    key = (fold_lin1, debug)
    if key not in _NC_CACHE:
        _NC_CACHE[key] = _build(fold_lin1, debug)
    return _NC_CACHE[key]


def prepare_host(inputs, fold_lin1=True):
    """All host-side index preprocessing + per-core input maps."""
    x = np.asarray(inputs["x"]).astype(np.int64)
    edge_index = np.asarray(inputs["edge_index"]).astype(np.int64)
    emb = np.ascontiguousarray(np.asarray(inputs["emb"], dtype=np.float32))
    W_convs = np.asarray(inputs["W_convs"], dtype=np.float32)
    conv1_w = np.asarray(inputs["conv1_w"], dtype=np.float32)
    conv1_b = np.asarray(inputs["conv1_b"], dtype=np.float32)
    conv2_w = np.asarray(inputs["conv2_w"], dtype=np.float32)
    conv2_b = np.asarray(inputs["conv2_b"], dtype=np.float32)
    d1_w = np.asarray(inputs["d1_w"], dtype=np.float32)
    d1_b = np.asarray(inputs["d1_b"], dtype=np.float32)
    d2_w = np.asarray(inputs["d2_w"], dtype=np.float32)
    d2_b = np.asarray(inputs["d2_b"], dtype=np.float32)
    W_last = np.asarray(inputs["W_last"], dtype=np.float32)

    src, dst = edge_index[0], edge_index[1]
    deg = (np.bincount(src, minlength=N_TOTAL) + 1).astype(np.float32)
    invdeg = (np.float32(1.0) / deg).astype(np.float32)
    gid = dst >> 9
    flat = (gid * NPG + (dst & 511)) * NPG + (src & 511)
    A = np.bincount(flat, minlength=NUM_GRAPHS * NPG * NPG)
    A = A.astype(np.float32).reshape(NUM_GRAPHS, NPG, NPG)
    idx = np.arange(NPG)
    A[:, idx, idx] += 1.0
    S = A * invdeg.reshape(NUM_GRAPHS, NPG, 1)
    ST = np.ascontiguousarray(S.transpose(0, 2, 1)).reshape(
        NUM_GRAPHS, 4, 128, NPG)

    w1 = np.ascontiguousarray(conv1_w[:, 0, :].T)  # [513, 16]

    # ---- HOT blob [128, 577]: wc 0:512 | w5 512 | idn(bf16) 513:577 ----
    hot = np.zeros((128, HOT_W), np.float32)
    hot[:, 0:512] = W_convs.transpose(1, 0, 2).reshape(128, 512)
    hot[:, 512:513] = W_last
    hot[:, 513:577] = (np.eye(128, dtype=ml_dtypes.bfloat16)
                       .view(np.uint16).view(np.uint32).view(np.float32))

    # ---- COLD blob [128, 3406] ----
    cold = np.zeros((128, COLD_W), np.float32)
    cold[:, 0:2048] = (np.arange(NPG)[None, None, :]
                       < (np.arange(4)[:, None, None] * 128
                          + np.arange(128)[None, :, None])
                       ).astype(np.float32).transpose(1, 0, 2).reshape(128, 2048)
    cold[:, 2048:2112] = np.broadcast_to(
        ((np.arange(K) % 4) * 16 + np.arange(K) // 4)
        .astype(np.float32), (128, K))
    cold[:, 2112:2116] = (np.arange(4)[None, :] * 128
                          + np.arange(128)[:, None]).astype(np.float32)
    cold[:, 2116:2180] = w1[:512].reshape(4, 128, 16).transpose(1, 0, 2) \
        .reshape(128, 64)
    cold[0:1, 2180:2196] = w1[512:513]
    cold[0:16, 2196:2356] = conv2_w.transpose(2, 1, 0) \
        .transpose(1, 0, 2).reshape(16, 160)
    cold[0:32, 2356:3252] = d1_w.reshape(DD, CONV2_LEN * 32)
    cold[0:32, 3252:3254] = (d2_w.astype(np.float64)
                             @ np.array([[1.0, -1.0], [-1.0, 1.0]])
                             ).astype(np.float32)
    cold[0:16, 3254:3382] = (np.arange(128)[None, :] % 16
                             == np.arange(16)[:, None]).astype(np.float32)
    cold[0:64, 3382:3398] = (np.arange(64)[:, None] // 4
                             == np.arange(16)[None, :]).astype(np.float32)
    cold[0:64, 3398:3402] = (np.arange(64)[:, None] % 4
                             == np.arange(4)[None, :]).astype(np.float32)
    cold[0:16, 3402:3403] = conv1_b.reshape(16, 1)
    cold[0:32, 3403:3404] = conv2_b.reshape(32, 1)
    cold[0:32, 3404:3405] = d1_b.reshape(32, 1)
    cold[0:2, 3405:3406] = (np.array([[1.0, -1.0], [-1.0, 1.0]])
                            @ d2_b.reshape(2, 1)).astype(np.float32)

    shared = {"HOT": hot, "COLD": cold}

    if fold_lin1:
        lin1 = emb @ W_convs[0]        # [1000, 128] f32
        h0lin = lin1[x]                # [N, 128]
    h0 = emb[x]

    in_maps = []
    for c in range(NCORES):
        m = dict(shared)
        if fold_lin1:
            loc = h0lin[c * NLOC:(c + 1) * NLOC]  # [4096, 128]
            # [g][p][cc*128+f] = lin1[g*512+cc*128+p, f]
            lm = loc.reshape(GPC, 4, 128, 128).transpose(0, 2, 1, 3)
            m["LIN1M"] = np.ascontiguousarray(
                lm.reshape(GPC, 128, NPG))
        else:
            m["H0T"] = np.ascontiguousarray(h0[c * NLOC:(c + 1) * NLOC].T)
        m["STD"] = np.ascontiguousarray(ST[c * GPC:(c + 1) * GPC])
        in_maps.append(m)
    return in_maps


def run(inputs, fold_lin1=True, debug=False, **spmd_kwargs):
    in_maps = prepare_host(inputs, fold_lin1)
    nc = _get_nc(fold_lin1, debug)
    res = run_bass_kernel_spmd(nc, in_maps, core_ids=list(range(NCORES)),
                               **spmd_kwargs)
    out = np.empty((NUM_GRAPHS, 2), dtype=np.float32)
    for c in range(NCORES):
        out[c * GPC:(c + 1) * GPC, :] = res.results[c]["OUT"].T
    return out, res


def kernel(**inputs):
    out, _ = run(inputs, fold_lin1=True)
    return out



# revision 22
# speedup vs baseline: 1.6277x; 1.0027x over previous
"""DGCNN (gnn_message_passing) Trainium2 Bass kernel, v2.

Strategy (data-parallel over graphs, 8 graphs per NeuronCore):
  - Host builds, per graph, the dense normalized propagation operator
    S[d, s] = (mult(s->d) + I) / deg_out[d]  (512x512 f32), shipped
    transposed as 4 chunks of [128, 512].
  - Layer-1 linear is folded on the host: LIN1 = (emb @ W1)[x] is shipped
    instead of h0 (flag FOLD_LIN1; layer-1 lin matmuls are skipped).
  - Graphs processed in pairs, graph-outer: each pair runs its 4 GCN
    layers on PE while the previous pair's sort-pooling tail runs on
    DVE/ACT/Pool, so the tail is hidden under PE time.
  - Per layer+graph on device: lin = h @ W as 4 chunk matmuls into one
    [128,512] PSUM bank, one copy to SBUF, then msgT = lin^T-chunks
    stationary x S^T chunks -> [128f, 512d] PSUM, tanh -> h (f32
    throughout: the top-64 sort is sensitive to ~1e-8 noise in h5, so
    the whole h chain must be bit-stable f32).
  - Sort-pooling tail per graph: exact stable ranks via comparison
    matrices (DVE/Pool alternating), one-hot rank matrix PT, top-64
    node indices extracted with a tiny iota matmul, converted to the
    wrapped int16 layout, and the pooled features gathered from h with
    gpsimd ap_gather (Pool engine) - no PE transposes or selection
    matmuls.
  - Conv1/maxpool/conv2/dense head batched across the 8 graphs.

Self-contained: hardcodes all shapes; no reads of /root/problem files.
"""

import sys

if "/opt/trn_rl_repo" not in sys.path:
    sys.path.insert(0, "/opt/trn_rl_repo")

import ml_dtypes
import numpy as np

import concourse.bacc as bacc
import concourse.mybir as mybir
import concourse.tile as tile
from concourse.bass_utils import run_bass_kernel_spmd

F32 = mybir.dt.float32
F32R = mybir.dt.float32r  # same bits/numerics as f32; 4x PE rate at >=256 cols
I16 = mybir.dt.int16
BF16 = mybir.dt.bfloat16

NUM_GRAPHS = 64
NPG = 512  # nodes per graph
N_TOTAL = NUM_GRAPHS * NPG
EMB = 128
DIMF = 128
NLAYERS = 4
K = 64
NCORES = 8
GPC = NUM_GRAPHS // NCORES  # graphs per core = 8
NLOC = GPC * NPG  # local nodes = 4096
LATENT = NLAYERS * DIMF + 1  # 513
DD = (K - 2) // 2 + 1  # 32
CONV2_LEN = DD - 5 + 1  # 28

HOT_W = 577     # wc 512 | w5 1 | idn(bf16 packed) 64
COLD_W = 3406   # mj 2048 | ki 64 | iota 4 | w1a 64 | w1b 16 | w2 160 |
                # d1 896 | d2 2 | rm 128 | e1 16 | qm 4 | b1 b2 bd1 bd2 4

_NC_CACHE = {}


def _build(fold_lin1, debug):
    """Trace + compile the per-core Bass program (same on all 8 cores)."""
    nc = bacc.Bacc("TRN2", target_bir_lowering=False, debug=False,
                   num_devices=NCORES)

    # ---- per-core DRAM I/O ----
    # LIN1M[g] = (emb @ W1)[x] for graph g, chunk-major: [g][p][cc*128+f]
    #          = lin1[node cc*128+p of graph g, feat f]
    if fold_lin1:
        LIN1M = nc.dram_tensor("LIN1M", [GPC, 128, NPG], F32R,
                               kind="ExternalInput")
    else:
        H0T = nc.dram_tensor("H0T", [128, NLOC], F32, kind="ExternalInput")
    STD = nc.dram_tensor("STD", [GPC, 4, 128, NPG], F32R, kind="ExternalInput")
    # all small constants packed into two blobs (1 DMA each): HWDGE serial
    # overhead is ~630ns per DMA instruction, so DMA count dominates tiny
    # transfers. Layout documented in prepare_host.
    HOT = nc.dram_tensor("HOT", [128, HOT_W], F32R, kind="ExternalInput")
    COLD = nc.dram_tensor("COLD", [128, COLD_W], F32R, kind="ExternalInput")
    OUT = nc.dram_tensor("OUT", [2, GPC], F32, kind="ExternalOutput")
    if debug:
        DBG_H = nc.dram_tensor("DBG_H", [NLAYERS, 128, NLOC], F32,
                               kind="ExternalOutput")
        DBG_H5 = nc.dram_tensor("DBG_H5", [GPC, 1, NPG], F32,
                                kind="ExternalOutput")
        DBG_RANK = nc.dram_tensor("DBG_RANK", [GPC, 128, 4], F32,
                                  kind="ExternalOutput")
        DBG_IDX = nc.dram_tensor("DBG_IDX", [GPC, 128, 4], F32,
                                 kind="ExternalOutput")
        DBG_POOL = nc.dram_tensor("DBG_POOL", [NLAYERS, 128, GPC * K], F32,
                                  kind="ExternalOutput")
        DBG_POOL5 = nc.dram_tensor("DBG_POOL5", [1, GPC * K], F32,
                                   kind="ExternalOutput")
        DBG_Y2 = nc.dram_tensor("DBG_Y2", [32, GPC * CONV2_LEN], F32,
                                kind="ExternalOutput")

    TANH = mybir.ActivationFunctionType.Tanh
    RELU = mybir.ActivationFunctionType.Relu
    SIGM = mybir.ActivationFunctionType.Sigmoid
    ADD = mybir.AluOpType.add
    MULT = mybir.AluOpType.mult
    MAX = mybir.AluOpType.max
    IS_GT = mybir.AluOpType.is_gt
    IS_EQ = mybir.AluOpType.is_equal

    with tile.TileContext(nc) as tc:
        with (
            tc.tile_pool(name="const", bufs=1) as cp,
            tc.tile_pool(name="hg", bufs=6) as hp,          # [128,2048]/graph
            tc.tile_pool(name="st", bufs=32) as stp,        # [128,512]/chunk
            tc.tile_pool(name="lin", bufs=4) as linp,       # [128,512]
            tc.tile_pool(name="sc", bufs=3) as scp,         # compare scratch
            tc.tile_pool(name="vbp", bufs=3) as vbp,        # v broadcast
            tc.tile_pool(name="ptp", bufs=2) as ptp,        # ptt one-hots
            tc.tile_pool(name="sm", bufs=4) as smp,        # small tiles
            tc.tile_pool(name="idx", bufs=3) as idxp_pool,  # idx wrapped
            tc.tile_pool(name="xs", bufs=20) as xsp,        # endgame sel xts
            tc.tile_pool(name="ps512", bufs=2, space="PSUM") as ps5,
            tc.tile_pool(name="ps128", bufs=2, space="PSUM") as ps1,
            tc.tile_pool(name="psy1", bufs=1, space="PSUM") as psy,
        ):
            # ---- load order: pair-0 working set first ----
            lin1_sb = []

            def load_lin1(g):
                if fold_lin1:
                    t = linp.tile([128, NPG], F32R, tag="lin1", bufs=GPC,
                                  name=f"lin1_{g}")
                    nc.sync.dma_start(t[:], LIN1M[g, :, :])
                    return t
                return None

            st_sb = {}

            def load_st(g, chunked=False):
                # one DMA per graph (chunked for g=0 so the first prop can
                # start after the first quarter lands)
                t = stp.tile([128, 4 * NPG], F32R, tag="st", bufs=8,
                             name=f"st_t{g}")
                if chunked:
                    for c in range(4):
                        nc.sync.dma_start(t[:, c * NPG:(c + 1) * NPG],
                                          STD[g, c])
                else:
                    nc.sync.dma_start(t[:].rearrange("p (c w) -> p c w", c=4),
                                      STD[g].rearrange("c p w -> p c w"))
                st_sb[g] = [t[:, c * NPG:(c + 1) * NPG] for c in range(4)]

            if fold_lin1:
                lin1_sb = [load_lin1(0)]
                load_st(0, chunked=True)
                lin1_sb.append(load_lin1(1))
                load_st(1)
            else:
                h0 = hp.tile([128, NLOC], F32, tag="h0x", bufs=1)
                for s in range(0, 8):
                    nc.sync.dma_start(h0[:, s * 512:(s + 1) * 512],
                                      H0T[:, s * 512:(s + 1) * 512])
                load_st(0)
                load_st(1)
            hot = cp.tile([128, HOT_W], F32R, tag="hot")
            nc.sync.dma_start(hot[:], HOT[:])
            wc_sb = hot[:, 0:512]
            w5_sb = hot[:, 512:513]
            id_sb = hot[:, 513:577].bitcast(BF16)
            if fold_lin1:
                lin1_sb.append(load_lin1(2))
            load_st(2)
            cold = cp.tile([128, COLD_W], F32R, tag="cold")
            nc.sync.dma_start(cold[:], COLD[:])
            mj_sb = cold[:, 0:2048].bitcast(F32)
            ki_sb = cold[:, 2048:2112].bitcast(F32)
            iota_sb = cold[:, 2112:2116]
            w1a_sb = cold[:, 2116:2180]
            w1b_sb = cold[0:1, 2180:2196]
            w2_sb = cold[0:16, 2196:2356]
            d1_sb = cold[0:32, 2356:3252]
            d2_sb = cold[0:32, 3252:3254]
            r_sb = cold[0:16, 3254:3382]
            e1_sb = cold[0:64, 3382:3398]
            qm_sb = cold[0:64, 3398:3402]
            b1_sb = cold[0:16, 3402:3403].bitcast(F32)
            b2_sb = cold[0:32, 3403:3404].bitcast(F32)
            bd1_sb = cold[0:32, 3404:3405].bitcast(F32)
            bd2_sb = cold[0:2, 3405:3406].bitcast(F32)
            for g in range(3, GPC):
                if fold_lin1:
                    lin1_sb.append(load_lin1(g))
                load_st(g)

            # per-graph state
            hgs = {}       # g -> [128, 4*512] tile (layers 1..4)
            vcols = {}     # g -> [128, 4] node-major h5
            vbs = {}       # g -> [128, 512] v broadcast
            ranks = {}     # g -> [128, 4]
            ptts = {}      # g -> [128, 4K] one-hot rank matrix
            idxw = {}      # g -> [128, 4] int16 wrapped indices
            # pooled features, all graphs side by side, one tile per layer
            pooled_sb = [cp.tile([128, GPC * K], F32R, tag=f"pool{l}",
                                 name=f"pool{l}")
                         for l in range(NLAYERS)]
            p5all = cp.tile([16, GPC * K], F32R, tag="p5all")
            y1p = psy.tile([16, GPC * K], F32, tag="y1p")
            y2all = cp.tile([32, GPC * CONV2_LEN], F32R, tag="y2all")
            y1 = cp.tile([16, GPC * K], F32, tag="y1")
            mp = cp.tile([16, GPC * K // 2], F32R, tag="mp")

            # ---------------- layer machinery ----------------
            def lin_stage(g, l):
                # returns SBUF [128, 4*128] chunk-major lin
                if l == 0 and fold_lin1:
                    return lin1_sb[g]
                lp = ps5.tile([128, NPG], F32, tag="linp", bufs=2)
                for cc in range(4):
                    if l == 0:
                        stat = h0[:, g * NPG + cc * 128:
                                  g * NPG + (cc + 1) * 128]
                    else:
                        stat = hgs[g][:, (l - 1) * NPG + cc * 128:
                                      (l - 1) * NPG + (cc + 1) * 128]
                    nc.tensor.matmul(
                        lp[:, cc * 128:(cc + 1) * 128], stat,
                        wc_sb[:, l * 128:(l + 1) * 128],
                        start=True, stop=True)
                ln = linp.tile([128, NPG], F32R, tag="lin")
                # ACT, not DVE: DVE runs multi-us rank-compare bursts and a
                # lin copy queued behind one stalls the next prop on PE
                nc.scalar.copy(ln[:], lp[:])
                return ln

            def prop_stage(g, l, ln):
                sp = ps5.tile([128, NPG], F32, tag="msgp", bufs=2)
                for cc in range(4):
                    nc.tensor.matmul(
                        sp[:], ln[:, cc * 128:(cc + 1) * 128],
                        st_sb[g][cc][:],
                        start=(cc == 0), stop=(cc == 3))
                nc.scalar.activation(
                    hgs[g][:, l * NPG:(l + 1) * NPG], sp[:], TANH)

            def alloc_h(g):
                hgs[g] = hp.tile([128, NLAYERS * NPG], F32R, tag="hg",
                                 name=f"h_{g}")

            def layers_pair(ga, gb):
                """All 4 GCN layers for graphs ga, gb, interleaved."""
                alloc_h(ga)
                alloc_h(gb)
                for l in range(NLAYERS):
                    lns = {g: lin_stage(g, l) for g in (ga, gb)}
                    for g in (ga, gb):
                        prop_stage(g, l, lns[g])
                    yield l

            # ---------------- tail stages ----------------
            lin5s = {}

            def sA(g):
                """layer-5 matvec (PE) + copy (DVE)."""
                hl = hgs[g]
                l5p = ps1.tile([128, 4], F32, tag="ps128")
                for cc in range(4):
                    nc.tensor.matmul(
                        l5p[:, cc:cc + 1],
                        hl[:, 3 * NPG + cc * 128:3 * NPG + (cc + 1) * 128],
                        w5_sb[:], start=True, stop=True)
                lin5 = smp.tile([128, 4], F32R, tag="lin5")
                nc.vector.tensor_copy(lin5[:], l5p[:])
                lin5s[g] = lin5

            def sB(g):
                """msg5 = S @ lin5 (PE, 16 tiny) + tanh (ACT)."""
                lin5 = lin5s[g]
                m5p = ps1.tile([128, 4], F32, tag="ps128")
                for dc in range(4):
                    for sc in range(4):
                        nc.tensor.matmul(
                            m5p[:, dc:dc + 1],
                            st_sb[g][sc][:, dc * 128:(dc + 1) * 128],
                            lin5[:, sc:sc + 1],
                            start=(sc == 0), stop=(sc == 3))
                vcol = smp.tile([128, 4], F32, tag="vcol")
                nc.scalar.activation(vcol[:], m5p[:], TANH)
                vcols[g] = vcol

            def sC(g):
                """h5 row form + broadcast. Four column transposes land the
                row on partition 0 directly (no DMA: a DMA costs ~2.3us of
                semaphore latency on the tail chain; partition_broadcast
                requires a partition-0 source)."""
                vcol = vcols[g]
                vtp = ps1.tile([1, NPG], F32R, tag="ps128")
                for cc in range(4):
                    nc.tensor.matmul(
                        vtp[0:1, cc * 128:(cc + 1) * 128],
                        vcol[:, cc:cc + 1].bitcast(F32R), id_sb[:],
                        start=(cc == 0), stop=(cc == 3), is_transpose=True)
                h5row = smp.tile([1, NPG], F32R, tag="h5r", bufs=3)
                nc.vector.tensor_copy(h5row[:], vtp[:])
                vb = vbp.tile([128, NPG], F32R, tag="vb")
                nc.gpsimd.partition_broadcast(vb[:], h5row[0:1, :])
                vbs[g] = vb
                if debug:
                    nc.sync.dma_start(DBG_H5[g, :, :], h5row[:].bitcast(F32))

            def _cmp_eng(g):
                # even graphs -> DVE, odd -> Pool: the two graphs of a pair
                # rank concurrently instead of serializing on DVE
                return nc.vector if g % 2 == 0 else nc.gpsimd

            def sD(g):
                """exact stable ranks (engine by graph parity)."""
                vb, vcol = vbs[g], vcols[g]
                eng = _cmp_eng(g)
                rank = smp.tile([128, 4], F32, tag="rank")
                for cc in range(4):
                    t1 = scp.tile([128, NPG], F32, tag="tt")
                    ra = smp.tile([128, 2], F32, tag="ra")
                    eng.tensor_scalar(
                        out=t1[:], in0=vb[:], scalar1=vcol[:, cc:cc + 1],
                        scalar2=None, op0=IS_GT, op1=ADD,
                        accum_out=ra[:, 0:1])
                    t2 = scp.tile([128, NPG], F32, tag="tt")
                    eng.scalar_tensor_tensor(
                        out=t2[:], in0=vb[:], scalar=vcol[:, cc:cc + 1],
                        in1=mj_sb[:, cc * NPG:(cc + 1) * NPG],
                        op0=IS_EQ, op1=MULT, accum_out=ra[:, 1:2])
                    eng.tensor_tensor(
                        out=rank[:, cc:cc + 1], in0=ra[:, 0:1],
                        in1=ra[:, 1:2], op=ADD)
                ranks[g] = rank
                if debug:
                    nc.sync.dma_start(DBG_RANK[g, :, :], rank[:])

            def sE(g):
                """one-hot rank matrix (columns in wrapped-permuted order)."""
                rank = ranks[g]
                eng = _cmp_eng(g)
                ptt = ptp.tile([128, 4 * K], F32R, tag="pt")
                for cc in range(4):
                    eng.tensor_scalar(
                        out=ptt[:, cc * K:(cc + 1) * K], in0=ki_sb[:],
                        scalar1=rank[:, cc:cc + 1], scalar2=None, op0=IS_EQ)
                ptts[g] = ptt

            def sF(g):
                """ordered top-64 node indices, int16 wrapped for ap_gather."""
                ptt = ptts[g]
                # col64[q] = index of the node with rank perm(q); KI's
                # permutation makes the downstream folds land each index at
                # iw[p, s] = idx[s*16+p], the ap_gather wrapped layout.
                cxp = ps1.tile([K, 1], F32, tag="ps128")
                for cc in range(4):
                    nc.tensor.matmul(cxp[:], ptt[:, cc * K:(cc + 1) * K],
                                     iota_sb[:, cc:cc + 1],
                                     start=(cc == 0), stop=(cc == 3))
                c64 = smp.tile([K, 1], F32, tag="c64")
                nc.vector.tensor_copy(c64[:], cxp[:])
                m64 = smp.tile([K, 4], F32R, tag="m64")
                nc.vector.tensor_scalar(out=m64[:], in0=qm_sb[:],
                                        scalar1=c64[:, 0:1], scalar2=None,
                                        op0=MULT)
                wqp = ps1.tile([16, 4], F32, tag="ps128")
                nc.tensor.matmul(wqp[:], e1_sb[:], m64[:],
                                 start=True, stop=True)
                wq = smp.tile([16, 4], F32R, tag="wq")
                nc.vector.tensor_copy(wq[:], wqp[:])
                wfp = ps1.tile([128, 4], F32, tag="ps128")
                nc.tensor.matmul(wfp[:], r_sb[:], wq[:],
                                 start=True, stop=True)
                iw = idxp_pool.tile([128, 4], I16, tag="iw")
                nc.vector.tensor_copy(iw[:], wfp[:])
                idxw[g] = iw
                if debug:
                    dbgi = smp.tile([128, 4], F32, tag="dbgi")
                    nc.vector.tensor_copy(dbgi[:], iw[:])
                    nc.sync.dma_start(DBG_IDX[g, :, :], dbgi[:])

            def sG(g, pool5_on_pe=False):
                """gather pooled features on the Pool engine."""
                iw = idxw[g]
                hl = hgs[g]
                for l in range(NLAYERS):
                    nc.gpsimd.ap_gather(
                        pooled_sb[l][:, g * K:(g + 1) * K],
                        hl[:, l * NPG:(l + 1) * NPG], iw[:],
                        channels=128, num_elems=NPG, d=1, num_idxs=K)
                if pool5_on_pe:
                    # v[idx] via selection matmul; undo the column
                    # permutation with a strided copy out of PSUM.
                    vcol, ptt = vcols[g], ptts[g]
                    p5p = ps1.tile([1, K], F32, tag="ps128")
                    for cc in range(4):
                        nc.tensor.matmul(p5p[:],
                                         vcol[:, cc:cc + 1].bitcast(F32R),
                                         ptt[:, cc * K:(cc + 1) * K],
                                         start=(cc == 0), stop=(cc == 3))
                    dstv = p5all[0:1, g * K:(g + 1) * K].rearrange(
                        "a (s p) -> a p s", p=16)
                    srcv = p5p[0:1, :].rearrange("a (p s) -> a p s", s=4)
                    nc.vector.tensor_copy(dstv, srcv)
                else:
                    nc.gpsimd.ap_gather(
                        p5all[:, g * K:(g + 1) * K], vbs[g][0:16, :],
                        iw[0:16, :], channels=16, num_elems=NPG, d=1,
                        num_idxs=K)

            xts = {}

            def sSelXt(g, alt_eng=None):
                """node-major h chunks via PE transpose (endgame only);
                4 chunk transposes land in one PSUM tile -> one wide copy."""
                hl = hgs[g]
                lst = []
                for l in range(NLAYERS):
                    tg = "linp" if l % 2 == 0 else "msgp"
                    xp = ps5.tile([128, NPG], F32R, tag=tg, bufs=2)
                    for cc in range(4):
                        nc.tensor.transpose(
                            xp[:, cc * 128:(cc + 1) * 128],
                            hl[:, l * NPG + cc * 128:
                               l * NPG + (cc + 1) * 128], id_sb[:])
                    xt = xsp.tile([128, NPG], F32R, tag="xtb", bufs=6)
                    if alt_eng is not None and l % 2 == 1:
                        alt_eng.tensor_copy(xt[:], xp[:])
                    else:
                        nc.scalar.copy(xt[:], xp[:])
                    lst.append(xt)
                xts[g] = lst

            def sSelMM(g):
                """selection matmuls; ptt columns are permuted, so the
                copies to pooled_sb/p5all unpermute via strided views."""
                ptt, vcol = ptts[g], vcols[g]
                for l in range(NLAYERS):
                    tg = "linp" if l % 2 == 0 else "msgp"
                    pp = ps5.tile([128, K], F32, tag=tg, bufs=2)
                    for cc in range(4):
                        nc.tensor.matmul(pp[:],
                                         xts[g][l][:, cc * 128:(cc + 1) * 128],
                                         ptt[:, cc * K:(cc + 1) * K],
                                         start=(cc == 0), stop=(cc == 3))
                    dstv = pooled_sb[l][:, g * K:(g + 1) * K].rearrange(
                        "c (s p) -> c p s", p=16)
                    srcv = pp[:].rearrange("c (p s) -> c p s", s=4)
                    nc.scalar.copy(dstv, srcv)
                p5p = ps1.tile([1, K], F32, tag="ps128")
                for cc in range(4):
                    nc.tensor.matmul(p5p[:],
                                     vcol[:, cc:cc + 1].bitcast(F32R),
                                     ptt[:, cc * K:(cc + 1) * K],
                                     start=(cc == 0), stop=(cc == 3))
                dstv = p5all[0:1, g * K:(g + 1) * K].rearrange(
                    "a (s p) -> a p s", p=16)
                srcv = p5p[0:1, :].rearrange("a (p s) -> a p s", s=4)
                nc.vector.tensor_copy(dstv, srcv)

            def sHa(g):
                """conv1 + relu + maxpool for graph g."""
                for l in range(NLAYERS):
                    nc.tensor.matmul(y1p[:, g * K:(g + 1) * K],
                                     w1a_sb[:, l * 16:(l + 1) * 16],
                                     pooled_sb[l][:, g * K:(g + 1) * K],
                                     start=(l == 0), stop=False)
                nc.tensor.matmul(y1p[:, g * K:(g + 1) * K], w1b_sb[:],
                                 p5all[0:1, g * K:(g + 1) * K],
                                 start=False, stop=True)
                nc.scalar.activation(y1[:, g * K:(g + 1) * K],
                                     y1p[:, g * K:(g + 1) * K], RELU,
                                     bias=b1_sb[:, 0:1])
                y1v = y1[:, g * K:(g + 1) * K].rearrange(
                    "p (a b) -> p a b", b=2)
                nc.vector.tensor_tensor(
                    out=mp[:, g * DD:(g + 1) * DD], in0=y1v[:, :, 0:1],
                    in1=y1v[:, :, 1:2], op=MAX)

            def sHb(g):
                """conv2 + relu for graph g."""
                y2p = ps1.tile([32, CONV2_LEN], F32, tag="ps128")
                for t5 in range(5):
                    nc.tensor.matmul(
                        y2p[:],
                        w2_sb[:, t5 * 32:(t5 + 1) * 32],
                        mp[:, g * DD + t5:g * DD + t5 + CONV2_LEN],
                        start=(t5 == 0), stop=(t5 == 4))
                nc.scalar.activation(
                    y2all[:, g * CONV2_LEN:(g + 1) * CONV2_LEN], y2p[:],
                    RELU, bias=b2_sb[:, 0:1])

            def sH(g):
                """per-graph conv1 + relu + maxpool + conv2 + relu."""
                for l in range(NLAYERS):
                    nc.tensor.matmul(y1p[:, g * K:(g + 1) * K],
                                     w1a_sb[:, l * 16:(l + 1) * 16],
                                     pooled_sb[l][:, g * K:(g + 1) * K],
                                     start=(l == 0), stop=False)
                nc.tensor.matmul(y1p[:, g * K:(g + 1) * K], w1b_sb[:],
                                 p5all[0:1, g * K:(g + 1) * K],
                                 start=False, stop=True)
                nc.scalar.activation(y1[:, g * K:(g + 1) * K],
                                     y1p[:, g * K:(g + 1) * K], RELU,
                                     bias=b1_sb[:, 0:1])
                y1v = y1[:, g * K:(g + 1) * K].rearrange(
                    "p (a b) -> p a b", b=2)
                nc.vector.tensor_tensor(
                    out=mp[:, g * DD:(g + 1) * DD], in0=y1v[:, :, 0:1],
                    in1=y1v[:, :, 1:2], op=MAX)
                y2p = ps1.tile([32, CONV2_LEN], F32, tag="ps128")
                for t5 in range(5):
                    nc.tensor.matmul(
                        y2p[:],
                        w2_sb[:, t5 * 32:(t5 + 1) * 32],
                        mp[:, g * DD + t5:g * DD + t5 + CONV2_LEN],
                        start=(t5 == 0), stop=(t5 == 4))
                nc.scalar.activation(
                    y2all[:, g * CONV2_LEN:(g + 1) * CONV2_LEN], y2p[:],
                    RELU, bias=b2_sb[:, 0:1])


            # ---------------- schedule ----------------
            # pair p: own sA at l3; pair p-1 runs B,C,D,E+F at l0..l3;
            # pair p-2 runs G at l0 and H at l3.
            NP = GPC // 2
            for p in range(NP):
                ga, gb = 2 * p, 2 * p + 1
                gen = layers_pair(ga, gb)
                for l in gen:
                    g1 = (2 * (p - 1), 2 * (p - 1) + 1) if p >= 1 else ()
                    g2 = (2 * (p - 2), 2 * (p - 2) + 1) if p >= 2 else ()
                    if l == 0:
                        for g in g1:
                            sB(g)
                        for g in g2:
                            sG(g)
                    elif l == 1:
                        for g in g1:
                            sC(g)
                    elif l == 2:
                        for g in g1:
                            sD(g)
                    elif l == 3:
                        for g in g1:
                            sE(g)
                        for g in g1:
                            sF(g)
                        if p < NP - 1:
                            for g in g2:
                                sH(g)
                            sA(ga)
                            sA(gb)
                        else:
                            # critical tail chain of the last pair jumps
                            # ahead of the non-critical conv heads in the
                            # ACT/DVE queues
                            sA(ga)
                            sA(gb)
                            sB(ga)
                            sB(gb)
                            sC(ga)
                            sC(gb)
                            for g in g2:
                                sH(g)
                            sSelXt(ga)
            # endgame: pair NP-2 needs G,H; pair NP-1 needs D..H via the
            # selection-matmul path (no F/G). sD/sE split DVE/Pool by
            # parity so the two tails rank concurrently.
            q2 = (2 * (NP - 2), 2 * (NP - 2) + 1)
            q3 = (2 * (NP - 1), 2 * (NP - 1) + 1)
            sD(q3[0])   # DVE
            sD(q3[1])   # Pool
            sE(q3[0])
            sE(q3[1])
            sG(q2[0])   # Pool, queued after sD/sE(q3[1])
            sG(q2[1])
            sSelXt(q3[1])            # PE-ready immediately; copies on ACT
            sSelMM(q3[0])
            sSelMM(q3[1])
            sH(q2[0])
            sH(q2[1])
            sHa(q3[0])
            sHa(q3[1])
            sHb(q3[0])
            sHb(q3[1])

            if debug:
                for l in range(NLAYERS):
                    for g in range(GPC):
                        nc.sync.dma_start(
                            DBG_H[l, :, g * NPG:(g + 1) * NPG],
                            hgs[g][:, l * NPG:(l + 1) * NPG].bitcast(F32))
                    nc.sync.dma_start(DBG_POOL[l, :, :], pooled_sb[l][:].bitcast(F32))
                nc.sync.dma_start(DBG_POOL5[:], p5all[0:1, :].bitcast(F32))

            # ---------------- final head (y2all filled per-graph) ----------
            if debug:
                nc.sync.dma_start(DBG_Y2[:], y2all[:].bitcast(F32))

            h1p = ps1.tile([32, GPC], F32, tag="ps128")
            y2v = y2all[:].rearrange("p (g t) -> p g t", t=CONV2_LEN)
            for t5 in range(CONV2_LEN):
                nc.tensor.matmul(h1p[:], d1_sb[:, t5 * 32:(t5 + 1) * 32],
                                 y2v[:, :, t5:t5 + 1],
                                 start=(t5 == 0), stop=(t5 == CONV2_LEN - 1))
            h1s = smp.tile([32, GPC], F32R, tag="h1s", bufs=1)
            nc.scalar.activation(h1s[:], h1p[:], RELU, bias=bd1_sb[:, 0:1])
            dfp = ps1.tile([2, GPC], F32, tag="ps128")
            nc.tensor.matmul(dfp[:], d2_sb[:], h1s[:], start=True, stop=True)
            pr = smp.tile([2, GPC], F32, tag="pr", bufs=1)
            nc.scalar.activation(pr[:], dfp[:], SIGM, bias=bd2_sb[:, 0:1])
            nc.sync.dma_start(OUT[:], pr[:])

    nc.compile()
    return nc


def _get_nc(fold_lin1, debug):
    key = (fold_lin1, debug)
    if key not in _NC_CACHE:
        _NC_CACHE[key] = _build(fold_lin1, debug)
    return _NC_CACHE[key]


def prepare_host(inputs, fold_lin1=True):
    """All host-side index preprocessing + per-core input maps."""
    x = np.asarray(inputs["x"]).astype(np.int64)
    edge_index = np.asarray(inputs["edge_index"]).astype(np.int64)
    emb = np.ascontiguousarray(np.asarray(inputs["emb"], dtype=np.float32))
    W_convs = np.asarray(inputs["W_convs"], dtype=np.float32)
    conv1_w = np.asarray(inputs["conv1_w"], dtype=np.float32)
    conv1_b = np.asarray(inputs["conv1_b"], dtype=np.float32)
    conv2_w = np.asarray(inputs["conv2_w"], dtype=np.float32)
    conv2_b = np.asarray(inputs["conv2_b"], dtype=np.float32)
    d1_w = np.asarray(inputs["d1_w"], dtype=np.float32)
    d1_b = np.asarray(inputs["d1_b"], dtype=np.float32)
    d2_w = np.asarray(inputs["d2_w"], dtype=np.float32)
    d2_b = np.asarray(inputs["d2_b"], dtype=np.float32)
    W_last = np.asarray(inputs["W_last"], dtype=np.float32)

    src, dst = edge_index[0], edge_index[1]
    deg = (np.bincount(src, minlength=N_TOTAL) + 1).astype(np.float32)
    invdeg = (np.float32(1.0) / deg).astype(np.float32)
    gid = dst >> 9
    flat = (gid * NPG + (dst & 511)) * NPG + (src & 511)
    A = np.bincount(flat, minlength=NUM_GRAPHS * NPG * NPG)
    A = A.astype(np.float32).reshape(NUM_GRAPHS, NPG, NPG)
    idx = np.arange(NPG)
    A[:, idx, idx] += 1.0
    S = A * invdeg.reshape(NUM_GRAPHS, NPG, 1)
    ST = np.ascontiguousarray(S.transpose(0, 2, 1)).reshape(
        NUM_GRAPHS, 4, 128, NPG)

    w1 = np.ascontiguousarray(conv1_w[:, 0, :].T)  # [513, 16]

    # ---- HOT blob [128, 577]: wc 0:512 | w5 512 | idn(bf16) 513:577 ----
    hot = np.zeros((128, HOT_W), np.float32)
    hot[:, 0:512] = W_convs.transpose(1, 0, 2).reshape(128, 512)
    hot[:, 512:513] = W_last
    hot[:, 513:577] = (np.eye(128, dtype=ml_dtypes.bfloat16)
                       .view(np.uint16).view(np.uint32).view(np.float32))

    # ---- COLD blob [128, 3406] ----
    cold = np.zeros((128, COLD_W), np.float32)
    cold[:, 0:2048] = (np.arange(NPG)[None, None, :]
                       < (np.arange(4)[:, None, None] * 128
                          + np.arange(128)[None, :, None])
                       ).astype(np.float32).transpose(1, 0, 2).reshape(128, 2048)
    cold[:, 2048:2112] = np.broadcast_to(
        ((np.arange(K) % 4) * 16 + np.arange(K) // 4)
        .astype(np.float32), (128, K))
    cold[:, 2112:2116] = (np.arange(4)[None, :] * 128
                          + np.arange(128)[:, None]).astype(np.float32)
    cold[:, 2116:2180] = w1[:512].reshape(4, 128, 16).transpose(1, 0, 2) \
        .reshape(128, 64)
    cold[0:1, 2180:2196] = w1[512:513]
    cold[0:16, 2196:2356] = conv2_w.transpose(2, 1, 0) \
        .transpose(1, 0, 2).reshape(16, 160)
    cold[0:32, 2356:3252] = d1_w.reshape(DD, CONV2_LEN * 32)
    cold[0:32, 3252:3254] = (d2_w.astype(np.float64)
                             @ np.array([[1.0, -1.0], [-1.0, 1.0]])
                             ).astype(np.float32)
    cold[0:16, 3254:3382] = (np.arange(128)[None, :] % 16
                             == np.arange(16)[:, None]).astype(np.float32)
    cold[0:64, 3382:3398] = (np.arange(64)[:, None] // 4
                             == np.arange(16)[None, :]).astype(np.float32)
    cold[0:64, 3398:3402] = (np.arange(64)[:, None] % 4
                             == np.arange(4)[None, :]).astype(np.float32)
    cold[0:16, 3402:3403] = conv1_b.reshape(16, 1)
    cold[0:32, 3403:3404] = conv2_b.reshape(32, 1)
    cold[0:32, 3404:3405] = d1_b.reshape(32, 1)
    cold[0:2, 3405:3406] = (np.array([[1.0, -1.0], [-1.0, 1.0]])
                            @ d2_b.reshape(2, 1)).astype(np.float32)

    shared = {"HOT": hot, "COLD": cold}

    if fold_lin1:
        lin1 = emb @ W_convs[0]        # [1000, 128] f32
        h0lin = lin1[x]                # [N, 128]
    h0 = emb[x]

    in_maps = []
    for c in range(NCORES):
        m = dict(shared)
        if fold_lin1:
            loc = h0lin[c * NLOC:(c + 1) * NLOC]  # [4096, 128]
            # [g][p][cc*128+f] = lin1[g*512+cc*128+p, f]
            lm = loc.reshape(GPC, 4, 128, 128).transpose(0, 2, 1, 3)
            m["LIN1M"] = np.ascontiguousarray(
                lm.reshape(GPC, 128, NPG))
        else:
            m["H0T"] = np.ascontiguousarray(h0[c * NLOC:(c + 1) * NLOC].T)
        m["STD"] = np.ascontiguousarray(ST[c * GPC:(c + 1) * GPC])
        in_maps.append(m)
    return in_maps


def run(inputs, fold_lin1=True, debug=False, **spmd_kwargs):
    in_maps = prepare_host(inputs, fold_lin1)
    nc = _get_nc(fold_lin1, debug)
    res = run_bass_kernel_spmd(nc, in_maps, core_ids=list(range(NCORES)),
                               **spmd_kwargs)
    out = np.empty((NUM_GRAPHS, 2), dtype=np.float32)
    for c in range(NCORES):
        out[c * GPC:(c + 1) * GPC, :] = res.results[c]["OUT"].T
    return out, res


def kernel(**inputs):
    out, _ = run(inputs, fold_lin1=True)
    return out

